# revision 58
# baseline (speedup 1.0000x reference)
"""DilateBlock kernel for 8x Trainium2 NeuronCores (Bass/Tile).

Data-parallel over batch B=8 (one image per core). Per core, the whole block
(LN1 -> qkv -> 3-dilation 3x3 neighborhood attention -> proj -> residual ->
LN2 -> MLP -> residual) runs in channels-on-partitions layout; spatial shifts
for the attention unfold live on the free dimension of zero-padded (h, w)
planes, packed 4-hbands x 32-channels across partitions.

Key tricks:
  - LayerNorm stats via ones-matmul on PE into a [32, 512] PSUM strip tile
    (chunk index on partitions), stats math runs wide on 32 partitions, and
    the per-token scale/shift rows feed rank-1 PSUM matmuls directly.
  - qkv/fc1 biases folded into the matmuls via a 97th ones-row of the
    LN-applied activations (contract-97 lhsT with a bias row).
  - K/V drained to contiguous staging as single 96-partition copies, then
    band-packed into padded planes by SBUF->SBUF DMAs on the idle DMA rings.
  - QK tap logits reduced over head_dim AND replicated back to all 16
    channel rows in one PE matmul with a static block-ones matrix; exp runs
    full width on Act; softmax denominator and output accumulate across taps
    via gpsimd DMA-accumulate (even/odd partial tiles, merged on DVE).
  - Softmax normalization applied to the attention OUTPUT.
  - Attention output repacked in SBUF (no DRAM roundtrip) for the proj.
"""
import sys
import time

sys.path.insert(0, '/opt/trn_rl_repo')

import numpy as np

# ---- problem constants (hardcoded per contract) ----
B, C, H, W = 8, 96, 128, 128
DILS = (1, 2, 3)
GD = 32                 # channels per dilation branch
HD = 16                 # head dim
NB = 4                  # h-bands packed on partitions
BH = H // NB            # rows per band = 32
N = H * W               # tokens per image
NCHUNK = 32             # token chunks of 512 (4 image rows each)
CH = N // NCHUNK        # 512
PADR = 38               # BH + 6 halo rows
PADC = 136              # W + 8 halo cols (EVEN pitch: enables DVE 2x mode)
EPS = 1e-5
SCALE = HD ** -0.5
MLPH = 384

_cache = {}
import os
_USE_DMA_ACCUM = os.environ.get('KDMA', '0') == '1'


def _patch_tile(tile_mod, bass_mod):
    """Work around this walrus build's 1-sem-wait-per-instruction limit and
    the multi-wait tail drain."""
    from concourse.vector_clock import ScopedClock, VectorClock

    def _drain_and_barrier(self, tick_clock, wait_clock):
        vclock = tick_clock.global_clock
        n = len(vclock)
        idxs = [i for i in range(n) if vclock[i] > 0]
        for i in idxs:
            vec = [0] * n
            vec[i] = vclock[i]
            nop_inst = self.nc.sync.nop(nofuse=True)
            wait_clock.add_sem_waits(nop_inst.ins,
                                     ScopedClock({None: VectorClock(vec)}))
        self.nc.sync.drain()
        self.nc.all_engine_barrier()
        popped = self.nc._tile_sem_poison_stack.pop()
        assert popped is self._sem_poison
        self.nc.clear_and_free_semaphores(list(self.sems.allocated().values()))
        self.nc.all_engine_barrier()

    tile_mod.TileContext._drain_and_barrier = _drain_and_barrier


_ws_counter = [0]


def _split_multi_waits(nc, mybir):
    for fn in nc.m.functions:
        for blk in fn.blocks:
            insts = list(blk.instructions)
            out = []
            changed = False
            for inst in insts:
                si = inst.sync_info
                waits = list(si.on_wait) if si and si.on_wait else []
                if len(waits) > 1:
                    for w in waits[:-1]:
                        _ws_counter[0] += 1
                        out.append(mybir.InstNoOp(
                            name=f"I-ws-{_ws_counter[0]}",
                            engine=inst.engine, ins=[], outs=[],
                            sync_info=mybir.SyncInfo(on_wait=[w], on_update=[])))
                    si.on_wait = [waits[-1]]
                    changed = True
                out.append(inst)
            if changed:
                blk.instructions[:] = out


def _build():
    import concourse.bass as bass
    import concourse.tile as tile
    from concourse import mybir

    _patch_tile(tile, bass)

    f32 = mybir.dt.float32
    f32r = mybir.dt.float32r
    bf16 = mybir.dt.bfloat16
    AF = mybir.ActivationFunctionType
    ALU = mybir.AluOpType

    nc = bass.Bass()

    # ---- DRAM I/O ----
    x_d = nc.dram_tensor("x", (C, H, W), f32, kind="ExternalInput")
    wq_d = nc.dram_tensor("wqkv", (C + 1, 3 * C), f32, kind="ExternalInput")  # lhsT+bias
    wp_d = nc.dram_tensor("wproj", (C, C), f32, kind="ExternalInput")         # lhsT
    pb_d = nc.dram_tensor("projb", (C, 1), f32, kind="ExternalInput")
    w1_d = nc.dram_tensor("w1", (C + 1, MLPH), f32, kind="ExternalInput")     # lhsT+bias
    w2_d = nc.dram_tensor("w2", (MLPH, C), f32, kind="ExternalInput")         # lhsT
    b2_d = nc.dram_tensor("b2", (C, 1), f32, kind="ExternalInput")
    repl_d = nc.dram_tensor("repl", (128, 128), f32, kind="ExternalInput")
    ws_d = nc.dram_tensor("wsum3", (C, 3 * C), f32, kind="ExternalInput")
    ones_d = nc.dram_tensor("onesc", (C, 1), f32, kind="ExternalInput")

    y_d = nc.dram_tensor("y", (C, H, W), f32, kind="ExternalOutput")

    with tile.TileContext(nc) as tc:
        # ---------------- persistent pools ----------------
        wpool = tc.alloc_tile_pool(name="weights", bufs=1)
        wq = wpool.tile([C + 1, 3 * C], f32r)
        nc.sync.dma_start(out=wq, in_=wq_d[:, :].bitcast(f32r))
        wp = wpool.tile([C, C], bf16)
        nc.gpsimd.dma_start(out=wp, in_=wp_d[:, :])     # gpsimd dma casts
        pbt = wpool.tile([1, C], f32r)                  # proj bias as rank-1 lhsT
        nc.sync.dma_start(out=pbt, in_=pb_d[:, :].rearrange("a b -> b a").bitcast(f32r))
        w1 = wpool.tile([C + 1, MLPH], f32r)
        nc.sync.dma_start(out=w1, in_=w1_d[:, :].bitcast(f32r))
        w2 = [wpool.tile([128, C], bf16, tag=f"w2{i}", name=f"w2{i}") for i in range(3)]
        for i in range(3):
            nc.gpsimd.dma_start(out=w2[i], in_=w2_d[128 * i:128 * (i + 1), :])
        b2t = wpool.tile([1, C], f32r)                  # fc2 bias as rank-1 lhsT
        nc.sync.dma_start(out=b2t, in_=b2_d[:, :].rearrange("a b -> b a").bitcast(f32r))
        repl = wpool.tile([128, 128], bf16)
        nc.gpsimd.dma_start(out=repl, in_=repl_d[:, :])
        srepl = wpool.tile([128, 128], bf16)            # repl/16: S accumulation
        nc.scalar.mul(out=srepl, in_=repl, mul=1.0 / HD)
        onescol = wpool.tile([C, 1], f32r)              # stats lhsT [96,1]
        nc.sync.dma_start(out=onescol, in_=ones_d[:, :].bitcast(f32r))
        onescol_b = wpool.tile([C, 1], bf16)            # bf16 variant (bf16 rhs)
        nc.vector.memset(onescol_b, 1.0)
        wsum3 = wpool.tile([C, 3 * C], f32r)            # qkv col-sums at {0,32,64}
        nc.sync.dma_start(out=wsum3, in_=ws_d[:, :].bitcast(f32r))
        ones1x = wpool.tile([1, C], f32r)               # rank-1 lhsT [1,96]
        nc.sync.dma_start(out=ones1x, in_=ones_d[:, :].rearrange("a b -> b a").bitcast(f32r))
        onesrow = wpool.tile([1, CH], f32r)             # static ones row (f32r)
        ones3x = wpool.tile([C, C], f32r)               # ones rows at {0,32,64}
        epst = wpool.tile([128, 1], f32)
        nc.vector.memset(epst, EPS)
        # LN-applied activation tiles with a persistent ones bias row
        xns = [wpool.tile([C + 1, CH], f32r, tag=f"xn{i}", name=f"xn{i}")
               for i in range(2)]
        # f32 scratch (init-only) to produce properly-rounded f32r constants
        initp = tc.alloc_tile_pool(name="initp", bufs=1)
        onesrow_f = initp.tile([1, CH], f32)
        nc.vector.memset(onesrow_f, 1.0)
        nc.vector.tensor_copy(out=onesrow, in_=onesrow_f)
        ones3f = initp.tile([C, C], f32)
        nc.vector.memset(ones3f, 0.0)
        for j in range(3):
            nc.vector.memset(ones3f[32 * j:32 * j + 1, :], 1.0)
        nc.vector.tensor_copy(out=ones3x, in_=ones3f)
        for i in range(2):
            nc.vector.tensor_copy(out=xns[i][C:C + 1, :], in_=onesrow_f)
        initp.release()
        # LN stats rows: rs/nb per chunk on partitions [32, 512]
        # (LN2 pass reuses the same tiles after LN1's readers are done)
        rs1 = wpool.tile([NCHUNK, CH], f32r)
        nb1 = wpool.tile([NCHUNK, CH], f32r)
        rs2, nb2 = rs1, nb1
        # channel-sum / channel-sumsq strips (chunk on partitions)
        ss1 = wpool.tile([NCHUNK, CH], f32)
        sq1 = wpool.tile([NCHUNK, CH], f32)
        ss2, sq2 = ss1, sq1

        # big persistent activation tensors
        apool = tc.alloc_tile_pool(name="acts", bufs=1)
        Qd = [apool.tile([128, BH, W], bf16, tag=f"qd{d}", name=f"qd{d}") for d in range(3)]
        Kp = [apool.tile([128, PADR, PADC], bf16, tag=f"kp{d}", name=f"kp{d}") for d in range(3)]
        Vp = [apool.tile([128, PADR, PADC], bf16, tag=f"vp{d}", name=f"vp{d}") for d in range(3)]

        # zero only the pad strips (interior fully overwritten by repack)
        for d in range(3):
            for t in (Kp[d], Vp[d]):
                nc.gpsimd.memset(t[:, 0:3, :], 0.0)
                nc.gpsimd.memset(t[:, 35:38, :], 0.0)
                nc.gpsimd.memset(t[:, 3:35, 0:3], 0.0)
                nc.gpsimd.memset(t[:, 3:35, 3 + W:PADC], 0.0)

        # ============ shared stats math ============
        # strips ss/sq: [32, CH] SBUF, chunk on partitions.  Runs on row
        # slices [c0:c0+n] right after each flush so downstream chunks can
        # start without waiting for the whole stats pass.  mu overwrites ss,
        # var/sd overwrite sq in place; vtmp holds -mu^2.
        vtmp = wpool.tile([NCHUNK, CH], f32)

        def stats_math(ss, sq, rs, nb, c0, n):
            sl = slice(c0, c0 + n)
            nc.scalar.mul(out=ss[sl, :], in_=ss[sl, :], mul=1.0 / C)
            nc.scalar.mul(out=sq[sl, :], in_=sq[sl, :], mul=1.0 / C)
            nc.vector.scalar_tensor_tensor(out=vtmp[sl, :], in0=ss[sl, :],
                                           scalar=-1.0, in1=ss[sl, :],
                                           op0=ALU.mult, op1=ALU.mult)
            nc.vector.tensor_tensor(out=sq[sl, :], in0=sq[sl, :],
                                    in1=vtmp[sl, :], op=ALU.add)
            nc.scalar.activation(out=sq[sl, :], in_=sq[sl, :], func=AF.Sqrt,
                                 bias=epst[c0:c0 + n, 0:1], scale=1.0)
            with nc.allow_low_precision(reason="f32r-typed LN stats rows"):
                nc.vector.reciprocal(out=rs[sl, :], in_=sq[sl, :])
                nc.vector.scalar_tensor_tensor(out=nb[sl, :], in0=ss[sl, :],
                                               scalar=-1.0,
                                               in1=rs[sl, :].bitcast(f32),
                                               op0=ALU.mult, op1=ALU.mult)

        # strip helper: drain a [1, n*CH] psum strip (partition 0) to a
        # 1-partition SBUF stage, then one DMA reshapes it into rows
        # [c0:c0+n] of the compact [32, CH] stats tiles.
        def strip_flush(pool, ps_s, ps_q, ss, sq, c0, n, qeng=None):
            stg_s = pool.tile([1, n * CH], f32, tag="stg_s")
            nc.scalar.copy(stg_s, ps_s[:, 0:n * CH])
            stg_q = pool.tile([1, n * CH], f32, tag="stg_q")
            if qeng is None:
                nc.scalar.copy(stg_q, ps_q[:, 0:n * CH])
            else:
                qeng.tensor_copy(out=stg_q, in_=ps_q[:, 0:n * CH])
            nc.sync.dma_start(out=ss[c0:c0 + n, :], in_=stg_s)
            nc.sync.dma_start(out=sq[c0:c0 + n, :], in_=stg_q)

        # ============ PH1: LN1 stats sweep ============
        with tc.tile_pool(name="ph1", bufs=2) as pool, \
             tc.tile_pool(name="ph1st", bufs=2) as sgpool, \
             tc.tile_pool(name="ph1ps", bufs=1, space="PSUM") as stps:
            ps_s = ps_q = None
            for g in range(NCHUNK // 4):
                xt4 = pool.tile([C, 4, CH], f32r, tag="xt")
                nc.sync.dma_start(out=xt4,
                                  in_=x_d[:, 16 * g:16 * g + 16, :].bitcast(f32r))
                for i in range(4):
                    c = 4 * g + i
                    if i == 0:
                        ps_s = stps.tile([1, 4 * CH], f32, tag="ps_s")
                        ps_q = stps.tile([1, 4 * CH], f32, tag="ps_q")
                    h = CH * i
                    xt = xt4[:, i, :]
                    nc.tensor.matmul(ps_s[:, h:h + CH], lhsT=onescol,
                                     rhs=xt, start=True, stop=True)
                    xf = xt.bitcast(f32)
                    xsq = pool.tile([C, CH], f32r, tag="xsq")
                    if c % 2 == 0:
                        nc.vector.tensor_tensor(out=xsq, in0=xf, in1=xf, op=ALU.mult)
                    else:
                        nc.gpsimd.tensor_tensor(out=xsq, in0=xf, in1=xf, op=ALU.mult)
                    nc.tensor.matmul(ps_q[:, h:h + CH], lhsT=onescol,
                                     rhs=xsq, start=True, stop=True)
                    if i == 3:
                        strip_flush(sgpool, ps_s, ps_q, ss1, sq1, 4 * g, 4,
                                    qeng=nc.vector)
                        if g == 7:
                            stats_math(ss1, sq1, rs1, nb1, 0, NCHUNK)

        stgpool = tc.alloc_tile_pool(name="stg", bufs=1)
        stg_k = stgpool.tile([C, N], bf16)
        stg_v = stgpool.tile([C, N], bf16)


        # ============ PH2: LN1 apply + qkv + stage/scatter ============
        def band_rows(b):
            lo = max(0, BH * b - 3)
            hi = min(H, BH * b + BH + 3)
            return lo, hi

        # stage rs/nb rows at partitions {0,32,64} so rank-1 matmuls can
        # read them (PE base-partition rule); one strided DMA per 3 chunks
        def stage_stats(sgp, rs, nb, c0):
            n = min(3, NCHUNK - c0)
            srs = sgp.tile([C, CH], f32r, tag="srs")
            snb = sgp.tile([C, CH], f32r, tag="snb")
            dst_rs = srs.rearrange("(a b) f -> a b f", a=3)[0:n, 0:1, :]
            dst_nb = snb.rearrange("(a b) f -> a b f", a=3)[0:n, 0:1, :]
            nc.sync.dma_start(out=dst_rs, in_=rs[c0:c0 + n, :])
            nc.sync.dma_start(out=dst_nb, in_=nb[c0:c0 + n, :])
            return srs, snb

        with tc.tile_pool(name="ph2", bufs=3) as pool, \
             tc.tile_pool(name="ph2t", bufs=2) as tpool, \
             tc.tile_pool(name="ph2sg", bufs=2) as sgp, \
             tc.tile_pool(name="ph2ps", bufs=2, space="PSUM") as psum, \
             tc.tile_pool(name="ph2ps2", bufs=2, space="PSUM") as psum2:
            srs = snb = None
            for c in range(NCHUNK):
                g, i = c // 4, c % 4
                if i == 0:
                    xt4 = pool.tile([C, 4, CH], f32, tag="xt2")
                    nc.sync.dma_start(out=xt4, in_=x_d[:, 16 * g:16 * g + 16, :])
                if c % 3 == 0:
                    srs, snb = stage_stats(sgp, rs1, nb1, c)
                j = c % 3
                xt = xt4[:, i, :]
                pa = psum2.tile([C, CH], f32, tag="pa")
                nc.tensor.matmul(pa, lhsT=ones3x[32 * j:32 * j + 1, :],
                                 rhs=srs[32 * j:32 * j + 1, :],
                                 start=True, stop=True)
                xn = xns[c % 2]
                nc.vector.tensor_tensor(out=xn[0:C, :], in0=xt, in1=pa,
                                        op=ALU.mult)

                pq = psum.tile([C, CH], f32, tag="pq")
                pk = psum.tile([C, CH], f32, tag="pk")
                pv = psum.tile([C, CH], f32, tag="pv")
                snbj = snb[32 * j:32 * j + 1, :]
                for t, pt in enumerate((pq, pk, pv)):
                    nc.tensor.matmul(pt, lhsT=wsum3[32 * j:32 * j + 1,
                                                    C * t:C * (t + 1)],
                                     rhs=snbj, start=True, stop=False)
                    nc.tensor.matmul(pt, lhsT=wq[:, C * t:C * (t + 1)], rhs=xn,
                                     start=False, stop=True)

                # K/V -> contiguous staging (single 96-partition copies);
                # gpsimd cannot touch PSUM, so drains go DVE/Act only
                nc.scalar.copy(stg_k[:, CH * c:CH * (c + 1)], pk)
                if c % 2 == 0:
                    nc.vector.tensor_copy(out=stg_v[:, CH * c:CH * (c + 1)], in_=pv)
                else:
                    nc.scalar.copy(stg_v[:, CH * c:CH * (c + 1)], pv)
                # Q -> band-packed planes directly
                b = c // 8
                r_off = 4 * c - BH * b
                for d in range(3):
                    src = pq[32 * d:32 * d + 32, :].rearrange("p (r w) -> p r w", r=4)
                    dst = Qd[d][32 * b:32 * b + 32, r_off:r_off + 4, :]
                    if d == 0 or (d == 2 and c % 2 == 1):
                        nc.vector.tensor_copy(out=dst, in_=src)
                    else:
                        nc.scalar.copy(dst, src)

                # band-packed K/V repack via SBUF->SBUF DMA on idle rings
                if c in (8, 16, 24, 31):
                    b_ = (c - 1) // 8
                    lo, hi = band_rows(b_)
                    nr = hi - lo
                    r0 = lo - (BH * b_ - 3)
                    for d in range(3):
                        for stg, dstp in ((stg_k, Kp[d]), (stg_v, Vp[d])):
                            nc.sync.dma_start(
                                out=dstp[32 * b_:32 * b_ + 32, r0:r0 + nr, 3:3 + W],
                                in_=stg[32 * d:32 * d + 32, W * lo:W * hi]
                                    .rearrange("p (r w) -> p r w", r=nr))

        stgpool.release()

        # attention output (channel-major), in space freed by the staging
        ofpool = tc.alloc_tile_pool(name="ofp", bufs=1)
        ofull = ofpool.tile([C, N], bf16)

        # ============ PH3: attention per dilation ============
        # Processed in half-planes (16 band-rows each) so the softmax
        # denominator S accumulates across taps in PSUM via PE matmuls with
        # srepl (= repl/16), freeing DVE of the S adds entirely.
        HF = BH * W // 2                                # 2048 tokens per half
        with tc.tile_pool(name="ph3", bufs=2) as pool, \
             tc.tile_pool(name="ph3f", bufs=2) as fpool, \
             tc.tile_pool(name="ph3acc", bufs=2) as acc, \
             tc.tile_pool(name="ph3ps", bufs=2, space="PSUM") as psum, \
             tc.tile_pool(name="ph3sps", bufs=1, space="PSUM") as spsum:
            for di, dil in enumerate(DILS):
                for hh in range(2):
                    S_ps = spsum.tile([128, HF], f32, tag="Sps")
                    Oab = [acc.tile([128, HF], bf16, tag=f"O{p}",
                                    name=f"O{p}_{di}_{hh}") for p in range(2)]
                    rcp = acc.tile([128, HF], bf16, tag="rcp",
                                   name=f"rcp_{di}_{hh}")
                    qv = Qd[di][:, 16 * hh:16 * hh + 16, :]
                    for ti, (dr, dc) in enumerate([(i - 1, j - 1)
                                                   for i in range(3) for j in range(3)]):
                        r0 = 3 + dr * dil + 16 * hh
                        kwin = Kp[di][:, r0:r0 + 16, 3 + dc * dil:3 + dc * dil + W]
                        vwin = Vp[di][:, r0:r0 + 16, 3 + dc * dil:3 + dc * dil + W]
                        P = fpool.tile([128, HF], bf16, tag="P")
                        nc.vector.tensor_tensor(
                            out=P.rearrange("p (r w) -> p r w", r=16),
                            in0=qv, in1=kwin, op=ALU.mult)
                        # logits -> exp (overwrites P) -> S accumulation
                        for q in range(2):
                            pl = psum.tile([128, 1024], f32, tag="pl")
                            for j in range(2):
                                nc.tensor.matmul(
                                    pl[:, 512 * j:512 * (j + 1)], lhsT=repl,
                                    rhs=P[:, 1024 * q + 512 * j:
                                          1024 * q + 512 * (j + 1)],
                                    start=True, stop=True)
                            nc.scalar.activation(
                                out=P[:, 1024 * q:1024 * (q + 1)], in_=pl,
                                func=AF.Exp)
                            for j in range(2):
                                nc.tensor.matmul(
                                    S_ps[:, 1024 * q + 512 * j:
                                         1024 * q + 512 * (j + 1)],
                                    lhsT=srepl,
                                    rhs=P[:, 1024 * q + 512 * j:
                                          1024 * q + 512 * (j + 1)],
                                    start=(ti == 0), stop=(ti == 8))
                        ev = P.rearrange("p (r w) -> p r w", r=16)
                        Pv = Oab[ti] if ti < 2 else pool.tile([128, HF], bf16,
                                                              tag="Pv")
                        nc.vector.tensor_tensor(
                            out=Pv.rearrange("p (r w) -> p r w", r=16),
                            in0=ev, in1=vwin, op=ALU.mult)
                        if ti >= 2:
                            eng = nc.gpsimd if ti in (3, 5, 7) else nc.vector
                            eng.tensor_tensor(out=Oab[ti % 2], in0=Oab[ti % 2],
                                              in1=Pv, op=ALU.add)
                    with nc.allow_low_precision(reason="softmax recip bf16"):
                        nc.vector.reciprocal(out=rcp, in_=S_ps)
                    nc.gpsimd.tensor_tensor(out=Oab[0], in0=Oab[0], in1=Oab[1],
                                             op=ALU.add)
                    nc.vector.tensor_tensor(out=Oab[0], in0=Oab[0], in1=rcp,
                                            op=ALU.mult)
                    for b in range(NB):
                        nc.sync.dma_start(
                            out=ofull[32 * di:32 * di + 32,
                                      4096 * b + 2048 * hh:
                                      4096 * b + 2048 * hh + 2048],
                            in_=Oab[0][32 * b:32 * b + 32, :])

        # ============ PH4: proj + residual + LN2 stats ============
        r1pool = tc.alloc_tile_pool(name="r1p", bufs=1)
        r1 = r1pool.tile([C, N], bf16)
        with tc.tile_pool(name="ph4", bufs=2) as pool, \
             tc.tile_pool(name="ph4sg", bufs=1) as sgpool, \
             tc.tile_pool(name="ph4st", bufs=1, space="PSUM") as stps, \
             tc.tile_pool(name="ph4ps", bufs=2, space="PSUM") as psum:
            ps_s = ps_q = None
            for c in range(NCHUNK):
                g, i = c // 4, c % 4
                if i == 0:
                    xt4 = pool.tile([C, 4, CH], f32, tag="xt4")
                    nc.sync.dma_start(out=xt4, in_=x_d[:, 16 * g:16 * g + 16, :])
                if c % 2 == 0:
                    ps_s = stps.tile([1, 2 * CH], f32, tag="ps_s4")
                    ps_q = stps.tile([1, 2 * CH], f32, tag="ps_q4")
                h4 = CH * (c % 2)
                pp = psum.tile([C, CH], f32, tag="pp")
                nc.tensor.matmul(pp, lhsT=pbt, rhs=onesrow, start=True, stop=False)
                nc.tensor.matmul(pp, lhsT=wp, rhs=ofull[:, CH * c:CH * (c + 1)],
                                 start=False, stop=True)
                rsl = r1[:, CH * c:CH * (c + 1)]
                nc.vector.tensor_tensor(out=rsl, in0=xt4[:, i, :],
                                        in1=pp, op=ALU.add)
                nc.tensor.matmul(ps_s[:, h4:h4 + CH], lhsT=onescol_b,
                                 rhs=rsl, start=True, stop=True)
                xsq = pool.tile([C, CH], f32r, tag="xsq5")
                nc.vector.tensor_tensor(out=xsq, in0=rsl, in1=rsl, op=ALU.mult)
                nc.tensor.matmul(ps_q[:, h4:h4 + CH], lhsT=onescol,
                                 rhs=xsq, start=True, stop=True)
                if c % 2 == 1:
                    strip_flush(sgpool, ps_s, ps_q, ss2, sq2, c - 1, 2)
                    if c == 31:
                        stats_math(ss2, sq2, rs2, nb2, 0, NCHUNK)

        # ============ PH5: MLP + residual ============
        with tc.tile_pool(name="ph5b", bufs=3) as pool, \
             tc.tile_pool(name="ph5h", bufs=2) as hpool, \
             tc.tile_pool(name="ph5y", bufs=2) as ypool, \
             tc.tile_pool(name="ph5sg", bufs=2) as sgp, \
             tc.tile_pool(name="ph5ps", bufs=2, space="PSUM") as psum, \
             tc.tile_pool(name="ph5ps2", bufs=1, space="PSUM") as psum2:
            srs = snb = None
            for c in range(NCHUNK):
                g, i = c // 4, c % 4
                rsl = r1[:, CH * c:CH * (c + 1)]
                if i == 0:
                    yout4 = ypool.tile([C, 4, CH], f32, tag="yout4")
                if c % 3 == 0:
                    srs, snb = stage_stats(sgp, rs2, nb2, c)
                j = c % 3
                pa = psum2.tile([C, CH], f32, tag="pa5")
                nc.tensor.matmul(pa, lhsT=ones3x[32 * j:32 * j + 1, :],
                                 rhs=srs[32 * j:32 * j + 1, :],
                                 start=True, stop=True)
                pb2 = psum2.tile([C, CH], f32, tag="pb5")
                nc.tensor.matmul(pb2, lhsT=ones3x[32 * j:32 * j + 1, :],
                                 rhs=snb[32 * j:32 * j + 1, :],
                                 start=True, stop=True)
                t1 = pool.tile([C, CH], f32, tag="t15")
                nc.vector.tensor_tensor(out=t1, in0=rsl, in1=pa, op=ALU.mult)
                xn = xns[c % 2]
                nc.vector.tensor_tensor(out=xn[0:C, :], in0=t1,
                                        in1=pb2, op=ALU.add)

                h1 = hpool.tile([128, 3, CH], bf16, tag="h1")
                for j in range(3):
                    pf = psum.tile([128, CH], f32, tag="pf")
                    nc.tensor.matmul(pf, lhsT=w1[:, 128 * j:128 * (j + 1)], rhs=xn,
                                     start=True, stop=True)
                    nc.scalar.activation(out=h1[:, j, :], in_=pf, func=AF.Gelu)
                pm = psum.tile([C, CH], f32, tag="pm")
                nc.tensor.matmul(pm, lhsT=b2t, rhs=onesrow, start=True, stop=False)
                for j in range(3):
                    nc.tensor.matmul(pm, lhsT=w2[j],
                                     rhs=h1[:, j, :], start=False, stop=(j == 2))
                nc.vector.tensor_tensor(out=yout4[:, i, :], in0=rsl,
                                        in1=pm, op=ALU.add)
                if i == 3:
                    nc.sync.dma_start(out=y_d[:, 16 * g:16 * g + 16, :], in_=yout4)

        r1pool.release()
        ofpool.release()
        apool.release()
        wpool.release()

    _split_multi_waits(nc, mybir)
    return nc


def _prep_weights(inputs):
    """Host-side weight preparation (fold LN affine, scale, bias rows)."""
    qkv_w = np.asarray(inputs['qkv_w'], np.float32)       # (288, 96)
    proj_w = np.asarray(inputs['proj_w'], np.float32)     # (96, 96)
    proj_b = np.asarray(inputs['proj_b'], np.float32)
    ln1_w = np.asarray(inputs['ln1_w'], np.float32)
    ln1_b = np.asarray(inputs['ln1_b'], np.float32)
    ln2_w = np.asarray(inputs['ln2_w'], np.float32)
    ln2_b = np.asarray(inputs['ln2_b'], np.float32)
    fc1_w = np.asarray(inputs['fc1_w'], np.float32)       # (384, 96)
    fc1_b = np.asarray(inputs['fc1_b'], np.float32)
    fc2_w = np.asarray(inputs['fc2_w'], np.float32)       # (96, 384)
    fc2_b = np.asarray(inputs['fc2_b'], np.float32)

    wq = qkv_w * ln1_w[None, :]                            # (288, 96)
    c0 = qkv_w @ ln1_b                                     # (288,)
    wq[0:C] *= SCALE                                       # scale q rows
    c0[0:C] *= SCALE
    wqb = np.concatenate([wq.T, c0[None, :]], axis=0)      # (97, 288)

    w1 = fc1_w * ln2_w[None, :]
    c1 = fc1_w @ ln2_b + fc1_b
    w1b = np.concatenate([w1.T, c1[None, :]], axis=0)      # (97, 384)

    repl = np.zeros((128, 128), np.float32)
    for b in range(NB):
        for ch in range(GD):
            h0 = (ch // HD) * HD
            repl[32 * b + h0:32 * b + h0 + HD, 32 * b + ch] = 1.0

    wsum3 = np.zeros((C, 3 * C), np.float32)
    for j in range(3):
        wsum3[32 * j, :] = wq.T[0:C, :].sum(axis=0)        # per-column sums

    return {
        'wsum3': wsum3,
        'wqkv': np.ascontiguousarray(wqb),                 # (97, 288) lhsT
        'wproj': np.ascontiguousarray(proj_w.T),           # (96, 96) lhsT
        'projb': proj_b.reshape(-1, 1).astype(np.float32),
        'w1': np.ascontiguousarray(w1b),                   # (97, 384) lhsT
        'w2': np.ascontiguousarray(fc2_w.T),               # (384, 96) lhsT
        'b2': fc2_b.reshape(-1, 1).astype(np.float32),
        'repl': repl,
        'onesc': np.ones((C, 1), np.float32),
    }


def kernel(**inputs):
    from concourse.bass_utils import run_bass_kernel_spmd

    if 'nc' not in _cache:
        t0 = time.time()
        _cache['nc'] = _build()
        print(f"[kernel] built bass module in {time.time() - t0:.1f}s",
              file=sys.stderr)

    nc = _cache['nc']
    wmap = _prep_weights(inputs)
    x = np.asarray(inputs['x'], np.float32)                # (8, 96, 128, 128)

    in_maps = []
    for b in range(B):
        m = {'x': np.ascontiguousarray(x[b])}
        m.update(wmap)
        in_maps.append(m)

    res = run_bass_kernel_spmd(nc, in_maps, core_ids=list(range(B)))
    _cache['last_exec_ns'] = res.exec_time_ns
    out = np.stack([res.results[b]['y'] for b in range(B)], axis=0)
    return out.astype(np.float32)


# revision 71
# speedup vs baseline: 1.0505x; 1.0505x over previous
"""DilateBlock kernel for 8x Trainium2 NeuronCores (Bass/Tile).

Data-parallel over batch B=8 (one image per core). Per core, the whole block
(LN1 -> qkv -> 3-dilation 3x3 neighborhood attention -> proj -> residual ->
LN2 -> MLP -> residual) runs in channels-on-partitions layout; spatial shifts
for the attention unfold live on the free dimension of zero-padded (h, w)
planes, packed 4-hbands x 32-channels across partitions.

Key tricks:
  - LayerNorm stats via ones-matmul on PE into a [32, 512] PSUM strip tile
    (chunk index on partitions), stats math runs wide on 32 partitions, and
    the per-token scale/shift rows feed rank-1 PSUM matmuls directly.
  - qkv/fc1 biases folded into the matmuls via a 97th ones-row of the
    LN-applied activations (contract-97 lhsT with a bias row).
  - K/V drained to contiguous staging as single 96-partition copies, then
    band-packed into padded planes by SBUF->SBUF DMAs on the idle DMA rings.
  - QK tap logits reduced over head_dim AND replicated back to all 16
    channel rows in one PE matmul with a static block-ones matrix; exp runs
    full width on Act; softmax denominator and output accumulate across taps
    via gpsimd DMA-accumulate (even/odd partial tiles, merged on DVE).
  - Softmax normalization applied to the attention OUTPUT.
  - Attention output repacked in SBUF (no DRAM roundtrip) for the proj.
"""
import sys
import time

sys.path.insert(0, '/opt/trn_rl_repo')

import numpy as np

# ---- problem constants (hardcoded per contract) ----
B, C, H, W = 8, 96, 128, 128
DILS = (1, 2, 3)
GD = 32                 # channels per dilation branch
HD = 16                 # head dim
NB = 4                  # h-bands packed on partitions
BH = H // NB            # rows per band = 32
N = H * W               # tokens per image
NCHUNK = 32             # token chunks of 512 (4 image rows each)
CH = N // NCHUNK        # 512
PADR = 38               # BH + 6 halo rows
PADC = 136              # W + 8 halo cols (EVEN pitch: enables DVE 2x mode)
EPS = 1e-5
SCALE = HD ** -0.5
MLPH = 384

_cache = {}
import os
_USE_DMA_ACCUM = os.environ.get('KDMA', '0') == '1'


def _patch_tile(tile_mod, bass_mod):
    """Work around this walrus build's 1-sem-wait-per-instruction limit and
    the multi-wait tail drain."""
    from concourse.vector_clock import ScopedClock, VectorClock

    def _drain_and_barrier(self, tick_clock, wait_clock):
        vclock = tick_clock.global_clock
        n = len(vclock)
        idxs = [i for i in range(n) if vclock[i] > 0]
        for i in idxs:
            vec = [0] * n
            vec[i] = vclock[i]
            nop_inst = self.nc.sync.nop(nofuse=True)
            wait_clock.add_sem_waits(nop_inst.ins,
                                     ScopedClock({None: VectorClock(vec)}))
        self.nc.sync.drain()
        self.nc.all_engine_barrier()
        popped = self.nc._tile_sem_poison_stack.pop()
        assert popped is self._sem_poison
        self.nc.clear_and_free_semaphores(list(self.sems.allocated().values()))
        self.nc.all_engine_barrier()

    tile_mod.TileContext._drain_and_barrier = _drain_and_barrier


_ws_counter = [0]


def _split_multi_waits(nc, mybir):
    for fn in nc.m.functions:
        for blk in fn.blocks:
            insts = list(blk.instructions)
            out = []
            changed = False
            for inst in insts:
                si = inst.sync_info
                waits = list(si.on_wait) if si and si.on_wait else []
                if len(waits) > 1:
                    for w in waits[:-1]:
                        _ws_counter[0] += 1
                        out.append(mybir.InstNoOp(
                            name=f"I-ws-{_ws_counter[0]}",
                            engine=inst.engine, ins=[], outs=[],
                            sync_info=mybir.SyncInfo(on_wait=[w], on_update=[])))
                    si.on_wait = [waits[-1]]
                    changed = True
                out.append(inst)
            if changed:
                blk.instructions[:] = out


def _build():
    import concourse.bass as bass
    import concourse.tile as tile
    from concourse import mybir

    _patch_tile(tile, bass)

    f32 = mybir.dt.float32
    f32r = mybir.dt.float32r
    bf16 = mybir.dt.bfloat16
    AF = mybir.ActivationFunctionType
    ALU = mybir.AluOpType

    nc = bass.Bass()

    # ---- DRAM I/O ----
    x_d = nc.dram_tensor("x", (C, H, W), f32, kind="ExternalInput")
    wq_d = nc.dram_tensor("wqkv", (C + 1, 3 * C), f32, kind="ExternalInput")  # lhsT+bias
    wp_d = nc.dram_tensor("wproj", (C, C), f32, kind="ExternalInput")         # lhsT
    pb_d = nc.dram_tensor("projb", (C, 1), f32, kind="ExternalInput")
    w1_d = nc.dram_tensor("w1", (C + 1, MLPH), f32, kind="ExternalInput")     # lhsT+bias
    w2_d = nc.dram_tensor("w2", (MLPH, C), f32, kind="ExternalInput")         # lhsT
    b2_d = nc.dram_tensor("b2", (C, 1), f32, kind="ExternalInput")
    repl_d = nc.dram_tensor("repl", (128, 128), f32, kind="ExternalInput")
    ws_d = nc.dram_tensor("wsum3", (C, 3 * C), f32, kind="ExternalInput")
    ones_d = nc.dram_tensor("onesc", (C, 1), f32, kind="ExternalInput")

    y_d = nc.dram_tensor("y", (C, H, W), f32, kind="ExternalOutput")

    with tile.TileContext(nc) as tc:
        # ---------------- persistent pools ----------------
        wpool = tc.alloc_tile_pool(name="weights", bufs=1)
        wq = wpool.tile([C + 1, 3 * C], f32r)
        nc.sync.dma_start(out=wq, in_=wq_d[:, :].bitcast(f32r))
        wp = wpool.tile([C, C], bf16)
        nc.gpsimd.dma_start(out=wp, in_=wp_d[:, :])     # gpsimd dma casts
        pbt = wpool.tile([1, C], f32r)                  # proj bias as rank-1 lhsT
        nc.sync.dma_start(out=pbt, in_=pb_d[:, :].rearrange("a b -> b a").bitcast(f32r))
        w1 = wpool.tile([C + 1, MLPH], f32r)
        nc.sync.dma_start(out=w1, in_=w1_d[:, :].bitcast(f32r))
        w2 = [wpool.tile([128, C], bf16, tag=f"w2{i}", name=f"w2{i}") for i in range(3)]
        for i in range(3):
            nc.gpsimd.dma_start(out=w2[i], in_=w2_d[128 * i:128 * (i + 1), :])
        b2t = wpool.tile([1, C], f32r)                  # fc2 bias as rank-1 lhsT
        nc.sync.dma_start(out=b2t, in_=b2_d[:, :].rearrange("a b -> b a").bitcast(f32r))
        repl = wpool.tile([128, 128], bf16)
        nc.gpsimd.dma_start(out=repl, in_=repl_d[:, :])
        srepl = wpool.tile([128, 128], bf16)            # repl/16: S accumulation
        nc.scalar.mul(out=srepl, in_=repl, mul=1.0 / HD)
        onescol = wpool.tile([C, 1], f32r)              # stats lhsT [96,1]
        nc.sync.dma_start(out=onescol, in_=ones_d[:, :].bitcast(f32r))
        onescol_b = wpool.tile([C, 1], bf16)            # bf16 variant (bf16 rhs)
        nc.vector.memset(onescol_b, 1.0)
        wsum3 = wpool.tile([C, 3 * C], f32r)            # qkv col-sums at {0,32,64}
        nc.sync.dma_start(out=wsum3, in_=ws_d[:, :].bitcast(f32r))
        ones1x = wpool.tile([1, C], f32r)               # rank-1 lhsT [1,96]
        nc.sync.dma_start(out=ones1x, in_=ones_d[:, :].rearrange("a b -> b a").bitcast(f32r))
        onesrow = wpool.tile([1, CH], f32r)             # static ones row (f32r)
        ones3x = wpool.tile([C, C], f32r)               # ones rows at {0,32,64}
        epst = wpool.tile([128, 1], f32)
        nc.vector.memset(epst, EPS)
        # LN-applied activation tiles with a persistent ones bias row
        xns = [wpool.tile([C + 1, CH], f32r, tag=f"xn{i}", name=f"xn{i}")
               for i in range(2)]
        # f32 scratch (init-only) to produce properly-rounded f32r constants
        initp = tc.alloc_tile_pool(name="initp", bufs=1)
        onesrow_f = initp.tile([1, CH], f32)
        nc.vector.memset(onesrow_f, 1.0)
        nc.vector.tensor_copy(out=onesrow, in_=onesrow_f)
        ones3f = initp.tile([C, C], f32)
        nc.vector.memset(ones3f, 0.0)
        for j in range(3):
            nc.vector.memset(ones3f[32 * j:32 * j + 1, :], 1.0)
        nc.vector.tensor_copy(out=ones3x, in_=ones3f)
        for i in range(2):
            nc.vector.tensor_copy(out=xns[i][C:C + 1, :], in_=onesrow_f)
        initp.release()
        # LN stats rows: rs/nb per chunk on partitions [32, 512]
        # (LN2 pass reuses the same tiles after LN1's readers are done)
        rs1 = wpool.tile([NCHUNK, CH], f32r)
        nb1 = wpool.tile([NCHUNK, CH], f32r)
        rs2, nb2 = rs1, nb1
        # channel-sum / channel-sumsq strips (chunk on partitions)
        ss1 = wpool.tile([NCHUNK, CH], f32)
        sq1 = wpool.tile([NCHUNK, CH], f32)
        ss2, sq2 = ss1, sq1

        # big persistent activation tensors
        apool = tc.alloc_tile_pool(name="acts", bufs=1)
        Qd = [apool.tile([128, BH, W], bf16, tag=f"qd{d}", name=f"qd{d}") for d in range(3)]
        Kp = [apool.tile([128, PADR, PADC], bf16, tag=f"kp{d}", name=f"kp{d}") for d in range(3)]
        Vp = [apool.tile([128, PADR, PADC], bf16, tag=f"vp{d}", name=f"vp{d}") for d in range(3)]

        # zero only the pad strips (interior fully overwritten by repack)
        for d in range(3):
            for t in (Kp[d], Vp[d]):
                nc.gpsimd.memset(t[:, 0:3, :], 0.0)
                nc.gpsimd.memset(t[:, 35:38, :], 0.0)
                nc.gpsimd.memset(t[:, 3:35, 0:3], 0.0)
                nc.gpsimd.memset(t[:, 3:35, 3 + W:PADC], 0.0)

        # ============ shared stats math ============
        # strips ss/sq: [32, CH] SBUF, chunk on partitions.  Runs on row
        # slices [c0:c0+n] right after each flush so downstream chunks can
        # start without waiting for the whole stats pass.  mu overwrites ss,
        # var/sd overwrite sq in place; vtmp holds -mu^2.
        vtmp = wpool.tile([NCHUNK, CH], f32)

        def stats_math_early(ss, sq, rs, nb):
            sl = slice(0, 8)
            mu = vtmp[sl, :]
            nc.scalar.mul(out=mu, in_=ss[sl, :], mul=1.0 / C)
            t = rs[sl, :].bitcast(f32)
            nc.scalar.mul(out=t, in_=sq[sl, :], mul=1.0 / C)
            nc.vector.scalar_tensor_tensor(out=nb[sl, :].bitcast(f32), in0=mu,
                                           scalar=-1.0, in1=mu,
                                           op0=ALU.mult, op1=ALU.mult)
            nc.vector.tensor_tensor(out=t, in0=t, in1=nb[sl, :].bitcast(f32),
                                    op=ALU.add)
            nc.scalar.activation(out=t, in_=t, func=AF.Sqrt,
                                 bias=epst[0:8, 0:1], scale=1.0)
            with nc.allow_low_precision(reason="f32r-typed LN stats rows"):
                nc.vector.reciprocal(out=rs[sl, :], in_=t)
                nc.vector.scalar_tensor_tensor(out=nb[sl, :], in0=mu,
                                               scalar=-1.0,
                                               in1=rs[sl, :].bitcast(f32),
                                               op0=ALU.mult, op1=ALU.mult)

        def stats_math(ss, sq, rs, nb, c0, n):
            sl = slice(c0, c0 + n)
            nc.scalar.mul(out=ss[sl, :], in_=ss[sl, :], mul=1.0 / C)
            nc.scalar.mul(out=sq[sl, :], in_=sq[sl, :], mul=1.0 / C)
            nc.vector.scalar_tensor_tensor(out=vtmp[sl, :], in0=ss[sl, :],
                                           scalar=-1.0, in1=ss[sl, :],
                                           op0=ALU.mult, op1=ALU.mult)
            nc.vector.tensor_tensor(out=sq[sl, :], in0=sq[sl, :],
                                    in1=vtmp[sl, :], op=ALU.add)
            nc.scalar.activation(out=sq[sl, :], in_=sq[sl, :], func=AF.Sqrt,
                                 bias=epst[c0:c0 + n, 0:1], scale=1.0)
            with nc.allow_low_precision(reason="f32r-typed LN stats rows"):
                nc.vector.reciprocal(out=rs[sl, :], in_=sq[sl, :])
                nc.vector.scalar_tensor_tensor(out=nb[sl, :], in0=ss[sl, :],
                                               scalar=-1.0,
                                               in1=rs[sl, :].bitcast(f32),
                                               op0=ALU.mult, op1=ALU.mult)

        # strip helper: drain a [1, n*CH] psum strip (partition 0) to a
        # 1-partition SBUF stage, then one DMA reshapes it into rows
        # [c0:c0+n] of the compact [32, CH] stats tiles.
        def strip_flush(pool, ps_s, ps_q, ss, sq, c0, n, qeng=None):
            stg_s = pool.tile([1, n * CH], f32, tag="stg_s")
            nc.scalar.copy(stg_s, ps_s[:, 0:n * CH])
            stg_q = pool.tile([1, n * CH], f32, tag="stg_q")
            if qeng is None:
                nc.scalar.copy(stg_q, ps_q[:, 0:n * CH])
            else:
                qeng.tensor_copy(out=stg_q, in_=ps_q[:, 0:n * CH])
            nc.sync.dma_start(out=ss[c0:c0 + n, :], in_=stg_s)
            nc.sync.dma_start(out=sq[c0:c0 + n, :], in_=stg_q)

        # ============ PH1: LN1 stats sweep ============
        with tc.tile_pool(name="ph1", bufs=3) as pool, \
             tc.tile_pool(name="ph1st", bufs=2) as sgpool, \
             tc.tile_pool(name="ph1ps", bufs=2, space="PSUM") as stps:
            ps_s = ps_q = None
            for g in range(NCHUNK // 4):
                xt4 = pool.tile([C, 4, CH], f32r, tag="xt")
                nc.sync.dma_start(out=xt4,
                                  in_=x_d[:, 16 * g:16 * g + 16, :].bitcast(f32r))
                for i in range(4):
                    c = 4 * g + i
                    if i % 2 == 0:
                        ps_s = stps.tile([1, 2 * CH], f32, tag="ps_s")
                        ps_q = stps.tile([1, 2 * CH], f32, tag="ps_q")
                    h = CH * (i % 2)
                    xt = xt4[:, i, :]
                    nc.tensor.matmul(ps_s[:, h:h + CH], lhsT=onescol,
                                     rhs=xt, start=True, stop=True)
                    xf = xt.bitcast(f32)
                    xsq = pool.tile([C, CH], f32r, tag="xsq")
                    if c % 2 == 0:
                        nc.vector.tensor_tensor(out=xsq, in0=xf, in1=xf, op=ALU.mult)
                    else:
                        nc.gpsimd.tensor_tensor(out=xsq, in0=xf, in1=xf, op=ALU.mult)
                    nc.tensor.matmul(ps_q[:, h:h + CH], lhsT=onescol,
                                     rhs=xsq, start=True, stop=True)
                    if i % 2 == 1:
                        strip_flush(sgpool, ps_s, ps_q, ss1, sq1, c - 1, 2,
                                    qeng=nc.vector if g % 2 == 0 else None)
                    if i == 3:
                        if g == 7:
                            stats_math(ss1, sq1, rs1, nb1, 0, NCHUNK)

        stgpool = tc.alloc_tile_pool(name="stg", bufs=1)
        stg_k = stgpool.tile([C, N], bf16)
        stg_v = stgpool.tile([C, N], bf16)


        # ============ PH2: LN1 apply + qkv + stage/scatter ============
        def band_rows(b):
            lo = max(0, BH * b - 3)
            hi = min(H, BH * b + BH + 3)
            return lo, hi

        # stage rs/nb rows at partitions {0,32,64} so rank-1 matmuls can
        # read them (PE base-partition rule); one strided DMA per 3 chunks
        def stage_stats(sgp, rs, nb, c0):
            n = min(3, NCHUNK - c0)
            srs = sgp.tile([C, CH], f32r, tag="srs")
            snb = sgp.tile([C, CH], f32r, tag="snb")
            dst_rs = srs.rearrange("(a b) f -> a b f", a=3)[0:n, 0:1, :]
            dst_nb = snb.rearrange("(a b) f -> a b f", a=3)[0:n, 0:1, :]
            nc.sync.dma_start(out=dst_rs, in_=rs[c0:c0 + n, :])
            nc.sync.dma_start(out=dst_nb, in_=nb[c0:c0 + n, :])
            return srs, snb

        with tc.tile_pool(name="ph2", bufs=3) as pool, \
             tc.tile_pool(name="ph2t", bufs=2) as tpool, \
             tc.tile_pool(name="ph2sg", bufs=2) as sgp, \
             tc.tile_pool(name="ph2ps", bufs=2, space="PSUM") as psum, \
             tc.tile_pool(name="ph2ps2", bufs=2, space="PSUM") as psum2:
            srs = snb = None
            for c in range(NCHUNK):
                g, i = c // 4, c % 4
                if i == 0:
                    xt4 = pool.tile([C, 4, CH], f32, tag="xt2")
                    nc.sync.dma_start(out=xt4, in_=x_d[:, 16 * g:16 * g + 16, :])
                if c % 3 == 0:
                    srs, snb = stage_stats(sgp, rs1, nb1, c)
                j = c % 3
                xt = xt4[:, i, :]
                pa = psum2.tile([C, CH], f32, tag="pa")
                nc.tensor.matmul(pa, lhsT=ones3x[32 * j:32 * j + 1, :],
                                 rhs=srs[32 * j:32 * j + 1, :],
                                 start=True, stop=True)
                xn = xns[c % 2]
                nc.vector.tensor_tensor(out=xn[0:C, :], in0=xt, in1=pa,
                                        op=ALU.mult)

                pq = psum.tile([C, CH], f32, tag="pq")
                pk = psum.tile([C, CH], f32, tag="pk")
                pv = psum.tile([C, CH], f32, tag="pv")
                snbj = snb[32 * j:32 * j + 1, :]
                for t, pt in enumerate((pq, pk, pv)):
                    nc.tensor.matmul(pt, lhsT=wsum3[32 * j:32 * j + 1,
                                                    C * t:C * (t + 1)],
                                     rhs=snbj, start=True, stop=False)
                    nc.tensor.matmul(pt, lhsT=wq[:, C * t:C * (t + 1)], rhs=xn,
                                     start=False, stop=True)

                # K/V -> contiguous staging (single 96-partition copies);
                # gpsimd cannot touch PSUM, so drains go DVE/Act only
                nc.scalar.copy(stg_k[:, CH * c:CH * (c + 1)], pk)
                if c % 2 == 0:
                    nc.vector.tensor_copy(out=stg_v[:, CH * c:CH * (c + 1)], in_=pv)
                else:
                    nc.scalar.copy(stg_v[:, CH * c:CH * (c + 1)], pv)
                # Q -> band-packed planes directly
                b = c // 8
                r_off = 4 * c - BH * b
                for d in range(3):
                    src = pq[32 * d:32 * d + 32, :].rearrange("p (r w) -> p r w", r=4)
                    dst = Qd[d][32 * b:32 * b + 32, r_off:r_off + 4, :]
                    if d == 0 or (d == 2 and c % 2 == 1):
                        nc.vector.tensor_copy(out=dst, in_=src)
                    else:
                        nc.scalar.copy(dst, src)

                # band-packed K/V repack via SBUF->SBUF DMA on idle rings
                if c in (8, 16, 24, 31):
                    b_ = (c - 1) // 8
                    lo, hi = band_rows(b_)
                    nr = hi - lo
                    r0 = lo - (BH * b_ - 3)
                    for d in range(3):
                        for stg, dstp in ((stg_k, Kp[d]), (stg_v, Vp[d])):
                            nc.sync.dma_start(
                                out=dstp[32 * b_:32 * b_ + 32, r0:r0 + nr, 3:3 + W],
                                in_=stg[32 * d:32 * d + 32, W * lo:W * hi]
                                    .rearrange("p (r w) -> p r w", r=nr))

        stgpool.release()

        # attention output (channel-major), in space freed by the staging
        ofpool = tc.alloc_tile_pool(name="ofp", bufs=1)
        ofull = ofpool.tile([C, N], bf16)

        # ============ PH3: attention per dilation ============
        # Processed in half-planes (16 band-rows each) so the softmax
        # denominator S accumulates across taps in PSUM via PE matmuls with
        # srepl (= repl/16), freeing DVE of the S adds entirely.
        HF = BH * W // 2                                # 2048 tokens per half
        with tc.tile_pool(name="ph3", bufs=4) as pool, \
             tc.tile_pool(name="ph3f", bufs=4) as fpool, \
             tc.tile_pool(name="ph3acc", bufs=2) as acc, \
             tc.tile_pool(name="ph3ps", bufs=2, space="PSUM") as psum, \
             tc.tile_pool(name="ph3sps", bufs=1, space="PSUM") as spsum:
            for di, dil in enumerate(DILS):
                for hh in range(2):
                    S_ps = spsum.tile([128, HF], f32, tag="Sps")
                    Oab = [acc.tile([128, HF], bf16, tag=f"O{p}",
                                    name=f"O{p}_{di}_{hh}") for p in range(2)]
                    rcp = acc.tile([128, HF], bf16, tag="rcp",
                                   name=f"rcp_{di}_{hh}")
                    qv = Qd[di][:, 16 * hh:16 * hh + 16, :]
                    for ti, (dr, dc) in enumerate([(i - 1, j - 1)
                                                   for i in range(3) for j in range(3)]):
                        r0 = 3 + dr * dil + 16 * hh
                        kwin = Kp[di][:, r0:r0 + 16, 3 + dc * dil:3 + dc * dil + W]
                        vwin = Vp[di][:, r0:r0 + 16, 3 + dc * dil:3 + dc * dil + W]
                        P = fpool.tile([128, HF], bf16, tag="P")
                        nc.vector.tensor_tensor(
                            out=P.rearrange("p (r w) -> p r w", r=16),
                            in0=qv, in1=kwin, op=ALU.mult)
                        # logits -> exp (overwrites P) -> S accumulation
                        for q in range(2):
                            pl = psum.tile([128, 1024], f32, tag="pl")
                            for j in range(2):
                                nc.tensor.matmul(
                                    pl[:, 512 * j:512 * (j + 1)], lhsT=repl,
                                    rhs=P[:, 1024 * q + 512 * j:
                                          1024 * q + 512 * (j + 1)],
                                    start=True, stop=True)
                            nc.scalar.activation(
                                out=P[:, 1024 * q:1024 * (q + 1)], in_=pl,
                                func=AF.Exp)
                            for j in range(2):
                                nc.tensor.matmul(
                                    S_ps[:, 1024 * q + 512 * j:
                                         1024 * q + 512 * (j + 1)],
                                    lhsT=srepl,
                                    rhs=P[:, 1024 * q + 512 * j:
                                          1024 * q + 512 * (j + 1)],
                                    start=(ti == 0), stop=(ti == 8))
                        ev = P.rearrange("p (r w) -> p r w", r=16)
                        Pv = Oab[ti] if ti < 2 else pool.tile([128, HF], bf16,
                                                              tag="Pv")
                        nc.vector.tensor_tensor(
                            out=Pv.rearrange("p (r w) -> p r w", r=16),
                            in0=ev, in1=vwin, op=ALU.mult)
                        if ti >= 2:
                            eng = nc.gpsimd if ti in (3, 5, 7) else nc.vector
                            eng.tensor_tensor(out=Oab[ti % 2], in0=Oab[ti % 2],
                                              in1=Pv, op=ALU.add)
                    with nc.allow_low_precision(reason="softmax recip bf16"):
                        nc.vector.reciprocal(out=rcp, in_=S_ps)
                    nc.gpsimd.tensor_tensor(out=Oab[0], in0=Oab[0], in1=Oab[1],
                                             op=ALU.add)
                    nc.vector.tensor_tensor(out=Oab[0], in0=Oab[0], in1=rcp,
                                            op=ALU.mult)
                    for b in range(NB):
                        nc.sync.dma_start(
                            out=ofull[32 * di:32 * di + 32,
                                      4096 * b + 2048 * hh:
                                      4096 * b + 2048 * hh + 2048],
                            in_=Oab[0][32 * b:32 * b + 32, :])

        # ============ PH4: proj + residual + LN2 stats ============
        r1pool = tc.alloc_tile_pool(name="r1p", bufs=1)
        r1 = r1pool.tile([C, N], bf16)
        with tc.tile_pool(name="ph4", bufs=2) as pool, \
             tc.tile_pool(name="ph4sg", bufs=1) as sgpool, \
             tc.tile_pool(name="ph4st", bufs=1, space="PSUM") as stps, \
             tc.tile_pool(name="ph4ps", bufs=2, space="PSUM") as psum:
            ps_s = ps_q = None
            for c in range(NCHUNK):
                g, i = c // 4, c % 4
                if i == 0:
                    xt4 = pool.tile([C, 4, CH], f32, tag="xt4")
                    nc.sync.dma_start(out=xt4, in_=x_d[:, 16 * g:16 * g + 16, :])
                if c % 2 == 0:
                    ps_s = stps.tile([1, 2 * CH], f32, tag="ps_s4")
                    ps_q = stps.tile([1, 2 * CH], f32, tag="ps_q4")
                h4 = CH * (c % 2)
                pp = psum.tile([C, CH], f32, tag="pp")
                nc.tensor.matmul(pp, lhsT=pbt, rhs=onesrow, start=True, stop=False)
                nc.tensor.matmul(pp, lhsT=wp, rhs=ofull[:, CH * c:CH * (c + 1)],
                                 start=False, stop=True)
                rsl = r1[:, CH * c:CH * (c + 1)]
                nc.vector.tensor_tensor(out=rsl, in0=xt4[:, i, :],
                                        in1=pp, op=ALU.add)
                nc.tensor.matmul(ps_s[:, h4:h4 + CH], lhsT=onescol_b,
                                 rhs=rsl, start=True, stop=True)
                xsq = pool.tile([C, CH], f32r, tag="xsq5")
                nc.vector.tensor_tensor(out=xsq, in0=rsl, in1=rsl, op=ALU.mult)
                nc.tensor.matmul(ps_q[:, h4:h4 + CH], lhsT=onescol,
                                 rhs=xsq, start=True, stop=True)
                if c % 2 == 1:
                    strip_flush(sgpool, ps_s, ps_q, ss2, sq2, c - 1, 2)
                    if c == 31:
                        stats_math(ss2, sq2, rs2, nb2, 0, NCHUNK)

        # ============ PH5: MLP + residual ============
        with tc.tile_pool(name="ph5b", bufs=3) as pool, \
             tc.tile_pool(name="ph5h", bufs=2) as hpool, \
             tc.tile_pool(name="ph5y", bufs=2) as ypool, \
             tc.tile_pool(name="ph5sg", bufs=2) as sgp, \
             tc.tile_pool(name="ph5ps", bufs=2, space="PSUM") as psum, \
             tc.tile_pool(name="ph5ps2", bufs=1, space="PSUM") as psum2:
            srs = snb = None
            for c in range(NCHUNK):
                g, i = c // 4, c % 4
                rsl = r1[:, CH * c:CH * (c + 1)]
                if i == 0:
                    yout4 = ypool.tile([C, 4, CH], f32, tag="yout4")
                if c % 3 == 0:
                    srs, snb = stage_stats(sgp, rs2, nb2, c)
                j = c % 3
                pa = psum2.tile([C, CH], f32, tag="pa5")
                nc.tensor.matmul(pa, lhsT=ones3x[32 * j:32 * j + 1, :],
                                 rhs=srs[32 * j:32 * j + 1, :],
                                 start=True, stop=True)
                pb2 = psum2.tile([C, CH], f32, tag="pb5")
                nc.tensor.matmul(pb2, lhsT=ones3x[32 * j:32 * j + 1, :],
                                 rhs=snb[32 * j:32 * j + 1, :],
                                 start=True, stop=True)
                t1 = pool.tile([C, CH], f32, tag="t15")
                nc.vector.tensor_tensor(out=t1, in0=rsl, in1=pa, op=ALU.mult)
                xn = xns[c % 2]
                nc.vector.tensor_tensor(out=xn[0:C, :], in0=t1,
                                        in1=pb2, op=ALU.add)

                h1 = hpool.tile([128, 3, CH], bf16, tag="h1")
                for j in range(3):
                    pf = psum.tile([128, CH], f32, tag="pf")
                    nc.tensor.matmul(pf, lhsT=w1[:, 128 * j:128 * (j + 1)], rhs=xn,
                                     start=True, stop=True)
                    nc.scalar.activation(out=h1[:, j, :], in_=pf, func=AF.Gelu)
                pm = psum.tile([C, CH], f32, tag="pm")
                nc.tensor.matmul(pm, lhsT=b2t, rhs=onesrow, start=True, stop=False)
                for j in range(3):
                    nc.tensor.matmul(pm, lhsT=w2[j],
                                     rhs=h1[:, j, :], start=False, stop=(j == 2))
                nc.vector.tensor_tensor(out=yout4[:, i, :], in0=rsl,
                                        in1=pm, op=ALU.add)
                if i == 3:
                    nc.sync.dma_start(out=y_d[:, 16 * g:16 * g + 16, :], in_=yout4)

        r1pool.release()
        ofpool.release()
        apool.release()
        wpool.release()

    _split_multi_waits(nc, mybir)
    return nc


def _prep_weights(inputs):
    """Host-side weight preparation (fold LN affine, scale, bias rows)."""
    qkv_w = np.asarray(inputs['qkv_w'], np.float32)       # (288, 96)
    proj_w = np.asarray(inputs['proj_w'], np.float32)     # (96, 96)
    proj_b = np.asarray(inputs['proj_b'], np.float32)
    ln1_w = np.asarray(inputs['ln1_w'], np.float32)
    ln1_b = np.asarray(inputs['ln1_b'], np.float32)
    ln2_w = np.asarray(inputs['ln2_w'], np.float32)
    ln2_b = np.asarray(inputs['ln2_b'], np.float32)
    fc1_w = np.asarray(inputs['fc1_w'], np.float32)       # (384, 96)
    fc1_b = np.asarray(inputs['fc1_b'], np.float32)
    fc2_w = np.asarray(inputs['fc2_w'], np.float32)       # (96, 384)
    fc2_b = np.asarray(inputs['fc2_b'], np.float32)

    wq = qkv_w * ln1_w[None, :]                            # (288, 96)
    c0 = qkv_w @ ln1_b                                     # (288,)
    wq[0:C] *= SCALE                                       # scale q rows
    c0[0:C] *= SCALE
    wqb = np.concatenate([wq.T, c0[None, :]], axis=0)      # (97, 288)

    w1 = fc1_w * ln2_w[None, :]
    c1 = fc1_w @ ln2_b + fc1_b
    w1b = np.concatenate([w1.T, c1[None, :]], axis=0)      # (97, 384)

    repl = np.zeros((128, 128), np.float32)
    for b in range(NB):
        for ch in range(GD):
            h0 = (ch // HD) * HD
            repl[32 * b + h0:32 * b + h0 + HD, 32 * b + ch] = 1.0

    wsum3 = np.zeros((C, 3 * C), np.float32)
    for j in range(3):
        wsum3[32 * j, :] = wq.T[0:C, :].sum(axis=0)        # per-column sums

    return {
        'wsum3': wsum3,
        'wqkv': np.ascontiguousarray(wqb),                 # (97, 288) lhsT
        'wproj': np.ascontiguousarray(proj_w.T),           # (96, 96) lhsT
        'projb': proj_b.reshape(-1, 1).astype(np.float32),
        'w1': np.ascontiguousarray(w1b),                   # (97, 384) lhsT
        'w2': np.ascontiguousarray(fc2_w.T),               # (384, 96) lhsT
        'b2': fc2_b.reshape(-1, 1).astype(np.float32),
        'repl': repl,
        'onesc': np.ones((C, 1), np.float32),
    }


def kernel(**inputs):
    from concourse.bass_utils import run_bass_kernel_spmd

    if 'nc' not in _cache:
        t0 = time.time()
        _cache['nc'] = _build()
        print(f"[kernel] built bass module in {time.time() - t0:.1f}s",
              file=sys.stderr)

    nc = _cache['nc']
    wmap = _prep_weights(inputs)
    x = np.asarray(inputs['x'], np.float32)                # (8, 96, 128, 128)

    in_maps = []
    for b in range(B):
        m = {'x': np.ascontiguousarray(x[b])}
        m.update(wmap)
        in_maps.append(m)

    res = run_bass_kernel_spmd(nc, in_maps, core_ids=list(range(B)))
    _cache['last_exec_ns'] = res.exec_time_ns
    out = np.stack([res.results[b]['y'] for b in range(B)], axis=0)
    return out.astype(np.float32)


# revision 72
# speedup vs baseline: 1.0510x; 1.0005x over previous
"""DilateBlock kernel for 8x Trainium2 NeuronCores (Bass/Tile).

Data-parallel over batch B=8 (one image per core). Per core, the whole block
(LN1 -> qkv -> 3-dilation 3x3 neighborhood attention -> proj -> residual ->
LN2 -> MLP -> residual) runs in channels-on-partitions layout; spatial shifts
for the attention unfold live on the free dimension of zero-padded (h, w)
planes, packed 4-hbands x 32-channels across partitions.

Key tricks:
  - LayerNorm stats via ones-matmul on PE into a [32, 512] PSUM strip tile
    (chunk index on partitions), stats math runs wide on 32 partitions, and
    the per-token scale/shift rows feed rank-1 PSUM matmuls directly.
  - qkv/fc1 biases folded into the matmuls via a 97th ones-row of the
    LN-applied activations (contract-97 lhsT with a bias row).
  - K/V drained to contiguous staging as single 96-partition copies, then
    band-packed into padded planes by SBUF->SBUF DMAs on the idle DMA rings.
  - QK tap logits reduced over head_dim AND replicated back to all 16
    channel rows in one PE matmul with a static block-ones matrix; exp runs
    full width on Act; softmax denominator and output accumulate across taps
    via gpsimd DMA-accumulate (even/odd partial tiles, merged on DVE).
  - Softmax normalization applied to the attention OUTPUT.
  - Attention output repacked in SBUF (no DRAM roundtrip) for the proj.
"""
import sys
import time

sys.path.insert(0, '/opt/trn_rl_repo')

import numpy as np

# ---- problem constants (hardcoded per contract) ----
B, C, H, W = 8, 96, 128, 128
DILS = (1, 2, 3)
GD = 32                 # channels per dilation branch
HD = 16                 # head dim
NB = 4                  # h-bands packed on partitions
BH = H // NB            # rows per band = 32
N = H * W               # tokens per image
NCHUNK = 32             # token chunks of 512 (4 image rows each)
CH = N // NCHUNK        # 512
PADR = 38               # BH + 6 halo rows
PADC = 136              # W + 8 halo cols (EVEN pitch: enables DVE 2x mode)
EPS = 1e-5
SCALE = HD ** -0.5
MLPH = 384

_cache = {}
import os
_USE_DMA_ACCUM = os.environ.get('KDMA', '0') == '1'


def _patch_tile(tile_mod, bass_mod):
    """Work around this walrus build's 1-sem-wait-per-instruction limit and
    the multi-wait tail drain."""
    from concourse.vector_clock import ScopedClock, VectorClock

    def _drain_and_barrier(self, tick_clock, wait_clock):
        vclock = tick_clock.global_clock
        n = len(vclock)
        idxs = [i for i in range(n) if vclock[i] > 0]
        for i in idxs:
            vec = [0] * n
            vec[i] = vclock[i]
            nop_inst = self.nc.sync.nop(nofuse=True)
            wait_clock.add_sem_waits(nop_inst.ins,
                                     ScopedClock({None: VectorClock(vec)}))
        self.nc.sync.drain()
        self.nc.all_engine_barrier()
        popped = self.nc._tile_sem_poison_stack.pop()
        assert popped is self._sem_poison
        self.nc.clear_and_free_semaphores(list(self.sems.allocated().values()))
        self.nc.all_engine_barrier()

    tile_mod.TileContext._drain_and_barrier = _drain_and_barrier


_ws_counter = [0]


def _split_multi_waits(nc, mybir):
    for fn in nc.m.functions:
        for blk in fn.blocks:
            insts = list(blk.instructions)
            out = []
            changed = False
            for inst in insts:
                si = inst.sync_info
                waits = list(si.on_wait) if si and si.on_wait else []
                if len(waits) > 1:
                    for w in waits[:-1]:
                        _ws_counter[0] += 1
                        out.append(mybir.InstNoOp(
                            name=f"I-ws-{_ws_counter[0]}",
                            engine=inst.engine, ins=[], outs=[],
                            sync_info=mybir.SyncInfo(on_wait=[w], on_update=[])))
                    si.on_wait = [waits[-1]]
                    changed = True
                out.append(inst)
            if changed:
                blk.instructions[:] = out


def _build():
    import concourse.bass as bass
    import concourse.tile as tile
    from concourse import mybir

    _patch_tile(tile, bass)

    f32 = mybir.dt.float32
    f32r = mybir.dt.float32r
    bf16 = mybir.dt.bfloat16
    AF = mybir.ActivationFunctionType
    ALU = mybir.AluOpType

    nc = bass.Bass()

    # ---- DRAM I/O ----
    x_d = nc.dram_tensor("x", (C, H, W), f32, kind="ExternalInput")
    wq_d = nc.dram_tensor("wqkv", (C + 1, 3 * C), f32, kind="ExternalInput")  # lhsT+bias
    wp_d = nc.dram_tensor("wproj", (C, C), f32, kind="ExternalInput")         # lhsT
    pb_d = nc.dram_tensor("projb", (C, 1), f32, kind="ExternalInput")
    w1_d = nc.dram_tensor("w1", (C + 1, MLPH), f32, kind="ExternalInput")     # lhsT+bias
    w2_d = nc.dram_tensor("w2", (MLPH, C), f32, kind="ExternalInput")         # lhsT
    b2_d = nc.dram_tensor("b2", (C, 1), f32, kind="ExternalInput")
    repl_d = nc.dram_tensor("repl", (128, 128), f32, kind="ExternalInput")
    ws_d = nc.dram_tensor("wsum3", (C, 3 * C), f32, kind="ExternalInput")
    ones_d = nc.dram_tensor("onesc", (C, 1), f32, kind="ExternalInput")

    y_d = nc.dram_tensor("y", (C, H, W), f32, kind="ExternalOutput")

    with tile.TileContext(nc) as tc:
        # ---------------- persistent pools ----------------
        wpool = tc.alloc_tile_pool(name="weights", bufs=1)
        wq = wpool.tile([C + 1, 3 * C], f32r)
        nc.sync.dma_start(out=wq, in_=wq_d[:, :].bitcast(f32r))
        wp = wpool.tile([C, C], bf16)
        nc.gpsimd.dma_start(out=wp, in_=wp_d[:, :])     # gpsimd dma casts
        pbt = wpool.tile([1, C], f32r)                  # proj bias as rank-1 lhsT
        nc.sync.dma_start(out=pbt, in_=pb_d[:, :].rearrange("a b -> b a").bitcast(f32r))
        w1 = wpool.tile([C + 1, MLPH], f32r)
        nc.sync.dma_start(out=w1, in_=w1_d[:, :].bitcast(f32r))
        w2 = [wpool.tile([128, C], bf16, tag=f"w2{i}", name=f"w2{i}") for i in range(3)]
        for i in range(3):
            nc.gpsimd.dma_start(out=w2[i], in_=w2_d[128 * i:128 * (i + 1), :])
        b2t = wpool.tile([1, C], f32r)                  # fc2 bias as rank-1 lhsT
        nc.sync.dma_start(out=b2t, in_=b2_d[:, :].rearrange("a b -> b a").bitcast(f32r))
        repl = wpool.tile([128, 128], bf16)
        nc.gpsimd.dma_start(out=repl, in_=repl_d[:, :])
        srepl = wpool.tile([128, 128], bf16)            # repl/16: S accumulation
        nc.scalar.mul(out=srepl, in_=repl, mul=1.0 / HD)
        onescol = wpool.tile([C, 1], f32r)              # stats lhsT [96,1]
        nc.sync.dma_start(out=onescol, in_=ones_d[:, :].bitcast(f32r))
        onescol_b = wpool.tile([C, 1], bf16)            # bf16 variant (bf16 rhs)
        nc.vector.memset(onescol_b, 1.0)
        wsum3 = wpool.tile([C, 3 * C], f32r)            # qkv col-sums at {0,32,64}
        nc.sync.dma_start(out=wsum3, in_=ws_d[:, :].bitcast(f32r))
        ones1x = wpool.tile([1, C], f32r)               # rank-1 lhsT [1,96]
        nc.sync.dma_start(out=ones1x, in_=ones_d[:, :].rearrange("a b -> b a").bitcast(f32r))
        onesrow = wpool.tile([1, CH], f32r)             # static ones row (f32r)
        ones3x = wpool.tile([C, C], f32r)               # ones rows at {0,32,64}
        epst = wpool.tile([128, 1], f32)
        nc.vector.memset(epst, EPS)
        # LN-applied activation tiles with a persistent ones bias row
        xns = [wpool.tile([C + 1, CH], f32r, tag=f"xn{i}", name=f"xn{i}")
               for i in range(2)]
        # f32 scratch (init-only) to produce properly-rounded f32r constants
        initp = tc.alloc_tile_pool(name="initp", bufs=1)
        onesrow_f = initp.tile([1, CH], f32)
        nc.vector.memset(onesrow_f, 1.0)
        nc.vector.tensor_copy(out=onesrow, in_=onesrow_f)
        ones3f = initp.tile([C, C], f32)
        nc.vector.memset(ones3f, 0.0)
        for j in range(3):
            nc.vector.memset(ones3f[32 * j:32 * j + 1, :], 1.0)
        nc.vector.tensor_copy(out=ones3x, in_=ones3f)
        for i in range(2):
            nc.vector.tensor_copy(out=xns[i][C:C + 1, :], in_=onesrow_f)
        initp.release()
        # LN stats rows: rs/nb per chunk on partitions [32, 512]
        # (LN2 pass reuses the same tiles after LN1's readers are done)
        rs1 = wpool.tile([NCHUNK, CH], f32r)
        nb1 = wpool.tile([NCHUNK, CH], f32r)
        rs2, nb2 = rs1, nb1
        # channel-sum / channel-sumsq strips (chunk on partitions)
        ss1 = wpool.tile([NCHUNK, CH], f32)
        sq1 = wpool.tile([NCHUNK, CH], f32)
        ss2, sq2 = ss1, sq1

        # big persistent activation tensors
        apool = tc.alloc_tile_pool(name="acts", bufs=1)
        Qd = [apool.tile([128, BH, W], bf16, tag=f"qd{d}", name=f"qd{d}") for d in range(3)]
        Kp = [apool.tile([128, PADR, PADC], bf16, tag=f"kp{d}", name=f"kp{d}") for d in range(3)]
        Vp = [apool.tile([128, PADR, PADC], bf16, tag=f"vp{d}", name=f"vp{d}") for d in range(3)]

        # zero only the pad strips (interior fully overwritten by repack)
        for d in range(3):
            for t in (Kp[d], Vp[d]):
                nc.gpsimd.memset(t[:, 0:3, :], 0.0)
                nc.gpsimd.memset(t[:, 35:38, :], 0.0)
                nc.gpsimd.memset(t[:, 3:35, 0:3], 0.0)
                nc.gpsimd.memset(t[:, 3:35, 3 + W:PADC], 0.0)

        # ============ shared stats math ============
        # strips ss/sq: [32, CH] SBUF, chunk on partitions.  Runs on row
        # slices [c0:c0+n] right after each flush so downstream chunks can
        # start without waiting for the whole stats pass.  mu overwrites ss,
        # var/sd overwrite sq in place; vtmp holds -mu^2.
        vtmp = wpool.tile([NCHUNK, CH], f32)

        def stats_math_early(ss, sq, rs, nb):
            sl = slice(0, 8)
            mu = vtmp[sl, :]
            nc.scalar.mul(out=mu, in_=ss[sl, :], mul=1.0 / C)
            t = rs[sl, :].bitcast(f32)
            nc.scalar.mul(out=t, in_=sq[sl, :], mul=1.0 / C)
            nc.vector.scalar_tensor_tensor(out=nb[sl, :].bitcast(f32), in0=mu,
                                           scalar=-1.0, in1=mu,
                                           op0=ALU.mult, op1=ALU.mult)
            nc.vector.tensor_tensor(out=t, in0=t, in1=nb[sl, :].bitcast(f32),
                                    op=ALU.add)
            nc.scalar.activation(out=t, in_=t, func=AF.Sqrt,
                                 bias=epst[0:8, 0:1], scale=1.0)
            with nc.allow_low_precision(reason="f32r-typed LN stats rows"):
                nc.vector.reciprocal(out=rs[sl, :], in_=t)
                nc.vector.scalar_tensor_tensor(out=nb[sl, :], in0=mu,
                                               scalar=-1.0,
                                               in1=rs[sl, :].bitcast(f32),
                                               op0=ALU.mult, op1=ALU.mult)

        def stats_math(ss, sq, rs, nb, c0, n):
            sl = slice(c0, c0 + n)
            nc.scalar.mul(out=ss[sl, :], in_=ss[sl, :], mul=1.0 / C)
            nc.scalar.mul(out=sq[sl, :], in_=sq[sl, :], mul=1.0 / C)
            nc.vector.scalar_tensor_tensor(out=vtmp[sl, :], in0=ss[sl, :],
                                           scalar=-1.0, in1=ss[sl, :],
                                           op0=ALU.mult, op1=ALU.mult)
            nc.vector.tensor_tensor(out=sq[sl, :], in0=sq[sl, :],
                                    in1=vtmp[sl, :], op=ALU.add)
            nc.scalar.activation(out=sq[sl, :], in_=sq[sl, :], func=AF.Sqrt,
                                 bias=epst[c0:c0 + n, 0:1], scale=1.0)
            with nc.allow_low_precision(reason="f32r-typed LN stats rows"):
                nc.vector.reciprocal(out=rs[sl, :], in_=sq[sl, :])
                nc.vector.scalar_tensor_tensor(out=nb[sl, :], in0=ss[sl, :],
                                               scalar=-1.0,
                                               in1=rs[sl, :].bitcast(f32),
                                               op0=ALU.mult, op1=ALU.mult)

        # strip helper: drain a [1, n*CH] psum strip (partition 0) to a
        # 1-partition SBUF stage, then one DMA reshapes it into rows
        # [c0:c0+n] of the compact [32, CH] stats tiles.
        def strip_flush(pool, ps_s, ps_q, ss, sq, c0, n, qeng=None):
            stg_s = pool.tile([1, n * CH], f32, tag="stg_s")
            nc.scalar.copy(stg_s, ps_s[:, 0:n * CH])
            stg_q = pool.tile([1, n * CH], f32, tag="stg_q")
            if qeng is None:
                nc.scalar.copy(stg_q, ps_q[:, 0:n * CH])
            else:
                qeng.tensor_copy(out=stg_q, in_=ps_q[:, 0:n * CH])
            nc.sync.dma_start(out=ss[c0:c0 + n, :], in_=stg_s)
            nc.sync.dma_start(out=sq[c0:c0 + n, :], in_=stg_q)

        # ============ PH1: LN1 stats sweep ============
        with tc.tile_pool(name="ph1", bufs=3) as pool, \
             tc.tile_pool(name="ph1st", bufs=2) as sgpool, \
             tc.tile_pool(name="ph1ps", bufs=2, space="PSUM") as stps:
            ps_s = ps_q = None
            for g in range(NCHUNK // 4):
                xt4 = pool.tile([C, 4, CH], f32r, tag="xt")
                nc.sync.dma_start(out=xt4,
                                  in_=x_d[:, 16 * g:16 * g + 16, :].bitcast(f32r))
                for i in range(4):
                    c = 4 * g + i
                    if i % 2 == 0:
                        ps_s = stps.tile([1, 2 * CH], f32, tag="ps_s")
                        ps_q = stps.tile([1, 2 * CH], f32, tag="ps_q")
                    h = CH * (i % 2)
                    xt = xt4[:, i, :]
                    nc.tensor.matmul(ps_s[:, h:h + CH], lhsT=onescol,
                                     rhs=xt, start=True, stop=True)
                    xf = xt.bitcast(f32)
                    xsq = pool.tile([C, CH], f32r, tag="xsq")
                    if c % 2 == 0:
                        nc.vector.tensor_tensor(out=xsq, in0=xf, in1=xf, op=ALU.mult)
                    else:
                        nc.gpsimd.tensor_tensor(out=xsq, in0=xf, in1=xf, op=ALU.mult)
                    nc.tensor.matmul(ps_q[:, h:h + CH], lhsT=onescol,
                                     rhs=xsq, start=True, stop=True)
                    if i % 2 == 1:
                        strip_flush(sgpool, ps_s, ps_q, ss1, sq1, c - 1, 2,
                                    qeng=nc.vector if g % 2 == 0 else None)
                    if i == 3:
                        if g == 7:
                            stats_math(ss1, sq1, rs1, nb1, 0, NCHUNK)

        stgpool = tc.alloc_tile_pool(name="stg", bufs=1)
        stg_k = stgpool.tile([C, N], bf16)
        stg_v = stgpool.tile([C, N], bf16)


        # ============ PH2: LN1 apply + qkv + stage/scatter ============
        def band_rows(b):
            lo = max(0, BH * b - 3)
            hi = min(H, BH * b + BH + 3)
            return lo, hi

        # stage rs/nb rows at partitions {0,32,64} so rank-1 matmuls can
        # read them (PE base-partition rule); one strided DMA per 3 chunks
        def stage_stats(sgp, rs, nb, c0):
            n = min(3, NCHUNK - c0)
            srs = sgp.tile([C, CH], f32r, tag="srs")
            snb = sgp.tile([C, CH], f32r, tag="snb")
            dst_rs = srs.rearrange("(a b) f -> a b f", a=3)[0:n, 0:1, :]
            dst_nb = snb.rearrange("(a b) f -> a b f", a=3)[0:n, 0:1, :]
            nc.sync.dma_start(out=dst_rs, in_=rs[c0:c0 + n, :])
            nc.sync.dma_start(out=dst_nb, in_=nb[c0:c0 + n, :])
            return srs, snb

        with tc.tile_pool(name="ph2", bufs=3) as pool, \
             tc.tile_pool(name="ph2t", bufs=2) as tpool, \
             tc.tile_pool(name="ph2sg", bufs=2) as sgp, \
             tc.tile_pool(name="ph2ps", bufs=2, space="PSUM") as psum, \
             tc.tile_pool(name="ph2ps2", bufs=2, space="PSUM") as psum2:
            srs = snb = None
            for c in range(NCHUNK):
                g, i = c // 4, c % 4
                if i == 0:
                    xt4 = pool.tile([C, 4, CH], f32, tag="xt2")
                    nc.sync.dma_start(out=xt4, in_=x_d[:, 16 * g:16 * g + 16, :])
                if c % 3 == 0:
                    srs, snb = stage_stats(sgp, rs1, nb1, c)
                j = c % 3
                xt = xt4[:, i, :]
                pa = psum2.tile([C, CH], f32, tag="pa")
                nc.tensor.matmul(pa, lhsT=ones3x[32 * j:32 * j + 1, :],
                                 rhs=srs[32 * j:32 * j + 1, :],
                                 start=True, stop=True)
                xn = xns[c % 2]
                nc.vector.tensor_tensor(out=xn[0:C, :], in0=xt, in1=pa,
                                        op=ALU.mult)

                pq = psum.tile([C, CH], f32, tag="pq")
                pk = psum.tile([C, CH], f32, tag="pk")
                pv = psum.tile([C, CH], f32, tag="pv")
                snbj = snb[32 * j:32 * j + 1, :]
                for t, pt in enumerate((pq, pk, pv)):
                    nc.tensor.matmul(pt, lhsT=wsum3[32 * j:32 * j + 1,
                                                    C * t:C * (t + 1)],
                                     rhs=snbj, start=True, stop=False)
                    nc.tensor.matmul(pt, lhsT=wq[:, C * t:C * (t + 1)], rhs=xn,
                                     start=False, stop=True)

                # K/V -> contiguous staging (single 96-partition copies);
                # gpsimd cannot touch PSUM, so drains go DVE/Act only
                nc.scalar.copy(stg_k[:, CH * c:CH * (c + 1)], pk)
                if c % 2 == 0:
                    nc.vector.tensor_copy(out=stg_v[:, CH * c:CH * (c + 1)], in_=pv)
                else:
                    nc.scalar.copy(stg_v[:, CH * c:CH * (c + 1)], pv)
                # Q -> band-packed planes directly
                b = c // 8
                r_off = 4 * c - BH * b
                for d in range(3):
                    src = pq[32 * d:32 * d + 32, :].rearrange("p (r w) -> p r w", r=4)
                    dst = Qd[d][32 * b:32 * b + 32, r_off:r_off + 4, :]
                    if d == 0 or (d == 2 and c % 2 == 1):
                        nc.vector.tensor_copy(out=dst, in_=src)
                    else:
                        nc.scalar.copy(dst, src)

                # band-packed K/V repack via SBUF->SBUF DMA on idle rings
                if c in (8, 16, 24, 31):
                    b_ = (c - 1) // 8
                    lo, hi = band_rows(b_)
                    nr = hi - lo
                    r0 = lo - (BH * b_ - 3)
                    for d in range(3):
                        for stg, dstp in ((stg_k, Kp[d]), (stg_v, Vp[d])):
                            nc.sync.dma_start(
                                out=dstp[32 * b_:32 * b_ + 32, r0:r0 + nr, 3:3 + W],
                                in_=stg[32 * d:32 * d + 32, W * lo:W * hi]
                                    .rearrange("p (r w) -> p r w", r=nr))

        stgpool.release()

        # attention output (channel-major), in space freed by the staging
        ofpool = tc.alloc_tile_pool(name="ofp", bufs=1)
        ofull = ofpool.tile([C, N], bf16)

        # ============ PH3: attention per dilation ============
        # Processed in half-planes (16 band-rows each) so the softmax
        # denominator S accumulates across taps in PSUM via PE matmuls with
        # srepl (= repl/16), freeing DVE of the S adds entirely.
        HF = BH * W // 2                                # 2048 tokens per half
        with tc.tile_pool(name="ph3", bufs=4) as pool, \
             tc.tile_pool(name="ph3f", bufs=4) as fpool, \
             tc.tile_pool(name="ph3acc", bufs=3) as acc, \
             tc.tile_pool(name="ph3ps", bufs=2, space="PSUM") as psum, \
             tc.tile_pool(name="ph3sps", bufs=1, space="PSUM") as spsum:
            for di, dil in enumerate(DILS):
                for hh in range(2):
                    S_ps = spsum.tile([128, HF], f32, tag="Sps")
                    Oab = [acc.tile([128, HF], bf16, tag=f"O{p}",
                                    name=f"O{p}_{di}_{hh}") for p in range(2)]
                    rcp = acc.tile([128, HF], bf16, tag="rcp",
                                   name=f"rcp_{di}_{hh}")
                    qv = Qd[di][:, 16 * hh:16 * hh + 16, :]
                    for ti, (dr, dc) in enumerate([(i - 1, j - 1)
                                                   for i in range(3) for j in range(3)]):
                        r0 = 3 + dr * dil + 16 * hh
                        kwin = Kp[di][:, r0:r0 + 16, 3 + dc * dil:3 + dc * dil + W]
                        vwin = Vp[di][:, r0:r0 + 16, 3 + dc * dil:3 + dc * dil + W]
                        P = fpool.tile([128, HF], bf16, tag="P")
                        nc.vector.tensor_tensor(
                            out=P.rearrange("p (r w) -> p r w", r=16),
                            in0=qv, in1=kwin, op=ALU.mult)
                        # logits -> exp (overwrites P) -> S accumulation
                        for q in range(2):
                            pl = psum.tile([128, 1024], f32, tag="pl")
                            for j in range(2):
                                nc.tensor.matmul(
                                    pl[:, 512 * j:512 * (j + 1)], lhsT=repl,
                                    rhs=P[:, 1024 * q + 512 * j:
                                          1024 * q + 512 * (j + 1)],
                                    start=True, stop=True)
                            nc.scalar.activation(
                                out=P[:, 1024 * q:1024 * (q + 1)], in_=pl,
                                func=AF.Exp)
                            for j in range(2):
                                nc.tensor.matmul(
                                    S_ps[:, 1024 * q + 512 * j:
                                         1024 * q + 512 * (j + 1)],
                                    lhsT=srepl,
                                    rhs=P[:, 1024 * q + 512 * j:
                                          1024 * q + 512 * (j + 1)],
                                    start=(ti == 0), stop=(ti == 8))
                        ev = P.rearrange("p (r w) -> p r w", r=16)
                        Pv = Oab[ti] if ti < 2 else pool.tile([128, HF], bf16,
                                                              tag="Pv")
                        nc.vector.tensor_tensor(
                            out=Pv.rearrange("p (r w) -> p r w", r=16),
                            in0=ev, in1=vwin, op=ALU.mult)
                        if ti >= 2:
                            eng = nc.gpsimd if ti in (3, 5, 7) else nc.vector
                            eng.tensor_tensor(out=Oab[ti % 2], in0=Oab[ti % 2],
                                              in1=Pv, op=ALU.add)
                    with nc.allow_low_precision(reason="softmax recip bf16"):
                        nc.vector.reciprocal(out=rcp, in_=S_ps)
                    nc.gpsimd.tensor_tensor(out=Oab[0], in0=Oab[0], in1=Oab[1],
                                             op=ALU.add)
                    nc.vector.tensor_tensor(out=Oab[0], in0=Oab[0], in1=rcp,
                                            op=ALU.mult)
                    for b in range(NB):
                        nc.sync.dma_start(
                            out=ofull[32 * di:32 * di + 32,
                                      4096 * b + 2048 * hh:
                                      4096 * b + 2048 * hh + 2048],
                            in_=Oab[0][32 * b:32 * b + 32, :])

        # ============ PH4: proj + residual + LN2 stats ============
        r1pool = tc.alloc_tile_pool(name="r1p", bufs=1)
        r1 = r1pool.tile([C, N], bf16)
        with tc.tile_pool(name="ph4", bufs=2) as pool, \
             tc.tile_pool(name="ph4sg", bufs=1) as sgpool, \
             tc.tile_pool(name="ph4st", bufs=1, space="PSUM") as stps, \
             tc.tile_pool(name="ph4ps", bufs=2, space="PSUM") as psum:
            ps_s = ps_q = None
            for c in range(NCHUNK):
                g, i = c // 4, c % 4
                if i == 0:
                    xt4 = pool.tile([C, 4, CH], f32, tag="xt4")
                    nc.sync.dma_start(out=xt4, in_=x_d[:, 16 * g:16 * g + 16, :])
                if c % 2 == 0:
                    ps_s = stps.tile([1, 2 * CH], f32, tag="ps_s4")
                    ps_q = stps.tile([1, 2 * CH], f32, tag="ps_q4")
                h4 = CH * (c % 2)
                pp = psum.tile([C, CH], f32, tag="pp")
                nc.tensor.matmul(pp, lhsT=pbt, rhs=onesrow, start=True, stop=False)
                nc.tensor.matmul(pp, lhsT=wp, rhs=ofull[:, CH * c:CH * (c + 1)],
                                 start=False, stop=True)
                rsl = r1[:, CH * c:CH * (c + 1)]
                nc.vector.tensor_tensor(out=rsl, in0=xt4[:, i, :],
                                        in1=pp, op=ALU.add)
                nc.tensor.matmul(ps_s[:, h4:h4 + CH], lhsT=onescol_b,
                                 rhs=rsl, start=True, stop=True)
                xsq = pool.tile([C, CH], f32r, tag="xsq5")
                nc.vector.tensor_tensor(out=xsq, in0=rsl, in1=rsl, op=ALU.mult)
                nc.tensor.matmul(ps_q[:, h4:h4 + CH], lhsT=onescol,
                                 rhs=xsq, start=True, stop=True)
                if c % 2 == 1:
                    strip_flush(sgpool, ps_s, ps_q, ss2, sq2, c - 1, 2)
                    if c == 31:
                        stats_math(ss2, sq2, rs2, nb2, 0, NCHUNK)

        # ============ PH5: MLP + residual ============
        with tc.tile_pool(name="ph5b", bufs=3) as pool, \
             tc.tile_pool(name="ph5h", bufs=2) as hpool, \
             tc.tile_pool(name="ph5y", bufs=2) as ypool, \
             tc.tile_pool(name="ph5sg", bufs=2) as sgp, \
             tc.tile_pool(name="ph5ps", bufs=2, space="PSUM") as psum, \
             tc.tile_pool(name="ph5ps2", bufs=1, space="PSUM") as psum2:
            srs = snb = None
            for c in range(NCHUNK):
                g, i = c // 4, c % 4
                rsl = r1[:, CH * c:CH * (c + 1)]
                if i == 0:
                    yout4 = ypool.tile([C, 4, CH], f32, tag="yout4")
                if c % 3 == 0:
                    srs, snb = stage_stats(sgp, rs2, nb2, c)
                j = c % 3
                pa = psum2.tile([C, CH], f32, tag="pa5")
                nc.tensor.matmul(pa, lhsT=ones3x[32 * j:32 * j + 1, :],
                                 rhs=srs[32 * j:32 * j + 1, :],
                                 start=True, stop=True)
                pb2 = psum2.tile([C, CH], f32, tag="pb5")
                nc.tensor.matmul(pb2, lhsT=ones3x[32 * j:32 * j + 1, :],
                                 rhs=snb[32 * j:32 * j + 1, :],
                                 start=True, stop=True)
                t1 = pool.tile([C, CH], f32, tag="t15")
                nc.vector.tensor_tensor(out=t1, in0=rsl, in1=pa, op=ALU.mult)
                xn = xns[c % 2]
                nc.vector.tensor_tensor(out=xn[0:C, :], in0=t1,
                                        in1=pb2, op=ALU.add)

                h1 = hpool.tile([128, 3, CH], bf16, tag="h1")
                for j in range(3):
                    pf = psum.tile([128, CH], f32, tag="pf")
                    nc.tensor.matmul(pf, lhsT=w1[:, 128 * j:128 * (j + 1)], rhs=xn,
                                     start=True, stop=True)
                    nc.scalar.activation(out=h1[:, j, :], in_=pf, func=AF.Gelu)
                pm = psum.tile([C, CH], f32, tag="pm")
                nc.tensor.matmul(pm, lhsT=b2t, rhs=onesrow, start=True, stop=False)
                for j in range(3):
                    nc.tensor.matmul(pm, lhsT=w2[j],
                                     rhs=h1[:, j, :], start=False, stop=(j == 2))
                nc.vector.tensor_tensor(out=yout4[:, i, :], in0=rsl,
                                        in1=pm, op=ALU.add)
                if i == 3:
                    nc.sync.dma_start(out=y_d[:, 16 * g:16 * g + 16, :], in_=yout4)

        r1pool.release()
        ofpool.release()
        apool.release()
        wpool.release()

    _split_multi_waits(nc, mybir)
    return nc


def _prep_weights(inputs):
    """Host-side weight preparation (fold LN affine, scale, bias rows)."""
    qkv_w = np.asarray(inputs['qkv_w'], np.float32)       # (288, 96)
    proj_w = np.asarray(inputs['proj_w'], np.float32)     # (96, 96)
    proj_b = np.asarray(inputs['proj_b'], np.float32)
    ln1_w = np.asarray(inputs['ln1_w'], np.float32)
    ln1_b = np.asarray(inputs['ln1_b'], np.float32)
    ln2_w = np.asarray(inputs['ln2_w'], np.float32)
    ln2_b = np.asarray(inputs['ln2_b'], np.float32)
    fc1_w = np.asarray(inputs['fc1_w'], np.float32)       # (384, 96)
    fc1_b = np.asarray(inputs['fc1_b'], np.float32)
    fc2_w = np.asarray(inputs['fc2_w'], np.float32)       # (96, 384)
    fc2_b = np.asarray(inputs['fc2_b'], np.float32)

    wq = qkv_w * ln1_w[None, :]                            # (288, 96)
    c0 = qkv_w @ ln1_b                                     # (288,)
    wq[0:C] *= SCALE                                       # scale q rows
    c0[0:C] *= SCALE
    wqb = np.concatenate([wq.T, c0[None, :]], axis=0)      # (97, 288)

    w1 = fc1_w * ln2_w[None, :]
    c1 = fc1_w @ ln2_b + fc1_b
    w1b = np.concatenate([w1.T, c1[None, :]], axis=0)      # (97, 384)

    repl = np.zeros((128, 128), np.float32)
    for b in range(NB):
        for ch in range(GD):
            h0 = (ch // HD) * HD
            repl[32 * b + h0:32 * b + h0 + HD, 32 * b + ch] = 1.0

    wsum3 = np.zeros((C, 3 * C), np.float32)
    for j in range(3):
        wsum3[32 * j, :] = wq.T[0:C, :].sum(axis=0)        # per-column sums

    return {
        'wsum3': wsum3,
        'wqkv': np.ascontiguousarray(wqb),                 # (97, 288) lhsT
        'wproj': np.ascontiguousarray(proj_w.T),           # (96, 96) lhsT
        'projb': proj_b.reshape(-1, 1).astype(np.float32),
        'w1': np.ascontiguousarray(w1b),                   # (97, 384) lhsT
        'w2': np.ascontiguousarray(fc2_w.T),               # (384, 96) lhsT
        'b2': fc2_b.reshape(-1, 1).astype(np.float32),
        'repl': repl,
        'onesc': np.ones((C, 1), np.float32),
    }


def kernel(**inputs):
    from concourse.bass_utils import run_bass_kernel_spmd

    if 'nc' not in _cache:
        t0 = time.time()
        _cache['nc'] = _build()
        print(f"[kernel] built bass module in {time.time() - t0:.1f}s",
              file=sys.stderr)

    nc = _cache['nc']
    wmap = _prep_weights(inputs)
    x = np.asarray(inputs['x'], np.float32)                # (8, 96, 128, 128)

    in_maps = []
    for b in range(B):
        m = {'x': np.ascontiguousarray(x[b])}
        m.update(wmap)
        in_maps.append(m)

    res = run_bass_kernel_spmd(nc, in_maps, core_ids=list(range(B)))
    _cache['last_exec_ns'] = res.exec_time_ns
    out = np.stack([res.results[b]['y'] for b in range(B)], axis=0)
    return out.astype(np.float32)


# revision 80
# speedup vs baseline: 1.0545x; 1.0034x over previous
"""DilateBlock kernel for 8x Trainium2 NeuronCores (Bass/Tile).

Data-parallel over batch B=8 (one image per core). Per core, the whole block
(LN1 -> qkv -> 3-dilation 3x3 neighborhood attention -> proj -> residual ->
LN2 -> MLP -> residual) runs in channels-on-partitions layout; spatial shifts
for the attention unfold live on the free dimension of zero-padded (h, w)
planes, packed 4-hbands x 32-channels across partitions.

Key tricks:
  - LayerNorm stats via ones-matmul on PE into a [32, 512] PSUM strip tile
    (chunk index on partitions), stats math runs wide on 32 partitions, and
    the per-token scale/shift rows feed rank-1 PSUM matmuls directly.
  - qkv/fc1 biases folded into the matmuls via a 97th ones-row of the
    LN-applied activations (contract-97 lhsT with a bias row).
  - K/V drained to contiguous staging as single 96-partition copies, then
    band-packed into padded planes by SBUF->SBUF DMAs on the idle DMA rings.
  - QK tap logits reduced over head_dim AND replicated back to all 16
    channel rows in one PE matmul with a static block-ones matrix; exp runs
    full width on Act; softmax denominator and output accumulate across taps
    via gpsimd DMA-accumulate (even/odd partial tiles, merged on DVE).
  - Softmax normalization applied to the attention OUTPUT.
  - Attention output repacked in SBUF (no DRAM roundtrip) for the proj.
"""
import sys
import time

sys.path.insert(0, '/opt/trn_rl_repo')

import numpy as np

# ---- problem constants (hardcoded per contract) ----
B, C, H, W = 8, 96, 128, 128
DILS = (1, 2, 3)
GD = 32                 # channels per dilation branch
HD = 16                 # head dim
NB = 4                  # h-bands packed on partitions
BH = H // NB            # rows per band = 32
N = H * W               # tokens per image
NCHUNK = 32             # token chunks of 512 (4 image rows each)
CH = N // NCHUNK        # 512
PADR = 38               # BH + 6 halo rows
PADC = 136              # W + 8 halo cols (EVEN pitch: enables DVE 2x mode)
EPS = 1e-5
SCALE = HD ** -0.5
MLPH = 384

_cache = {}
import os
_USE_DMA_ACCUM = os.environ.get('KDMA', '0') == '1'


def _patch_tile(tile_mod, bass_mod):
    """Work around this walrus build's 1-sem-wait-per-instruction limit and
    the multi-wait tail drain."""
    from concourse.vector_clock import ScopedClock, VectorClock

    def _drain_and_barrier(self, tick_clock, wait_clock):
        vclock = tick_clock.global_clock
        n = len(vclock)
        idxs = [i for i in range(n) if vclock[i] > 0]
        for i in idxs:
            vec = [0] * n
            vec[i] = vclock[i]
            nop_inst = self.nc.sync.nop(nofuse=True)
            wait_clock.add_sem_waits(nop_inst.ins,
                                     ScopedClock({None: VectorClock(vec)}))
        self.nc.sync.drain()
        self.nc.all_engine_barrier()
        popped = self.nc._tile_sem_poison_stack.pop()
        assert popped is self._sem_poison
        self.nc.clear_and_free_semaphores(list(self.sems.allocated().values()))
        self.nc.all_engine_barrier()

    tile_mod.TileContext._drain_and_barrier = _drain_and_barrier


_ws_counter = [0]


def _split_multi_waits(nc, mybir):
    for fn in nc.m.functions:
        for blk in fn.blocks:
            insts = list(blk.instructions)
            out = []
            changed = False
            for inst in insts:
                si = inst.sync_info
                waits = list(si.on_wait) if si and si.on_wait else []
                if len(waits) > 1:
                    for w in waits[:-1]:
                        _ws_counter[0] += 1
                        out.append(mybir.InstNoOp(
                            name=f"I-ws-{_ws_counter[0]}",
                            engine=inst.engine, ins=[], outs=[],
                            sync_info=mybir.SyncInfo(on_wait=[w], on_update=[])))
                    si.on_wait = [waits[-1]]
                    changed = True
                out.append(inst)
            if changed:
                blk.instructions[:] = out


def _build():
    import concourse.bass as bass
    import concourse.tile as tile
    from concourse import mybir

    _patch_tile(tile, bass)

    f32 = mybir.dt.float32
    f32r = mybir.dt.float32r
    bf16 = mybir.dt.bfloat16
    AF = mybir.ActivationFunctionType
    ALU = mybir.AluOpType

    nc = bass.Bass()

    # ---- DRAM I/O ----
    x_d = nc.dram_tensor("x", (C, H, W), f32, kind="ExternalInput")
    wq_d = nc.dram_tensor("wqkv", (C + 1, 3 * C), f32, kind="ExternalInput")  # lhsT+bias
    wp_d = nc.dram_tensor("wproj", (C, C), f32, kind="ExternalInput")         # lhsT
    pb_d = nc.dram_tensor("projb", (C, 1), f32, kind="ExternalInput")
    w1_d = nc.dram_tensor("w1", (C + 1, MLPH), f32, kind="ExternalInput")     # lhsT+bias
    w2_d = nc.dram_tensor("w2", (MLPH, C), f32, kind="ExternalInput")         # lhsT
    b2_d = nc.dram_tensor("b2", (C, 1), f32, kind="ExternalInput")
    repl_d = nc.dram_tensor("repl", (128, 128), f32, kind="ExternalInput")
    ws_d = nc.dram_tensor("wsum3", (C, 3 * C), f32, kind="ExternalInput")
    ones_d = nc.dram_tensor("onesc", (C, 1), f32, kind="ExternalInput")

    y_d = nc.dram_tensor("y", (C, H, W), f32, kind="ExternalOutput")

    with tile.TileContext(nc) as tc:
        # ---------------- persistent pools ----------------
        wpool = tc.alloc_tile_pool(name="weights", bufs=1)
        wq = wpool.tile([C + 1, 3 * C], f32r)
        nc.sync.dma_start(out=wq, in_=wq_d[:, :].bitcast(f32r))
        wp = wpool.tile([C, C], bf16)
        nc.gpsimd.dma_start(out=wp, in_=wp_d[:, :])     # gpsimd dma casts
        pbt = wpool.tile([1, C], f32r)                  # proj bias as rank-1 lhsT
        nc.sync.dma_start(out=pbt, in_=pb_d[:, :].rearrange("a b -> b a").bitcast(f32r))
        w1 = wpool.tile([C + 1, MLPH], f32r)
        nc.sync.dma_start(out=w1, in_=w1_d[:, :].bitcast(f32r))
        w2 = [wpool.tile([128, C], bf16, tag=f"w2{i}", name=f"w2{i}") for i in range(3)]
        for i in range(3):
            nc.gpsimd.dma_start(out=w2[i], in_=w2_d[128 * i:128 * (i + 1), :])
        b2t = wpool.tile([1, C], f32r)                  # fc2 bias as rank-1 lhsT
        nc.sync.dma_start(out=b2t, in_=b2_d[:, :].rearrange("a b -> b a").bitcast(f32r))
        repl = wpool.tile([128, 128], bf16)
        nc.gpsimd.dma_start(out=repl, in_=repl_d[:, :])
        srepl = wpool.tile([128, 128], bf16)            # repl/16: S accumulation
        nc.scalar.mul(out=srepl, in_=repl, mul=1.0 / HD)
        onescol = wpool.tile([C, 1], f32r)              # stats lhsT [96,1]
        nc.sync.dma_start(out=onescol, in_=ones_d[:, :].bitcast(f32r))
        onescol_b = wpool.tile([C, 1], bf16)            # bf16 variant (bf16 rhs)
        nc.vector.memset(onescol_b, 1.0)
        wsum3 = wpool.tile([C, 3 * C], f32r)            # qkv col-sums at {0,32,64}
        nc.sync.dma_start(out=wsum3, in_=ws_d[:, :].bitcast(f32r))
        ones1x = wpool.tile([1, C], f32r)               # rank-1 lhsT [1,96]
        nc.sync.dma_start(out=ones1x, in_=ones_d[:, :].rearrange("a b -> b a").bitcast(f32r))
        onesrow = wpool.tile([1, CH], f32r)             # static ones row (f32r)
        ones3x = wpool.tile([C, C], f32r)               # ones rows at {0,32,64}
        epst = wpool.tile([128, 1], f32)
        nc.vector.memset(epst, EPS)
        # LN-applied activation tiles with a persistent ones bias row
        xns = [wpool.tile([C + 1, CH], f32r, tag=f"xn{i}", name=f"xn{i}")
               for i in range(2)]
        # f32 scratch (init-only) to produce properly-rounded f32r constants
        initp = tc.alloc_tile_pool(name="initp", bufs=1)
        onesrow_f = initp.tile([1, CH], f32)
        nc.vector.memset(onesrow_f, 1.0)
        nc.vector.tensor_copy(out=onesrow, in_=onesrow_f)
        ones3f = initp.tile([C, C], f32)
        nc.vector.memset(ones3f, 0.0)
        for j in range(3):
            nc.vector.memset(ones3f[32 * j:32 * j + 1, :], 1.0)
        nc.vector.tensor_copy(out=ones3x, in_=ones3f)
        for i in range(2):
            nc.vector.tensor_copy(out=xns[i][C:C + 1, :], in_=onesrow_f)
        initp.release()
        # LN stats rows: rs/nb per chunk on partitions [32, 512]
        # (LN2 pass reuses the same tiles after LN1's readers are done)
        rs1 = wpool.tile([NCHUNK, CH], f32r)
        nb1 = wpool.tile([NCHUNK, CH], f32r)
        rs2, nb2 = rs1, nb1
        # channel-sum / channel-sumsq strips (chunk on partitions)
        ss1 = wpool.tile([NCHUNK, CH], f32)
        sq1 = wpool.tile([NCHUNK, CH], f32)
        ss2, sq2 = ss1, sq1

        # big persistent activation tensors
        apool = tc.alloc_tile_pool(name="acts", bufs=1)
        Qd = [apool.tile([128, BH, W], bf16, tag=f"qd{d}", name=f"qd{d}") for d in range(3)]
        Kp = [apool.tile([128, PADR, PADC], bf16, tag=f"kp{d}", name=f"kp{d}") for d in range(3)]
        Vp = [apool.tile([128, PADR, PADC], bf16, tag=f"vp{d}", name=f"vp{d}") for d in range(3)]


        # ============ shared stats math ============
        # strips ss/sq: [32, CH] SBUF, chunk on partitions.  Runs on row
        # slices [c0:c0+n] right after each flush so downstream chunks can
        # start without waiting for the whole stats pass.  mu overwrites ss,
        # var/sd overwrite sq in place; vtmp holds -mu^2.
        vtmp = wpool.tile([NCHUNK, CH], f32)

        def stats_math_early(ss, sq, rs, nb):
            sl = slice(0, 8)
            mu = vtmp[sl, :]
            nc.scalar.mul(out=mu, in_=ss[sl, :], mul=1.0 / C)
            t = rs[sl, :].bitcast(f32)
            nc.scalar.mul(out=t, in_=sq[sl, :], mul=1.0 / C)
            nc.vector.scalar_tensor_tensor(out=nb[sl, :].bitcast(f32), in0=mu,
                                           scalar=-1.0, in1=mu,
                                           op0=ALU.mult, op1=ALU.mult)
            nc.vector.tensor_tensor(out=t, in0=t, in1=nb[sl, :].bitcast(f32),
                                    op=ALU.add)
            nc.scalar.activation(out=t, in_=t, func=AF.Sqrt,
                                 bias=epst[0:8, 0:1], scale=1.0)
            with nc.allow_low_precision(reason="f32r-typed LN stats rows"):
                nc.vector.reciprocal(out=rs[sl, :], in_=t)
                nc.vector.scalar_tensor_tensor(out=nb[sl, :], in0=mu,
                                               scalar=-1.0,
                                               in1=rs[sl, :].bitcast(f32),
                                               op0=ALU.mult, op1=ALU.mult)

        def stats_math(ss, sq, rs, nb, c0, n):
            sl = slice(c0, c0 + n)
            nc.scalar.mul(out=ss[sl, :], in_=ss[sl, :], mul=1.0 / C)
            nc.scalar.mul(out=sq[sl, :], in_=sq[sl, :], mul=1.0 / C)
            nc.vector.scalar_tensor_tensor(out=vtmp[sl, :], in0=ss[sl, :],
                                           scalar=-1.0, in1=ss[sl, :],
                                           op0=ALU.mult, op1=ALU.mult)
            nc.vector.tensor_tensor(out=sq[sl, :], in0=sq[sl, :],
                                    in1=vtmp[sl, :], op=ALU.add)
            nc.scalar.activation(out=sq[sl, :], in_=sq[sl, :], func=AF.Sqrt,
                                 bias=epst[c0:c0 + n, 0:1], scale=1.0)
            with nc.allow_low_precision(reason="f32r-typed LN stats rows"):
                nc.vector.reciprocal(out=rs[sl, :], in_=sq[sl, :])
                nc.vector.scalar_tensor_tensor(out=nb[sl, :], in0=ss[sl, :],
                                               scalar=-1.0,
                                               in1=rs[sl, :].bitcast(f32),
                                               op0=ALU.mult, op1=ALU.mult)

        # strip helper: drain a [1, n*CH] psum strip (partition 0) to a
        # 1-partition SBUF stage, then one DMA reshapes it into rows
        # [c0:c0+n] of the compact [32, CH] stats tiles.
        def strip_flush(pool, ps_s, ps_q, ss, sq, c0, n, qeng=None):
            stg_s = pool.tile([1, n * CH], f32, tag="stg_s")
            nc.scalar.copy(stg_s, ps_s[:, 0:n * CH])
            stg_q = pool.tile([1, n * CH], f32, tag="stg_q")
            if qeng is None:
                nc.scalar.copy(stg_q, ps_q[:, 0:n * CH])
            else:
                qeng.tensor_copy(out=stg_q, in_=ps_q[:, 0:n * CH])
            nc.sync.dma_start(out=ss[c0:c0 + n, :], in_=stg_s)
            nc.sync.dma_start(out=sq[c0:c0 + n, :], in_=stg_q)

        # ============ PH1: LN1 stats sweep ============
        with tc.tile_pool(name="ph1", bufs=3) as pool, \
             tc.tile_pool(name="ph1st", bufs=2) as sgpool, \
             tc.tile_pool(name="ph1ps", bufs=2, space="PSUM") as stps:
            ps_s = ps_q = None
            for g in range(NCHUNK // 4):
                xt4 = pool.tile([C, 4, CH], f32r, tag="xt")
                nc.sync.dma_start(out=xt4,
                                  in_=x_d[:, 16 * g:16 * g + 16, :].bitcast(f32r))
                for i in range(4):
                    c = 4 * g + i
                    if i % 2 == 0:
                        ps_s = stps.tile([1, 2 * CH], f32, tag="ps_s")
                        ps_q = stps.tile([1, 2 * CH], f32, tag="ps_q")
                    h = CH * (i % 2)
                    xt = xt4[:, i, :]
                    nc.tensor.matmul(ps_s[:, h:h + CH], lhsT=onescol,
                                     rhs=xt, start=True, stop=True)
                    xf = xt.bitcast(f32)
                    xsq = pool.tile([C, CH], f32r, tag="xsq")
                    nc.vector.tensor_tensor(out=xsq, in0=xf, in1=xf, op=ALU.mult)
                    nc.tensor.matmul(ps_q[:, h:h + CH], lhsT=onescol,
                                     rhs=xsq, start=True, stop=True)
                    if i % 2 == 1:
                        strip_flush(sgpool, ps_s, ps_q, ss1, sq1, c - 1, 2,
                                    qeng=nc.vector if g % 2 == 0 else None)
                    if i == 3:
                        if g == 7:
                            stats_math(ss1, sq1, rs1, nb1, 0, NCHUNK)

        # zero only the pad strips (interior fully overwritten by repack)
        for d in range(3):
            for t in (Kp[d], Vp[d]):
                nc.gpsimd.memset(t[:, 0:3, :], 0.0)
                nc.gpsimd.memset(t[:, 35:38, :], 0.0)
                nc.gpsimd.memset(t[:, 3:35, 0:3], 0.0)
                nc.gpsimd.memset(t[:, 3:35, 3 + W:PADC], 0.0)

        stgpool = tc.alloc_tile_pool(name="stg", bufs=1)
        stg_k = stgpool.tile([C, N], bf16)
        stg_v = stgpool.tile([C, N], bf16)


        # ============ PH2: LN1 apply + qkv + stage/scatter ============
        def band_rows(b):
            lo = max(0, BH * b - 3)
            hi = min(H, BH * b + BH + 3)
            return lo, hi

        # stage rs/nb rows at partitions {0,32,64} so rank-1 matmuls can
        # read them (PE base-partition rule); one strided DMA per 3 chunks
        def stage_stats(sgp, rs, nb, c0):
            n = min(3, NCHUNK - c0)
            srs = sgp.tile([C, CH], f32r, tag="srs")
            snb = sgp.tile([C, CH], f32r, tag="snb")
            dst_rs = srs.rearrange("(a b) f -> a b f", a=3)[0:n, 0:1, :]
            dst_nb = snb.rearrange("(a b) f -> a b f", a=3)[0:n, 0:1, :]
            nc.sync.dma_start(out=dst_rs, in_=rs[c0:c0 + n, :])
            nc.sync.dma_start(out=dst_nb, in_=nb[c0:c0 + n, :])
            return srs, snb

        with tc.tile_pool(name="ph2", bufs=3) as pool, \
             tc.tile_pool(name="ph2t", bufs=2) as tpool, \
             tc.tile_pool(name="ph2sg", bufs=2) as sgp, \
             tc.tile_pool(name="ph2ps", bufs=2, space="PSUM") as psum, \
             tc.tile_pool(name="ph2ps2", bufs=2, space="PSUM") as psum2:
            srs = snb = None
            for c in range(NCHUNK):
                g, i = c // 4, c % 4
                if i == 0:
                    xt4 = pool.tile([C, 4, CH], f32, tag="xt2")
                    nc.sync.dma_start(out=xt4, in_=x_d[:, 16 * g:16 * g + 16, :])
                if c % 3 == 0:
                    srs, snb = stage_stats(sgp, rs1, nb1, c)
                j = c % 3
                xt = xt4[:, i, :]
                pa = psum2.tile([C, CH], f32, tag="pa")
                nc.tensor.matmul(pa, lhsT=ones3x[32 * j:32 * j + 1, :],
                                 rhs=srs[32 * j:32 * j + 1, :],
                                 start=True, stop=True)
                xn = xns[c % 2]
                nc.vector.tensor_tensor(out=xn[0:C, :], in0=xt, in1=pa,
                                        op=ALU.mult)

                pq = psum.tile([C, CH], f32, tag="pq")
                pk = psum.tile([C, CH], f32, tag="pk")
                pv = psum.tile([C, CH], f32, tag="pv")
                snbj = snb[32 * j:32 * j + 1, :]
                for t, pt in enumerate((pq, pk, pv)):
                    nc.tensor.matmul(pt, lhsT=wsum3[32 * j:32 * j + 1,
                                                    C * t:C * (t + 1)],
                                     rhs=snbj, start=True, stop=False)
                    nc.tensor.matmul(pt, lhsT=wq[:, C * t:C * (t + 1)], rhs=xn,
                                     start=False, stop=True)

                # K/V -> contiguous staging (single 96-partition copies);
                # gpsimd cannot touch PSUM, so drains go DVE/Act only
                nc.scalar.copy(stg_k[:, CH * c:CH * (c + 1)], pk)
                if c % 2 == 0:
                    nc.vector.tensor_copy(out=stg_v[:, CH * c:CH * (c + 1)], in_=pv)
                else:
                    nc.scalar.copy(stg_v[:, CH * c:CH * (c + 1)], pv)
                # Q -> band-packed planes directly
                b = c // 8
                r_off = 4 * c - BH * b
                for d in range(3):
                    src = pq[32 * d:32 * d + 32, :].rearrange("p (r w) -> p r w", r=4)
                    dst = Qd[d][32 * b:32 * b + 32, r_off:r_off + 4, :]
                    if d == 0 or (d == 2 and c % 2 == 1):
                        nc.vector.tensor_copy(out=dst, in_=src)
                    else:
                        nc.scalar.copy(dst, src)

                # band-packed K/V repack via SBUF->SBUF DMA on idle rings
                if c in (8, 16, 24, 31):
                    b_ = (c - 1) // 8
                    lo, hi = band_rows(b_)
                    nr = hi - lo
                    r0 = lo - (BH * b_ - 3)
                    for d in range(3):
                        for stg, dstp in ((stg_k, Kp[d]), (stg_v, Vp[d])):
                            nc.sync.dma_start(
                                out=dstp[32 * b_:32 * b_ + 32, r0:r0 + nr, 3:3 + W],
                                in_=stg[32 * d:32 * d + 32, W * lo:W * hi]
                                    .rearrange("p (r w) -> p r w", r=nr))

        stgpool.release()

        # attention output (channel-major), in space freed by the staging
        ofpool = tc.alloc_tile_pool(name="ofp", bufs=1)
        ofull = ofpool.tile([C, N], bf16)

        # ============ PH3: attention per dilation ============
        # Processed in half-planes (16 band-rows each) so the softmax
        # denominator S accumulates across taps in PSUM via PE matmuls with
        # srepl (= repl/16), freeing DVE of the S adds entirely.
        HF = BH * W // 2                                # 2048 tokens per half
        with tc.tile_pool(name="ph3", bufs=4) as pool, \
             tc.tile_pool(name="ph3f", bufs=4) as fpool, \
             tc.tile_pool(name="ph3acc", bufs=3) as acc, \
             tc.tile_pool(name="ph3ps", bufs=2, space="PSUM") as psum, \
             tc.tile_pool(name="ph3sps", bufs=1, space="PSUM") as spsum:
            for di, dil in enumerate(DILS):
                for hh in range(2):
                    S_ps = spsum.tile([128, HF], f32, tag="Sps")
                    Oab = [acc.tile([128, HF], bf16, tag=f"O{p}",
                                    name=f"O{p}_{di}_{hh}") for p in range(2)]
                    rcp = acc.tile([128, HF], bf16, tag="rcp",
                                   name=f"rcp_{di}_{hh}")
                    qv = Qd[di][:, 16 * hh:16 * hh + 16, :]
                    for ti, (dr, dc) in enumerate([(i - 1, j - 1)
                                                   for i in range(3) for j in range(3)]):
                        r0 = 3 + dr * dil + 16 * hh
                        kwin = Kp[di][:, r0:r0 + 16, 3 + dc * dil:3 + dc * dil + W]
                        vwin = Vp[di][:, r0:r0 + 16, 3 + dc * dil:3 + dc * dil + W]
                        P = fpool.tile([128, HF], bf16, tag="P")
                        nc.vector.tensor_tensor(
                            out=P.rearrange("p (r w) -> p r w", r=16),
                            in0=qv, in1=kwin, op=ALU.mult)
                        # logits -> exp (overwrites P) -> S accumulation
                        for q in range(2):
                            pl = psum.tile([128, 1024], f32, tag="pl")
                            for j in range(2):
                                nc.tensor.matmul(
                                    pl[:, 512 * j:512 * (j + 1)], lhsT=repl,
                                    rhs=P[:, 1024 * q + 512 * j:
                                          1024 * q + 512 * (j + 1)],
                                    start=True, stop=True)
                            nc.scalar.activation(
                                out=P[:, 1024 * q:1024 * (q + 1)], in_=pl,
                                func=AF.Exp)
                            for j in range(2):
                                nc.tensor.matmul(
                                    S_ps[:, 1024 * q + 512 * j:
                                         1024 * q + 512 * (j + 1)],
                                    lhsT=srepl,
                                    rhs=P[:, 1024 * q + 512 * j:
                                          1024 * q + 512 * (j + 1)],
                                    start=(ti == 0), stop=(ti == 8))
                        ev = P.rearrange("p (r w) -> p r w", r=16)
                        Pv = Oab[ti] if ti < 2 else pool.tile([128, HF], bf16,
                                                              tag="Pv")
                        nc.vector.tensor_tensor(
                            out=Pv.rearrange("p (r w) -> p r w", r=16),
                            in0=ev, in1=vwin, op=ALU.mult)
                        if ti >= 2:
                            eng = nc.gpsimd if ti in (3, 5, 7) else nc.vector
                            eng.tensor_tensor(out=Oab[ti % 2], in0=Oab[ti % 2],
                                              in1=Pv, op=ALU.add)
                    with nc.allow_low_precision(reason="softmax recip bf16"):
                        nc.vector.reciprocal(out=rcp, in_=S_ps)
                    nc.gpsimd.tensor_tensor(out=Oab[0], in0=Oab[0], in1=Oab[1],
                                             op=ALU.add)
                    nc.vector.tensor_tensor(out=Oab[0], in0=Oab[0], in1=rcp,
                                            op=ALU.mult)
                    for b in range(NB):
                        nc.sync.dma_start(
                            out=ofull[32 * di:32 * di + 32,
                                      4096 * b + 2048 * hh:
                                      4096 * b + 2048 * hh + 2048],
                            in_=Oab[0][32 * b:32 * b + 32, :])

        # ============ PH4: proj + residual + LN2 stats ============
        r1pool = tc.alloc_tile_pool(name="r1p", bufs=1)
        r1 = r1pool.tile([C, N], bf16)
        with tc.tile_pool(name="ph4", bufs=2) as pool, \
             tc.tile_pool(name="ph4sg", bufs=1) as sgpool, \
             tc.tile_pool(name="ph4st", bufs=1, space="PSUM") as stps, \
             tc.tile_pool(name="ph4ps", bufs=2, space="PSUM") as psum:
            ps_s = ps_q = None
            for c in range(NCHUNK):
                g, i = c // 4, c % 4
                if i == 0:
                    xt4 = pool.tile([C, 4, CH], f32, tag="xt4")
                    nc.sync.dma_start(out=xt4, in_=x_d[:, 16 * g:16 * g + 16, :])
                if c % 2 == 0:
                    ps_s = stps.tile([1, 2 * CH], f32, tag="ps_s4")
                    ps_q = stps.tile([1, 2 * CH], f32, tag="ps_q4")
                h4 = CH * (c % 2)
                pp = psum.tile([C, CH], f32, tag="pp")
                nc.tensor.matmul(pp, lhsT=pbt, rhs=onesrow, start=True, stop=False)
                nc.tensor.matmul(pp, lhsT=wp, rhs=ofull[:, CH * c:CH * (c + 1)],
                                 start=False, stop=True)
                rsl = r1[:, CH * c:CH * (c + 1)]
                nc.vector.tensor_tensor(out=rsl, in0=xt4[:, i, :],
                                        in1=pp, op=ALU.add)
                nc.tensor.matmul(ps_s[:, h4:h4 + CH], lhsT=onescol_b,
                                 rhs=rsl, start=True, stop=True)
                xsq = pool.tile([C, CH], f32r, tag="xsq5")
                nc.vector.tensor_tensor(out=xsq, in0=rsl, in1=rsl, op=ALU.mult)
                nc.tensor.matmul(ps_q[:, h4:h4 + CH], lhsT=onescol,
                                 rhs=xsq, start=True, stop=True)
                if c % 2 == 1:
                    strip_flush(sgpool, ps_s, ps_q, ss2, sq2, c - 1, 2)
                    if c == 31:
                        stats_math(ss2, sq2, rs2, nb2, 0, NCHUNK)

        # ============ PH5: MLP + residual ============
        with tc.tile_pool(name="ph5b", bufs=3) as pool, \
             tc.tile_pool(name="ph5h", bufs=2) as hpool, \
             tc.tile_pool(name="ph5y", bufs=2) as ypool, \
             tc.tile_pool(name="ph5sg", bufs=2) as sgp, \
             tc.tile_pool(name="ph5ps", bufs=2, space="PSUM") as psum, \
             tc.tile_pool(name="ph5ps2", bufs=1, space="PSUM") as psum2:
            srs = snb = None
            for c in range(NCHUNK):
                g, i = c // 4, c % 4
                rsl = r1[:, CH * c:CH * (c + 1)]
                if i == 0:
                    yout4 = ypool.tile([C, 4, CH], f32, tag="yout4")
                if c % 3 == 0:
                    srs, snb = stage_stats(sgp, rs2, nb2, c)
                j = c % 3
                pa = psum2.tile([C, CH], f32, tag="pa5")
                nc.tensor.matmul(pa, lhsT=ones3x[32 * j:32 * j + 1, :],
                                 rhs=srs[32 * j:32 * j + 1, :],
                                 start=True, stop=True)
                pb2 = psum2.tile([C, CH], f32, tag="pb5")
                nc.tensor.matmul(pb2, lhsT=ones3x[32 * j:32 * j + 1, :],
                                 rhs=snb[32 * j:32 * j + 1, :],
                                 start=True, stop=True)
                t1 = pool.tile([C, CH], f32, tag="t15")
                nc.vector.tensor_tensor(out=t1, in0=rsl, in1=pa, op=ALU.mult)
                xn = xns[c % 2]
                nc.vector.tensor_tensor(out=xn[0:C, :], in0=t1,
                                        in1=pb2, op=ALU.add)

                h1 = hpool.tile([128, 3, CH], bf16, tag="h1")
                for j in range(3):
                    pf = psum.tile([128, CH], f32, tag="pf")
                    nc.tensor.matmul(pf, lhsT=w1[:, 128 * j:128 * (j + 1)], rhs=xn,
                                     start=True, stop=True)
                    nc.scalar.activation(out=h1[:, j, :], in_=pf, func=AF.Gelu)
                pm = psum.tile([C, CH], f32, tag="pm")
                nc.tensor.matmul(pm, lhsT=b2t, rhs=onesrow, start=True, stop=False)
                for j in range(3):
                    nc.tensor.matmul(pm, lhsT=w2[j],
                                     rhs=h1[:, j, :], start=False, stop=(j == 2))
                nc.vector.tensor_tensor(out=yout4[:, i, :], in0=rsl,
                                        in1=pm, op=ALU.add)
                if i == 3:
                    nc.sync.dma_start(out=y_d[:, 16 * g:16 * g + 16, :], in_=yout4)

        r1pool.release()
        ofpool.release()
        apool.release()
        wpool.release()

    _split_multi_waits(nc, mybir)
    return nc


def _prep_weights(inputs):
    """Host-side weight preparation (fold LN affine, scale, bias rows)."""
    qkv_w = np.asarray(inputs['qkv_w'], np.float32)       # (288, 96)
    proj_w = np.asarray(inputs['proj_w'], np.float32)     # (96, 96)
    proj_b = np.asarray(inputs['proj_b'], np.float32)
    ln1_w = np.asarray(inputs['ln1_w'], np.float32)
    ln1_b = np.asarray(inputs['ln1_b'], np.float32)
    ln2_w = np.asarray(inputs['ln2_w'], np.float32)
    ln2_b = np.asarray(inputs['ln2_b'], np.float32)
    fc1_w = np.asarray(inputs['fc1_w'], np.float32)       # (384, 96)
    fc1_b = np.asarray(inputs['fc1_b'], np.float32)
    fc2_w = np.asarray(inputs['fc2_w'], np.float32)       # (96, 384)
    fc2_b = np.asarray(inputs['fc2_b'], np.float32)

    wq = qkv_w * ln1_w[None, :]                            # (288, 96)
    c0 = qkv_w @ ln1_b                                     # (288,)
    wq[0:C] *= SCALE                                       # scale q rows
    c0[0:C] *= SCALE
    wqb = np.concatenate([wq.T, c0[None, :]], axis=0)      # (97, 288)

    w1 = fc1_w * ln2_w[None, :]
    c1 = fc1_w @ ln2_b + fc1_b
    w1b = np.concatenate([w1.T, c1[None, :]], axis=0)      # (97, 384)

    repl = np.zeros((128, 128), np.float32)
    for b in range(NB):
        for ch in range(GD):
            h0 = (ch // HD) * HD
            repl[32 * b + h0:32 * b + h0 + HD, 32 * b + ch] = 1.0

    wsum3 = np.zeros((C, 3 * C), np.float32)
    for j in range(3):
        wsum3[32 * j, :] = wq.T[0:C, :].sum(axis=0)        # per-column sums

    return {
        'wsum3': wsum3,
        'wqkv': np.ascontiguousarray(wqb),                 # (97, 288) lhsT
        'wproj': np.ascontiguousarray(proj_w.T),           # (96, 96) lhsT
        'projb': proj_b.reshape(-1, 1).astype(np.float32),
        'w1': np.ascontiguousarray(w1b),                   # (97, 384) lhsT
        'w2': np.ascontiguousarray(fc2_w.T),               # (384, 96) lhsT
        'b2': fc2_b.reshape(-1, 1).astype(np.float32),
        'repl': repl,
        'onesc': np.ones((C, 1), np.float32),
    }


def kernel(**inputs):
    from concourse.bass_utils import run_bass_kernel_spmd

    if 'nc' not in _cache:
        t0 = time.time()
        _cache['nc'] = _build()
        print(f"[kernel] built bass module in {time.time() - t0:.1f}s",
              file=sys.stderr)

    nc = _cache['nc']
    wmap = _prep_weights(inputs)
    x = np.asarray(inputs['x'], np.float32)                # (8, 96, 128, 128)

    in_maps = []
    for b in range(B):
        m = {'x': np.ascontiguousarray(x[b])}
        m.update(wmap)
        in_maps.append(m)

    res = run_bass_kernel_spmd(nc, in_maps, core_ids=list(range(B)))
    _cache['last_exec_ns'] = res.exec_time_ns
    out = np.stack([res.results[b]['y'] for b in range(B)], axis=0)
    return out.astype(np.float32)


# revision 88
# speedup vs baseline: 1.0621x; 1.0072x over previous
"""DilateBlock kernel for 8x Trainium2 NeuronCores (Bass/Tile).

Data-parallel over batch B=8 (one image per core). Per core, the whole block
(LN1 -> qkv -> 3-dilation 3x3 neighborhood attention -> proj -> residual ->
LN2 -> MLP -> residual) runs in channels-on-partitions layout; spatial shifts
for the attention unfold live on the free dimension of zero-padded (h, w)
planes, packed 4-hbands x 32-channels across partitions.

Key tricks:
  - LayerNorm stats via ones-matmul on PE into a [32, 512] PSUM strip tile
    (chunk index on partitions), stats math runs wide on 32 partitions, and
    the per-token scale/shift rows feed rank-1 PSUM matmuls directly.
  - qkv/fc1 biases folded into the matmuls via a 97th ones-row of the
    LN-applied activations (contract-97 lhsT with a bias row).
  - K/V drained to contiguous staging as single 96-partition copies, then
    band-packed into padded planes by SBUF->SBUF DMAs on the idle DMA rings.
  - QK tap logits reduced over head_dim AND replicated back to all 16
    channel rows in one PE matmul with a static block-ones matrix; exp runs
    full width on Act; softmax denominator and output accumulate across taps
    via gpsimd DMA-accumulate (even/odd partial tiles, merged on DVE).
  - Softmax normalization applied to the attention OUTPUT.
  - Attention output repacked in SBUF (no DRAM roundtrip) for the proj.
"""
import sys
import time

sys.path.insert(0, '/opt/trn_rl_repo')

import numpy as np

# ---- problem constants (hardcoded per contract) ----
B, C, H, W = 8, 96, 128, 128
DILS = (1, 2, 3)
GD = 32                 # channels per dilation branch
HD = 16                 # head dim
NB = 4                  # h-bands packed on partitions
BH = H // NB            # rows per band = 32
N = H * W               # tokens per image
NCHUNK = 32             # token chunks of 512 (4 image rows each)
CH = N // NCHUNK        # 512
PADR = 38               # BH + 6 halo rows
PADC = 136              # W + 8 halo cols (EVEN pitch: enables DVE 2x mode)
EPS = 1e-5
SCALE = HD ** -0.5
MLPH = 384

_cache = {}
import os
_USE_DMA_ACCUM = os.environ.get('KDMA', '0') == '1'


def _patch_tile(tile_mod, bass_mod):
    """Work around this walrus build's 1-sem-wait-per-instruction limit and
    the multi-wait tail drain."""
    from concourse.vector_clock import ScopedClock, VectorClock

    def _drain_and_barrier(self, tick_clock, wait_clock):
        vclock = tick_clock.global_clock
        n = len(vclock)
        idxs = [i for i in range(n) if vclock[i] > 0]
        for i in idxs:
            vec = [0] * n
            vec[i] = vclock[i]
            nop_inst = self.nc.sync.nop(nofuse=True)
            wait_clock.add_sem_waits(nop_inst.ins,
                                     ScopedClock({None: VectorClock(vec)}))
        self.nc.sync.drain()
        self.nc.all_engine_barrier()
        popped = self.nc._tile_sem_poison_stack.pop()
        assert popped is self._sem_poison
        self.nc.clear_and_free_semaphores(list(self.sems.allocated().values()))
        self.nc.all_engine_barrier()

    tile_mod.TileContext._drain_and_barrier = _drain_and_barrier


_ws_counter = [0]


def _split_multi_waits(nc, mybir):
    for fn in nc.m.functions:
        for blk in fn.blocks:
            insts = list(blk.instructions)
            out = []
            changed = False
            for inst in insts:
                si = inst.sync_info
                waits = list(si.on_wait) if si and si.on_wait else []
                if len(waits) > 1:
                    for w in waits[:-1]:
                        _ws_counter[0] += 1
                        out.append(mybir.InstNoOp(
                            name=f"I-ws-{_ws_counter[0]}",
                            engine=inst.engine, ins=[], outs=[],
                            sync_info=mybir.SyncInfo(on_wait=[w], on_update=[])))
                    si.on_wait = [waits[-1]]
                    changed = True
                out.append(inst)
            if changed:
                blk.instructions[:] = out


def _build():
    import concourse.bass as bass
    import concourse.tile as tile
    from concourse import mybir

    _patch_tile(tile, bass)

    f32 = mybir.dt.float32
    f32r = mybir.dt.float32r
    bf16 = mybir.dt.bfloat16
    AF = mybir.ActivationFunctionType
    ALU = mybir.AluOpType

    nc = bass.Bass()

    # ---- DRAM I/O ----
    x_d = nc.dram_tensor("x", (C, H, W), f32, kind="ExternalInput")
    wq_d = nc.dram_tensor("wqkv", (C + 1, 3 * C), f32, kind="ExternalInput")  # lhsT+bias
    wp_d = nc.dram_tensor("wproj", (C, C), f32, kind="ExternalInput")         # lhsT
    pb_d = nc.dram_tensor("projb", (C, 1), f32, kind="ExternalInput")
    w1_d = nc.dram_tensor("w1", (C + 1, MLPH), f32, kind="ExternalInput")     # lhsT+bias
    w2_d = nc.dram_tensor("w2", (MLPH, C), f32, kind="ExternalInput")         # lhsT
    b2_d = nc.dram_tensor("b2", (C, 1), f32, kind="ExternalInput")
    repl_d = nc.dram_tensor("repl", (128, 128), f32, kind="ExternalInput")
    ws_d = nc.dram_tensor("wsum3", (C, 3 * C), f32, kind="ExternalInput")
    ones_d = nc.dram_tensor("onesc", (C, 1), f32, kind="ExternalInput")

    y_d = nc.dram_tensor("y", (C, H, W), f32, kind="ExternalOutput")

    with tile.TileContext(nc) as tc:
        # ---------------- persistent pools ----------------
        wpool = tc.alloc_tile_pool(name="weights", bufs=1)
        wq = wpool.tile([C + 1, 3 * C], f32r)
        nc.sync.dma_start(out=wq, in_=wq_d[:, :].bitcast(f32r))
        wp = wpool.tile([C, C], bf16)
        nc.gpsimd.dma_start(out=wp, in_=wp_d[:, :])     # gpsimd dma casts
        pbt = wpool.tile([1, C], f32r)                  # proj bias as rank-1 lhsT
        nc.sync.dma_start(out=pbt, in_=pb_d[:, :].rearrange("a b -> b a").bitcast(f32r))
        w1 = wpool.tile([C + 1, MLPH], f32r)
        nc.sync.dma_start(out=w1, in_=w1_d[:, :].bitcast(f32r))
        w2 = [wpool.tile([128, C], bf16, tag=f"w2{i}", name=f"w2{i}") for i in range(3)]
        for i in range(3):
            nc.gpsimd.dma_start(out=w2[i], in_=w2_d[128 * i:128 * (i + 1), :])
        b2t = wpool.tile([1, C], f32r)                  # fc2 bias as rank-1 lhsT
        nc.sync.dma_start(out=b2t, in_=b2_d[:, :].rearrange("a b -> b a").bitcast(f32r))
        repl = wpool.tile([128, 128], bf16)
        nc.gpsimd.dma_start(out=repl, in_=repl_d[:, :])
        srepl = wpool.tile([128, 128], bf16)            # repl/16: S accumulation
        nc.scalar.mul(out=srepl, in_=repl, mul=1.0 / HD)
        onescol = wpool.tile([C, 1], f32r)              # stats lhsT [96,1]
        nc.sync.dma_start(out=onescol, in_=ones_d[:, :].bitcast(f32r))
        onescol_b = wpool.tile([C, 1], bf16)            # bf16 variant (bf16 rhs)
        nc.vector.memset(onescol_b, 1.0)
        wsum3 = wpool.tile([C, 3 * C], f32r)            # qkv col-sums at {0,32,64}
        nc.sync.dma_start(out=wsum3, in_=ws_d[:, :].bitcast(f32r))
        ones1x = wpool.tile([1, C], f32r)               # rank-1 lhsT [1,96]
        nc.sync.dma_start(out=ones1x, in_=ones_d[:, :].rearrange("a b -> b a").bitcast(f32r))
        onesrow = wpool.tile([1, CH], f32r)             # static ones row (f32r)
        ones3x = wpool.tile([C, C], f32r)               # ones rows at {0,32,64}
        epst = wpool.tile([128, 1], f32)
        nc.vector.memset(epst, EPS)
        # LN-applied activation tiles with a persistent ones bias row
        xns = [wpool.tile([C + 1, CH], f32r, tag=f"xn{i}", name=f"xn{i}")
               for i in range(2)]
        # f32 scratch (init-only) to produce properly-rounded f32r constants
        initp = tc.alloc_tile_pool(name="initp", bufs=1)
        onesrow_f = initp.tile([1, CH], f32)
        nc.vector.memset(onesrow_f, 1.0)
        nc.vector.tensor_copy(out=onesrow, in_=onesrow_f)
        ones3f = initp.tile([C, C], f32)
        nc.vector.memset(ones3f, 0.0)
        for j in range(3):
            nc.vector.memset(ones3f[32 * j:32 * j + 1, :], 1.0)
        nc.vector.tensor_copy(out=ones3x, in_=ones3f)
        for i in range(2):
            nc.vector.tensor_copy(out=xns[i][C:C + 1, :], in_=onesrow_f)
        initp.release()
        # LN stats rows: rs/nb per chunk on partitions [32, 512]
        # (LN2 pass reuses the same tiles after LN1's readers are done)
        rs1 = wpool.tile([NCHUNK, CH], f32r)
        nb1 = wpool.tile([NCHUNK, CH], f32r)
        rs2, nb2 = rs1, nb1
        # channel-sum / channel-sumsq strips (chunk on partitions)
        ss1 = wpool.tile([NCHUNK, CH], f32)
        sq1 = wpool.tile([NCHUNK, CH], f32)
        ss2, sq2 = ss1, sq1

        # big persistent activation tensors
        apool = tc.alloc_tile_pool(name="acts", bufs=1)
        Qd = [apool.tile([128, BH, W], bf16, tag=f"qd{d}", name=f"qd{d}") for d in range(3)]
        Kp = [apool.tile([128, PADR, PADC], bf16, tag=f"kp{d}", name=f"kp{d}") for d in range(3)]
        Vp = [apool.tile([128, PADR, PADC], bf16, tag=f"vp{d}", name=f"vp{d}") for d in range(3)]


        # ============ shared stats math ============
        # strips ss/sq: [32, CH] SBUF, chunk on partitions.  Runs on row
        # slices [c0:c0+n] right after each flush so downstream chunks can
        # start without waiting for the whole stats pass.  mu overwrites ss,
        # var/sd overwrite sq in place; vtmp holds -mu^2.
        vtmp = wpool.tile([NCHUNK, CH], f32)

        def stats_math_early(ss, sq, rs, nb):
            sl = slice(0, 8)
            mu = vtmp[sl, :]
            nc.scalar.mul(out=mu, in_=ss[sl, :], mul=1.0 / C)
            t = rs[sl, :].bitcast(f32)
            nc.scalar.mul(out=t, in_=sq[sl, :], mul=1.0 / C)
            nc.vector.scalar_tensor_tensor(out=nb[sl, :].bitcast(f32), in0=mu,
                                           scalar=-1.0, in1=mu,
                                           op0=ALU.mult, op1=ALU.mult)
            nc.vector.tensor_tensor(out=t, in0=t, in1=nb[sl, :].bitcast(f32),
                                    op=ALU.add)
            nc.scalar.activation(out=t, in_=t, func=AF.Sqrt,
                                 bias=epst[0:8, 0:1], scale=1.0)
            with nc.allow_low_precision(reason="f32r-typed LN stats rows"):
                nc.vector.reciprocal(out=rs[sl, :], in_=t)
                nc.vector.scalar_tensor_tensor(out=nb[sl, :], in0=mu,
                                               scalar=-1.0,
                                               in1=rs[sl, :].bitcast(f32),
                                               op0=ALU.mult, op1=ALU.mult)

        def stats_math(ss, sq, rs, nb, c0, n):
            sl = slice(c0, c0 + n)
            nc.scalar.mul(out=ss[sl, :], in_=ss[sl, :], mul=1.0 / C)
            nc.scalar.mul(out=sq[sl, :], in_=sq[sl, :], mul=1.0 / C)
            nc.vector.scalar_tensor_tensor(out=vtmp[sl, :], in0=ss[sl, :],
                                           scalar=-1.0, in1=ss[sl, :],
                                           op0=ALU.mult, op1=ALU.mult)
            nc.vector.tensor_tensor(out=sq[sl, :], in0=sq[sl, :],
                                    in1=vtmp[sl, :], op=ALU.add)
            nc.scalar.activation(out=sq[sl, :], in_=sq[sl, :], func=AF.Sqrt,
                                 bias=epst[c0:c0 + n, 0:1], scale=1.0)
            with nc.allow_low_precision(reason="f32r-typed LN stats rows"):
                nc.vector.reciprocal(out=rs[sl, :], in_=sq[sl, :])
                nc.vector.scalar_tensor_tensor(out=nb[sl, :], in0=ss[sl, :],
                                               scalar=-1.0,
                                               in1=rs[sl, :].bitcast(f32),
                                               op0=ALU.mult, op1=ALU.mult)

        # strip helper: drain a [1, n*CH] psum strip (partition 0) to a
        # 1-partition SBUF stage, then one DMA reshapes it into rows
        # [c0:c0+n] of the compact [32, CH] stats tiles.
        def strip_flush(pool, ps_s, ps_q, ss, sq, c0, n, qeng=None):
            stg_s = pool.tile([1, n * CH], f32, tag="stg_s")
            nc.scalar.copy(stg_s, ps_s[:, 0:n * CH])
            stg_q = pool.tile([1, n * CH], f32, tag="stg_q")
            if qeng is None:
                nc.scalar.copy(stg_q, ps_q[:, 0:n * CH])
            else:
                qeng.tensor_copy(out=stg_q, in_=ps_q[:, 0:n * CH])
            nc.sync.dma_start(out=ss[c0:c0 + n, :], in_=stg_s)
            nc.sync.dma_start(out=sq[c0:c0 + n, :], in_=stg_q)

        # ============ PH1: LN1 stats sweep ============
        with tc.tile_pool(name="ph1", bufs=3) as pool, \
             tc.tile_pool(name="ph1st", bufs=2) as sgpool, \
             tc.tile_pool(name="ph1ps", bufs=2, space="PSUM") as stps:
            ps_s = ps_q = None
            for g in range(NCHUNK // 4):
                xt4 = pool.tile([C, 4, CH], f32r, tag="xt")
                nc.sync.dma_start(out=xt4,
                                  in_=x_d[:, 16 * g:16 * g + 16, :].bitcast(f32r))
                for i in range(4):
                    c = 4 * g + i
                    if i % 2 == 0:
                        ps_s = stps.tile([1, 2 * CH], f32, tag="ps_s")
                        ps_q = stps.tile([1, 2 * CH], f32, tag="ps_q")
                    h = CH * (i % 2)
                    xt = xt4[:, i, :]
                    nc.tensor.matmul(ps_s[:, h:h + CH], lhsT=onescol,
                                     rhs=xt, start=True, stop=True)
                    xf = xt.bitcast(f32)
                    xsq = pool.tile([C, CH], f32r, tag="xsq")
                    nc.vector.tensor_tensor(out=xsq, in0=xf, in1=xf, op=ALU.mult)
                    nc.tensor.matmul(ps_q[:, h:h + CH], lhsT=onescol,
                                     rhs=xsq, start=True, stop=True)
                    if i % 2 == 1:
                        strip_flush(sgpool, ps_s, ps_q, ss1, sq1, c - 1, 2,
                                    qeng=nc.vector if g % 2 == 0 else None)
                    if i == 3:
                        if g == 7:
                            stats_math(ss1, sq1, rs1, nb1, 0, NCHUNK)

        # zero only the pad strips (interior fully overwritten by repack)
        for d in range(3):
            for t in (Kp[d], Vp[d]):
                nc.gpsimd.memset(t[:, 0:3, :], 0.0)
                nc.gpsimd.memset(t[:, 35:38, :], 0.0)
                nc.gpsimd.memset(t[:, 3:35, 0:3], 0.0)
                nc.gpsimd.memset(t[:, 3:35, 3 + W:PADC], 0.0)

        stgpool = tc.alloc_tile_pool(name="stg", bufs=1)
        stg_k = stgpool.tile([C, N], bf16)
        stg_v = stgpool.tile([C, N], bf16)


        # ============ PH2: LN1 apply + qkv + stage/scatter ============
        def band_rows(b):
            lo = max(0, BH * b - 3)
            hi = min(H, BH * b + BH + 3)
            return lo, hi

        # stage rs/nb rows at partitions {0,32,64} so rank-1 matmuls can
        # read them (PE base-partition rule); one strided DMA per 3 chunks
        def stage_stats(sgp, rs, nb, c0):
            n = min(3, NCHUNK - c0)
            srs = sgp.tile([C, CH], f32r, tag="srs")
            snb = sgp.tile([C, CH], f32r, tag="snb")
            dst_rs = srs.rearrange("(a b) f -> a b f", a=3)[0:n, 0:1, :]
            dst_nb = snb.rearrange("(a b) f -> a b f", a=3)[0:n, 0:1, :]
            nc.sync.dma_start(out=dst_rs, in_=rs[c0:c0 + n, :])
            nc.sync.dma_start(out=dst_nb, in_=nb[c0:c0 + n, :])
            return srs, snb

        with tc.tile_pool(name="ph2", bufs=3) as pool, \
             tc.tile_pool(name="ph2t", bufs=2) as tpool, \
             tc.tile_pool(name="ph2sg", bufs=2) as sgp, \
             tc.tile_pool(name="ph2ps", bufs=2, space="PSUM") as psum, \
             tc.tile_pool(name="ph2ps2", bufs=2, space="PSUM") as psum2:
            srs = snb = None
            for c in range(NCHUNK):
                g, i = c // 4, c % 4
                if i == 0:
                    xt4 = pool.tile([C, 4, CH], f32, tag="xt2")
                    nc.sync.dma_start(out=xt4, in_=x_d[:, 16 * g:16 * g + 16, :])
                if c % 3 == 0:
                    srs, snb = stage_stats(sgp, rs1, nb1, c)
                j = c % 3
                xt = xt4[:, i, :]
                pa = psum2.tile([C, CH], f32, tag="pa")
                nc.tensor.matmul(pa, lhsT=ones3x[32 * j:32 * j + 1, :],
                                 rhs=srs[32 * j:32 * j + 1, :],
                                 start=True, stop=True)
                xn = xns[c % 2]
                nc.vector.tensor_tensor(out=xn[0:C, :], in0=xt, in1=pa,
                                        op=ALU.mult)

                pq = psum.tile([C, CH], f32, tag="pq")
                pk = psum.tile([C, CH], f32, tag="pk")
                pv = psum.tile([C, CH], f32, tag="pv")
                snbj = snb[32 * j:32 * j + 1, :]
                for t, pt in enumerate((pq, pk, pv)):
                    nc.tensor.matmul(pt, lhsT=wsum3[32 * j:32 * j + 1,
                                                    C * t:C * (t + 1)],
                                     rhs=snbj, start=True, stop=False)
                    nc.tensor.matmul(pt, lhsT=wq[:, C * t:C * (t + 1)], rhs=xn,
                                     start=False, stop=True)

                # K/V -> contiguous staging (single 96-partition copies);
                # gpsimd cannot touch PSUM, so drains go DVE/Act only
                nc.scalar.copy(stg_k[:, CH * c:CH * (c + 1)], pk)
                if c % 2 == 0:
                    nc.vector.tensor_copy(out=stg_v[:, CH * c:CH * (c + 1)], in_=pv)
                else:
                    nc.scalar.copy(stg_v[:, CH * c:CH * (c + 1)], pv)
                # Q -> band-packed planes directly
                b = c // 8
                r_off = 4 * c - BH * b
                for d in range(3):
                    src = pq[32 * d:32 * d + 32, :].rearrange("p (r w) -> p r w", r=4)
                    dst = Qd[d][32 * b:32 * b + 32, r_off:r_off + 4, :]
                    if d == 0 or (d == 2 and c % 2 == 1):
                        nc.vector.tensor_copy(out=dst, in_=src)
                    else:
                        nc.scalar.copy(dst, src)

                # band-packed K/V repack via SBUF->SBUF DMA on idle rings
                if c in (8, 16, 24, 31):
                    b_ = (c - 1) // 8
                    lo, hi = band_rows(b_)
                    nr = hi - lo
                    r0 = lo - (BH * b_ - 3)
                    for d in range(3):
                        for stg, dstp in ((stg_k, Kp[d]), (stg_v, Vp[d])):
                            nc.sync.dma_start(
                                out=dstp[32 * b_:32 * b_ + 32, r0:r0 + nr, 3:3 + W],
                                in_=stg[32 * d:32 * d + 32, W * lo:W * hi]
                                    .rearrange("p (r w) -> p r w", r=nr))

        stgpool.release()

        # attention output (channel-major), in space freed by the staging
        ofpool = tc.alloc_tile_pool(name="ofp", bufs=1)
        ofull = ofpool.tile([C, N], bf16)

        # ============ PH3: attention per dilation ============
        # Processed in half-planes (16 band-rows each) so the softmax
        # denominator S accumulates across taps in PSUM via PE matmuls with
        # srepl (= repl/16), freeing DVE of the S adds entirely.
        HF = BH * W // 2                                # 2048 tokens per half
        with tc.tile_pool(name="ph3", bufs=4) as pool, \
             tc.tile_pool(name="ph3f", bufs=4) as fpool, \
             tc.tile_pool(name="ph3acc", bufs=3) as acc, \
             tc.tile_pool(name="ph3ps", bufs=2, space="PSUM") as psum, \
             tc.tile_pool(name="ph3sps", bufs=1, space="PSUM") as spsum:
            for di, dil in enumerate(DILS):
                for hh in range(2):
                    S_ps = spsum.tile([128, HF], f32, tag="Sps")
                    Oab = [acc.tile([128, HF], bf16, tag=f"O{p}",
                                    name=f"O{p}_{di}_{hh}") for p in range(2)]
                    rcp = acc.tile([128, HF], bf16, tag="rcp",
                                   name=f"rcp_{di}_{hh}")
                    qv = Qd[di][:, 16 * hh:16 * hh + 16, :]
                    for ti, (dr, dc) in enumerate([(i - 1, j - 1)
                                                   for i in range(3) for j in range(3)]):
                        r0 = 3 + dr * dil + 16 * hh
                        kwin = Kp[di][:, r0:r0 + 16, 3 + dc * dil:3 + dc * dil + W]
                        vwin = Vp[di][:, r0:r0 + 16, 3 + dc * dil:3 + dc * dil + W]
                        P = fpool.tile([128, HF], bf16, tag="P")
                        nc.vector.tensor_tensor(
                            out=P.rearrange("p (r w) -> p r w", r=16),
                            in0=qv, in1=kwin, op=ALU.mult)
                        # logits -> exp (overwrites P) -> S accumulation
                        for q in range(2):
                            pl = psum.tile([128, 1024], f32, tag="pl")
                            for j in range(2):
                                nc.tensor.matmul(
                                    pl[:, 512 * j:512 * (j + 1)], lhsT=repl,
                                    rhs=P[:, 1024 * q + 512 * j:
                                          1024 * q + 512 * (j + 1)],
                                    start=True, stop=True)
                            nc.scalar.activation(
                                out=P[:, 1024 * q:1024 * (q + 1)], in_=pl,
                                func=AF.Exp)
                            for j in range(2):
                                nc.tensor.matmul(
                                    S_ps[:, 1024 * q + 512 * j:
                                         1024 * q + 512 * (j + 1)],
                                    lhsT=srepl,
                                    rhs=P[:, 1024 * q + 512 * j:
                                          1024 * q + 512 * (j + 1)],
                                    start=(ti == 0), stop=(ti == 8))
                        ev = P.rearrange("p (r w) -> p r w", r=16)
                        Pv = Oab[ti] if ti < 2 else pool.tile([128, HF], bf16,
                                                              tag="Pv")
                        nc.vector.tensor_tensor(
                            out=Pv.rearrange("p (r w) -> p r w", r=16),
                            in0=ev, in1=vwin, op=ALU.mult)
                        if ti >= 2:
                            eng = nc.gpsimd if ti in (3, 4, 7) else nc.vector
                            eng.tensor_tensor(out=Oab[ti % 2], in0=Oab[ti % 2],
                                              in1=Pv, op=ALU.add)
                    with nc.allow_low_precision(reason="softmax recip bf16"):
                        nc.vector.reciprocal(out=rcp, in_=S_ps)
                    nc.vector.tensor_tensor(out=Oab[0], in0=Oab[0], in1=Oab[1],
                                            op=ALU.add)
                    nc.vector.tensor_tensor(out=Oab[0], in0=Oab[0], in1=rcp,
                                            op=ALU.mult)
                    for b in range(NB):
                        nc.sync.dma_start(
                            out=ofull[32 * di:32 * di + 32,
                                      4096 * b + 2048 * hh:
                                      4096 * b + 2048 * hh + 2048],
                            in_=Oab[0][32 * b:32 * b + 32, :])

        # ============ PH4: proj + residual + LN2 stats ============
        r1pool = tc.alloc_tile_pool(name="r1p", bufs=1)
        r1 = r1pool.tile([C, N], bf16)
        with tc.tile_pool(name="ph4", bufs=2) as pool, \
             tc.tile_pool(name="ph4sg", bufs=1) as sgpool, \
             tc.tile_pool(name="ph4st", bufs=1, space="PSUM") as stps, \
             tc.tile_pool(name="ph4ps", bufs=2, space="PSUM") as psum:
            ps_s = ps_q = None
            for c in range(NCHUNK):
                g, i = c // 4, c % 4
                if i == 0:
                    xt4 = pool.tile([C, 4, CH], f32, tag="xt4")
                    nc.sync.dma_start(out=xt4, in_=x_d[:, 16 * g:16 * g + 16, :])
                if c % 2 == 0:
                    ps_s = stps.tile([1, 2 * CH], f32, tag="ps_s4")
                    ps_q = stps.tile([1, 2 * CH], f32, tag="ps_q4")
                h4 = CH * (c % 2)
                pp = psum.tile([C, CH], f32, tag="pp")
                nc.tensor.matmul(pp, lhsT=pbt, rhs=onesrow, start=True, stop=False)
                nc.tensor.matmul(pp, lhsT=wp, rhs=ofull[:, CH * c:CH * (c + 1)],
                                 start=False, stop=True)
                rsl = r1[:, CH * c:CH * (c + 1)]
                nc.vector.tensor_tensor(out=rsl, in0=xt4[:, i, :],
                                        in1=pp, op=ALU.add)
                nc.tensor.matmul(ps_s[:, h4:h4 + CH], lhsT=onescol_b,
                                 rhs=rsl, start=True, stop=True)
                xsq = pool.tile([C, CH], f32r, tag="xsq5")
                nc.vector.tensor_tensor(out=xsq, in0=rsl, in1=rsl, op=ALU.mult)
                nc.tensor.matmul(ps_q[:, h4:h4 + CH], lhsT=onescol,
                                 rhs=xsq, start=True, stop=True)
                if c % 2 == 1:
                    strip_flush(sgpool, ps_s, ps_q, ss2, sq2, c - 1, 2)
                    if c == 31:
                        stats_math(ss2, sq2, rs2, nb2, 0, NCHUNK)

        # ============ PH5: MLP + residual ============
        with tc.tile_pool(name="ph5b", bufs=3) as pool, \
             tc.tile_pool(name="ph5h", bufs=2) as hpool, \
             tc.tile_pool(name="ph5y", bufs=2) as ypool, \
             tc.tile_pool(name="ph5sg", bufs=2) as sgp, \
             tc.tile_pool(name="ph5ps", bufs=2, space="PSUM") as psum, \
             tc.tile_pool(name="ph5ps2", bufs=1, space="PSUM") as psum2:
            srs = snb = None
            for c in range(NCHUNK):
                g, i = c // 4, c % 4
                rsl = r1[:, CH * c:CH * (c + 1)]
                if i == 0:
                    yout4 = ypool.tile([C, 4, CH], f32, tag="yout4")
                if c % 3 == 0:
                    srs, snb = stage_stats(sgp, rs2, nb2, c)
                j = c % 3
                pa = psum2.tile([C, CH], f32, tag="pa5")
                nc.tensor.matmul(pa, lhsT=ones3x[32 * j:32 * j + 1, :],
                                 rhs=srs[32 * j:32 * j + 1, :],
                                 start=True, stop=True)
                pb2 = psum2.tile([C, CH], f32, tag="pb5")
                nc.tensor.matmul(pb2, lhsT=ones3x[32 * j:32 * j + 1, :],
                                 rhs=snb[32 * j:32 * j + 1, :],
                                 start=True, stop=True)
                t1 = pool.tile([C, CH], f32, tag="t15")
                nc.vector.tensor_tensor(out=t1, in0=rsl, in1=pa, op=ALU.mult)
                xn = xns[c % 2]
                nc.vector.tensor_tensor(out=xn[0:C, :], in0=t1,
                                        in1=pb2, op=ALU.add)

                h1 = hpool.tile([128, 3, CH], bf16, tag="h1")
                for j in range(3):
                    pf = psum.tile([128, CH], f32, tag="pf")
                    nc.tensor.matmul(pf, lhsT=w1[:, 128 * j:128 * (j + 1)], rhs=xn,
                                     start=True, stop=True)
                    nc.scalar.activation(out=h1[:, j, :], in_=pf, func=AF.Gelu)
                pm = psum.tile([C, CH], f32, tag="pm")
                nc.tensor.matmul(pm, lhsT=b2t, rhs=onesrow, start=True, stop=False)
                for j in range(3):
                    nc.tensor.matmul(pm, lhsT=w2[j],
                                     rhs=h1[:, j, :], start=False, stop=(j == 2))
                nc.vector.tensor_tensor(out=yout4[:, i, :], in0=rsl,
                                        in1=pm, op=ALU.add)
                if i == 3:
                    nc.sync.dma_start(out=y_d[:, 16 * g:16 * g + 16, :], in_=yout4)

        r1pool.release()
        ofpool.release()
        apool.release()
        wpool.release()

    _split_multi_waits(nc, mybir)
    return nc


def _prep_weights(inputs):
    """Host-side weight preparation (fold LN affine, scale, bias rows)."""
    qkv_w = np.asarray(inputs['qkv_w'], np.float32)       # (288, 96)
    proj_w = np.asarray(inputs['proj_w'], np.float32)     # (96, 96)
    proj_b = np.asarray(inputs['proj_b'], np.float32)
    ln1_w = np.asarray(inputs['ln1_w'], np.float32)
    ln1_b = np.asarray(inputs['ln1_b'], np.float32)
    ln2_w = np.asarray(inputs['ln2_w'], np.float32)
    ln2_b = np.asarray(inputs['ln2_b'], np.float32)
    fc1_w = np.asarray(inputs['fc1_w'], np.float32)       # (384, 96)
    fc1_b = np.asarray(inputs['fc1_b'], np.float32)
    fc2_w = np.asarray(inputs['fc2_w'], np.float32)       # (96, 384)
    fc2_b = np.asarray(inputs['fc2_b'], np.float32)

    wq = qkv_w * ln1_w[None, :]                            # (288, 96)
    c0 = qkv_w @ ln1_b                                     # (288,)
    wq[0:C] *= SCALE                                       # scale q rows
    c0[0:C] *= SCALE
    wqb = np.concatenate([wq.T, c0[None, :]], axis=0)      # (97, 288)

    w1 = fc1_w * ln2_w[None, :]
    c1 = fc1_w @ ln2_b + fc1_b
    w1b = np.concatenate([w1.T, c1[None, :]], axis=0)      # (97, 384)

    repl = np.zeros((128, 128), np.float32)
    for b in range(NB):
        for ch in range(GD):
            h0 = (ch // HD) * HD
            repl[32 * b + h0:32 * b + h0 + HD, 32 * b + ch] = 1.0

    wsum3 = np.zeros((C, 3 * C), np.float32)
    for j in range(3):
        wsum3[32 * j, :] = wq.T[0:C, :].sum(axis=0)        # per-column sums

    return {
        'wsum3': wsum3,
        'wqkv': np.ascontiguousarray(wqb),                 # (97, 288) lhsT
        'wproj': np.ascontiguousarray(proj_w.T),           # (96, 96) lhsT
        'projb': proj_b.reshape(-1, 1).astype(np.float32),
        'w1': np.ascontiguousarray(w1b),                   # (97, 384) lhsT
        'w2': np.ascontiguousarray(fc2_w.T),               # (384, 96) lhsT
        'b2': fc2_b.reshape(-1, 1).astype(np.float32),
        'repl': repl,
        'onesc': np.ones((C, 1), np.float32),
    }


def kernel(**inputs):
    from concourse.bass_utils import run_bass_kernel_spmd

    if 'nc' not in _cache:
        t0 = time.time()
        _cache['nc'] = _build()
        print(f"[kernel] built bass module in {time.time() - t0:.1f}s",
              file=sys.stderr)

    nc = _cache['nc']
    wmap = _prep_weights(inputs)
    x = np.asarray(inputs['x'], np.float32)                # (8, 96, 128, 128)

    in_maps = []
    for b in range(B):
        m = {'x': np.ascontiguousarray(x[b])}
        m.update(wmap)
        in_maps.append(m)

    res = run_bass_kernel_spmd(nc, in_maps, core_ids=list(range(B)))
    _cache['last_exec_ns'] = res.exec_time_ns
    out = np.stack([res.results[b]['y'] for b in range(B)], axis=0)
    return out.astype(np.float32)


# revision 94
# speedup vs baseline: 1.0656x; 1.0033x over previous
"""DilateBlock kernel for 8x Trainium2 NeuronCores (Bass/Tile).

Data-parallel over batch B=8 (one image per core). Per core, the whole block
(LN1 -> qkv -> 3-dilation 3x3 neighborhood attention -> proj -> residual ->
LN2 -> MLP -> residual) runs in channels-on-partitions layout; spatial shifts
for the attention unfold live on the free dimension of zero-padded (h, w)
planes, packed 4-hbands x 32-channels across partitions.

Key tricks:
  - LayerNorm stats via ones-matmul on PE into a [32, 512] PSUM strip tile
    (chunk index on partitions), stats math runs wide on 32 partitions, and
    the per-token scale/shift rows feed rank-1 PSUM matmuls directly.
  - qkv/fc1 biases folded into the matmuls via a 97th ones-row of the
    LN-applied activations (contract-97 lhsT with a bias row).
  - K/V drained to contiguous staging as single 96-partition copies, then
    band-packed into padded planes by SBUF->SBUF DMAs on the idle DMA rings.
  - QK tap logits reduced over head_dim AND replicated back to all 16
    channel rows in one PE matmul with a static block-ones matrix; exp runs
    full width on Act; softmax denominator and output accumulate across taps
    via gpsimd DMA-accumulate (even/odd partial tiles, merged on DVE).
  - Softmax normalization applied to the attention OUTPUT.
  - Attention output repacked in SBUF (no DRAM roundtrip) for the proj.
"""
import sys
import time

sys.path.insert(0, '/opt/trn_rl_repo')

import numpy as np

# ---- problem constants (hardcoded per contract) ----
B, C, H, W = 8, 96, 128, 128
DILS = (1, 2, 3)
GD = 32                 # channels per dilation branch
HD = 16                 # head dim
NB = 4                  # h-bands packed on partitions
BH = H // NB            # rows per band = 32
N = H * W               # tokens per image
NCHUNK = 32             # token chunks of 512 (4 image rows each)
CH = N // NCHUNK        # 512
PADR = 38               # BH + 6 halo rows
PADC = 136              # W + 8 halo cols (EVEN pitch: enables DVE 2x mode)
EPS = 1e-5
SCALE = HD ** -0.5
MLPH = 384

_cache = {}
import os
_USE_DMA_ACCUM = os.environ.get('KDMA', '0') == '1'


def _patch_tile(tile_mod, bass_mod):
    """Work around this walrus build's 1-sem-wait-per-instruction limit and
    the multi-wait tail drain."""
    from concourse.vector_clock import ScopedClock, VectorClock

    def _drain_and_barrier(self, tick_clock, wait_clock):
        vclock = tick_clock.global_clock
        n = len(vclock)
        idxs = [i for i in range(n) if vclock[i] > 0]
        for i in idxs:
            vec = [0] * n
            vec[i] = vclock[i]
            nop_inst = self.nc.sync.nop(nofuse=True)
            wait_clock.add_sem_waits(nop_inst.ins,
                                     ScopedClock({None: VectorClock(vec)}))
        self.nc.sync.drain()
        self.nc.all_engine_barrier()
        popped = self.nc._tile_sem_poison_stack.pop()
        assert popped is self._sem_poison
        self.nc.clear_and_free_semaphores(list(self.sems.allocated().values()))
        self.nc.all_engine_barrier()

    tile_mod.TileContext._drain_and_barrier = _drain_and_barrier


_ws_counter = [0]


def _split_multi_waits(nc, mybir):
    for fn in nc.m.functions:
        for blk in fn.blocks:
            insts = list(blk.instructions)
            out = []
            changed = False
            for inst in insts:
                si = inst.sync_info
                waits = list(si.on_wait) if si and si.on_wait else []
                if len(waits) > 1:
                    for w in waits[:-1]:
                        _ws_counter[0] += 1
                        out.append(mybir.InstNoOp(
                            name=f"I-ws-{_ws_counter[0]}",
                            engine=inst.engine, ins=[], outs=[],
                            sync_info=mybir.SyncInfo(on_wait=[w], on_update=[])))
                    si.on_wait = [waits[-1]]
                    changed = True
                out.append(inst)
            if changed:
                blk.instructions[:] = out


def _build():
    import concourse.bass as bass
    import concourse.tile as tile
    from concourse import mybir

    _patch_tile(tile, bass)

    f32 = mybir.dt.float32
    f32r = mybir.dt.float32r
    bf16 = mybir.dt.bfloat16
    AF = mybir.ActivationFunctionType
    ALU = mybir.AluOpType

    nc = bass.Bass()

    # ---- DRAM I/O ----
    x_d = nc.dram_tensor("x", (C, H, W), f32, kind="ExternalInput")
    wq_d = nc.dram_tensor("wqkv", (C + 1, 3 * C), f32, kind="ExternalInput")  # lhsT+bias
    wp_d = nc.dram_tensor("wproj", (C, C), f32, kind="ExternalInput")         # lhsT
    pb_d = nc.dram_tensor("projb", (C, 1), f32, kind="ExternalInput")
    w1_d = nc.dram_tensor("w1", (C + 1, MLPH), f32, kind="ExternalInput")     # lhsT+bias
    w2_d = nc.dram_tensor("w2", (MLPH, C), f32, kind="ExternalInput")         # lhsT
    b2_d = nc.dram_tensor("b2", (C, 1), f32, kind="ExternalInput")
    repl_d = nc.dram_tensor("repl", (128, 128), f32, kind="ExternalInput")
    ws_d = nc.dram_tensor("wsum3", (C, 3 * C), f32, kind="ExternalInput")
    ones_d = nc.dram_tensor("onesc", (C, 1), f32, kind="ExternalInput")

    y_d = nc.dram_tensor("y", (C, H, W), f32, kind="ExternalOutput")

    with tile.TileContext(nc) as tc:
        # ---------------- persistent pools ----------------
        wpool = tc.alloc_tile_pool(name="weights", bufs=1)
        wq = wpool.tile([C + 1, 3 * C], f32r)
        nc.sync.dma_start(out=wq, in_=wq_d[:, :].bitcast(f32r))
        wp = wpool.tile([C, C], bf16)
        nc.gpsimd.dma_start(out=wp, in_=wp_d[:, :])     # gpsimd dma casts
        pbt = wpool.tile([1, C], f32r)                  # proj bias as rank-1 lhsT
        nc.sync.dma_start(out=pbt, in_=pb_d[:, :].rearrange("a b -> b a").bitcast(f32r))
        w1 = wpool.tile([C + 1, MLPH], f32r)
        nc.sync.dma_start(out=w1, in_=w1_d[:, :].bitcast(f32r))
        w2 = [wpool.tile([128, C], bf16, tag=f"w2{i}", name=f"w2{i}") for i in range(3)]
        for i in range(3):
            nc.gpsimd.dma_start(out=w2[i], in_=w2_d[128 * i:128 * (i + 1), :])
        b2t = wpool.tile([1, C], f32r)                  # fc2 bias as rank-1 lhsT
        nc.sync.dma_start(out=b2t, in_=b2_d[:, :].rearrange("a b -> b a").bitcast(f32r))
        repl = wpool.tile([128, 128], bf16)
        nc.gpsimd.dma_start(out=repl, in_=repl_d[:, :])
        srepl = wpool.tile([128, 128], bf16)            # repl/16: S accumulation
        nc.scalar.mul(out=srepl, in_=repl, mul=1.0 / HD)
        onescol = wpool.tile([C, 1], f32r)              # stats lhsT [96,1]
        nc.sync.dma_start(out=onescol, in_=ones_d[:, :].bitcast(f32r))
        onescol_b = wpool.tile([C, 1], bf16)            # bf16 variant (bf16 rhs)
        nc.vector.memset(onescol_b, 1.0)
        wsum3 = wpool.tile([C, 3 * C], f32r)            # qkv col-sums at {0,32,64}
        nc.sync.dma_start(out=wsum3, in_=ws_d[:, :].bitcast(f32r))
        ones1x = wpool.tile([1, C], f32r)               # rank-1 lhsT [1,96]
        nc.sync.dma_start(out=ones1x, in_=ones_d[:, :].rearrange("a b -> b a").bitcast(f32r))
        onesrow = wpool.tile([1, CH], f32r)             # static ones row (f32r)
        ones3x = wpool.tile([C, C], f32r)               # ones rows at {0,32,64}
        epst = wpool.tile([128, 1], f32)
        nc.vector.memset(epst, EPS)
        # LN-applied activation tiles with a persistent ones bias row
        xns = [wpool.tile([C + 1, CH], f32r, tag=f"xn{i}", name=f"xn{i}")
               for i in range(2)]
        # f32 scratch (init-only) to produce properly-rounded f32r constants
        initp = tc.alloc_tile_pool(name="initp", bufs=1)
        onesrow_f = initp.tile([1, CH], f32)
        nc.vector.memset(onesrow_f, 1.0)
        nc.vector.tensor_copy(out=onesrow, in_=onesrow_f)
        ones3f = initp.tile([C, C], f32)
        nc.vector.memset(ones3f, 0.0)
        for j in range(3):
            nc.vector.memset(ones3f[32 * j:32 * j + 1, :], 1.0)
        nc.vector.tensor_copy(out=ones3x, in_=ones3f)
        for i in range(2):
            nc.vector.tensor_copy(out=xns[i][C:C + 1, :], in_=onesrow_f)
        initp.release()
        # LN stats rows: rs/nb per chunk on partitions [32, 512]
        # (LN2 pass reuses the same tiles after LN1's readers are done)
        rs1 = wpool.tile([NCHUNK, CH], f32r)
        nb1 = wpool.tile([NCHUNK, CH], f32r)
        rs2, nb2 = rs1, nb1
        # channel-sum / channel-sumsq strips (chunk on partitions)
        ss1 = wpool.tile([NCHUNK, CH], f32)
        sq1 = wpool.tile([NCHUNK, CH], f32)
        ss2, sq2 = ss1, sq1

        # big persistent activation tensors
        apool = tc.alloc_tile_pool(name="acts", bufs=1)
        Qd = [apool.tile([128, BH, W], bf16, tag=f"qd{d}", name=f"qd{d}") for d in range(3)]
        Kp = [apool.tile([128, PADR, PADC], bf16, tag=f"kp{d}", name=f"kp{d}") for d in range(3)]
        Vp = [apool.tile([128, PADR, PADC], bf16, tag=f"vp{d}", name=f"vp{d}") for d in range(3)]


        # ============ shared stats math ============
        # strips ss/sq: [32, CH] SBUF, chunk on partitions.  Runs on row
        # slices [c0:c0+n] right after each flush so downstream chunks can
        # start without waiting for the whole stats pass.  mu overwrites ss,
        # var/sd overwrite sq in place; vtmp holds -mu^2.
        vtmp = wpool.tile([NCHUNK, CH], f32)

        def stats_math_early(ss, sq, rs, nb):
            sl = slice(0, 8)
            mu = vtmp[sl, :]
            nc.scalar.mul(out=mu, in_=ss[sl, :], mul=1.0 / C)
            t = rs[sl, :].bitcast(f32)
            nc.scalar.mul(out=t, in_=sq[sl, :], mul=1.0 / C)
            nc.vector.scalar_tensor_tensor(out=nb[sl, :].bitcast(f32), in0=mu,
                                           scalar=-1.0, in1=mu,
                                           op0=ALU.mult, op1=ALU.mult)
            nc.vector.tensor_tensor(out=t, in0=t, in1=nb[sl, :].bitcast(f32),
                                    op=ALU.add)
            nc.scalar.activation(out=t, in_=t, func=AF.Sqrt,
                                 bias=epst[0:8, 0:1], scale=1.0)
            with nc.allow_low_precision(reason="f32r-typed LN stats rows"):
                nc.vector.reciprocal(out=rs[sl, :], in_=t)
                nc.vector.scalar_tensor_tensor(out=nb[sl, :], in0=mu,
                                               scalar=-1.0,
                                               in1=rs[sl, :].bitcast(f32),
                                               op0=ALU.mult, op1=ALU.mult)

        def stats_math(ss, sq, rs, nb, c0, n):
            sl = slice(c0, c0 + n)
            nc.scalar.mul(out=ss[sl, :], in_=ss[sl, :], mul=1.0 / C)
            nc.scalar.mul(out=sq[sl, :], in_=sq[sl, :], mul=1.0 / C)
            nc.vector.scalar_tensor_tensor(out=vtmp[sl, :], in0=ss[sl, :],
                                           scalar=-1.0, in1=ss[sl, :],
                                           op0=ALU.mult, op1=ALU.mult)
            nc.vector.tensor_tensor(out=sq[sl, :], in0=sq[sl, :],
                                    in1=vtmp[sl, :], op=ALU.add)
            nc.scalar.activation(out=sq[sl, :], in_=sq[sl, :], func=AF.Sqrt,
                                 bias=epst[c0:c0 + n, 0:1], scale=1.0)
            with nc.allow_low_precision(reason="f32r-typed LN stats rows"):
                nc.vector.reciprocal(out=rs[sl, :], in_=sq[sl, :])
                nc.vector.scalar_tensor_tensor(out=nb[sl, :], in0=ss[sl, :],
                                               scalar=-1.0,
                                               in1=rs[sl, :].bitcast(f32),
                                               op0=ALU.mult, op1=ALU.mult)

        # strip helper: drain a [1, n*CH] psum strip (partition 0) to a
        # 1-partition SBUF stage, then one DMA reshapes it into rows
        # [c0:c0+n] of the compact [32, CH] stats tiles.
        def strip_flush(pool, ps_s, ps_q, ss, sq, c0, n, qeng=None):
            stg_s = pool.tile([1, n * CH], f32, tag="stg_s")
            nc.scalar.copy(stg_s, ps_s[:, 0:n * CH])
            stg_q = pool.tile([1, n * CH], f32, tag="stg_q")
            if qeng is None:
                nc.scalar.copy(stg_q, ps_q[:, 0:n * CH])
            else:
                qeng.tensor_copy(out=stg_q, in_=ps_q[:, 0:n * CH])
            nc.sync.dma_start(out=ss[c0:c0 + n, :], in_=stg_s)
            nc.sync.dma_start(out=sq[c0:c0 + n, :], in_=stg_q)

        # ============ PH1: LN1 stats sweep ============
        with tc.tile_pool(name="ph1", bufs=3) as pool, \
             tc.tile_pool(name="ph1st", bufs=2) as sgpool, \
             tc.tile_pool(name="ph1ps", bufs=2, space="PSUM") as stps:
            ps_s = ps_q = None
            for g in range(NCHUNK // 4):
                xt4 = pool.tile([C, 4, CH], f32r, tag="xt")
                nc.sync.dma_start(out=xt4,
                                  in_=x_d[:, 16 * g:16 * g + 16, :].bitcast(f32r))
                for i in range(4):
                    c = 4 * g + i
                    if i % 2 == 0:
                        ps_s = stps.tile([1, 2 * CH], f32, tag="ps_s")
                        ps_q = stps.tile([1, 2 * CH], f32, tag="ps_q")
                    h = CH * (i % 2)
                    xt = xt4[:, i, :]
                    nc.tensor.matmul(ps_s[:, h:h + CH], lhsT=onescol,
                                     rhs=xt, start=True, stop=True)
                    xf = xt.bitcast(f32)
                    xsq = pool.tile([C, CH], f32r, tag="xsq")
                    nc.vector.tensor_tensor(out=xsq, in0=xf, in1=xf, op=ALU.mult)
                    nc.tensor.matmul(ps_q[:, h:h + CH], lhsT=onescol,
                                     rhs=xsq, start=True, stop=True)
                    if i % 2 == 1:
                        strip_flush(sgpool, ps_s, ps_q, ss1, sq1, c - 1, 2,
                                    qeng=nc.vector if g % 2 == 0 else None)
                    if i == 3:
                        if g == 7:
                            stats_math(ss1, sq1, rs1, nb1, 0, NCHUNK)

        # zero only the pad strips (interior fully overwritten by repack)
        for d in range(3):
            for t in (Kp[d], Vp[d]):
                nc.gpsimd.memset(t[:, 0:3, :], 0.0)
                nc.gpsimd.memset(t[:, 35:38, :], 0.0)
                nc.gpsimd.memset(t[:, 3:35, 0:3], 0.0)
                nc.gpsimd.memset(t[:, 3:35, 3 + W:PADC], 0.0)

        stgpool = tc.alloc_tile_pool(name="stg", bufs=1)
        stg_k = stgpool.tile([C, N], bf16)
        stg_v = stgpool.tile([C, N], bf16)


        # ============ PH2: LN1 apply + qkv + stage/scatter ============
        def band_rows(b):
            lo = max(0, BH * b - 3)
            hi = min(H, BH * b + BH + 3)
            return lo, hi

        # stage rs/nb rows at partitions {0,32,64} so rank-1 matmuls can
        # read them (PE base-partition rule); one strided DMA per 3 chunks
        def stage_stats(sgp, rs, nb, c0):
            n = min(3, NCHUNK - c0)
            srs = sgp.tile([C, CH], f32r, tag="srs")
            snb = sgp.tile([C, CH], f32r, tag="snb")
            dst_rs = srs.rearrange("(a b) f -> a b f", a=3)[0:n, 0:1, :]
            dst_nb = snb.rearrange("(a b) f -> a b f", a=3)[0:n, 0:1, :]
            nc.sync.dma_start(out=dst_rs, in_=rs[c0:c0 + n, :])
            nc.sync.dma_start(out=dst_nb, in_=nb[c0:c0 + n, :])
            return srs, snb

        with tc.tile_pool(name="ph2", bufs=3) as pool, \
             tc.tile_pool(name="ph2t", bufs=2) as tpool, \
             tc.tile_pool(name="ph2sg", bufs=2) as sgp, \
             tc.tile_pool(name="ph2ps", bufs=2, space="PSUM") as psum, \
             tc.tile_pool(name="ph2ps2", bufs=2, space="PSUM") as psum2:
            srs = snb = None
            for c in range(NCHUNK):
                g, i = c // 4, c % 4
                if i == 0:
                    xt4 = pool.tile([C, 4, CH], f32, tag="xt2")
                    nc.sync.dma_start(out=xt4, in_=x_d[:, 16 * g:16 * g + 16, :])
                if c % 3 == 0:
                    srs, snb = stage_stats(sgp, rs1, nb1, c)
                j = c % 3
                xt = xt4[:, i, :]
                pa = psum2.tile([C, CH], f32, tag="pa")
                nc.tensor.matmul(pa, lhsT=ones3x[32 * j:32 * j + 1, :],
                                 rhs=srs[32 * j:32 * j + 1, :],
                                 start=True, stop=True)
                xn = xns[c % 2]
                nc.vector.tensor_tensor(out=xn[0:C, :], in0=xt, in1=pa,
                                        op=ALU.mult)

                pq = psum.tile([C, CH], f32, tag="pq")
                pk = psum.tile([C, CH], f32, tag="pk")
                pv = psum.tile([C, CH], f32, tag="pv")
                snbj = snb[32 * j:32 * j + 1, :]
                for t, pt in enumerate((pq, pk, pv)):
                    nc.tensor.matmul(pt, lhsT=wsum3[32 * j:32 * j + 1,
                                                    C * t:C * (t + 1)],
                                     rhs=snbj, start=True, stop=False)
                    nc.tensor.matmul(pt, lhsT=wq[:, C * t:C * (t + 1)], rhs=xn,
                                     start=False, stop=True)

                # K/V -> contiguous staging (single 96-partition copies);
                # gpsimd cannot touch PSUM, so drains go DVE/Act only
                nc.scalar.copy(stg_k[:, CH * c:CH * (c + 1)], pk)
                if c % 2 == 0:
                    nc.vector.tensor_copy(out=stg_v[:, CH * c:CH * (c + 1)], in_=pv)
                else:
                    nc.scalar.copy(stg_v[:, CH * c:CH * (c + 1)], pv)
                # Q -> band-packed planes directly
                b = c // 8
                r_off = 4 * c - BH * b
                for d in range(3):
                    src = pq[32 * d:32 * d + 32, :].rearrange("p (r w) -> p r w", r=4)
                    dst = Qd[d][32 * b:32 * b + 32, r_off:r_off + 4, :]
                    if d == 0 or (d == 2 and c % 2 == 1):
                        nc.vector.tensor_copy(out=dst, in_=src)
                    else:
                        nc.scalar.copy(dst, src)

                # band-packed K/V repack via SBUF->SBUF DMA on idle rings
                if c in (8, 16, 24, 31):
                    b_ = (c - 1) // 8
                    lo, hi = band_rows(b_)
                    nr = hi - lo
                    r0 = lo - (BH * b_ - 3)
                    for d in range(3):
                        for stg, dstp in ((stg_k, Kp[d]), (stg_v, Vp[d])):
                            nc.sync.dma_start(
                                out=dstp[32 * b_:32 * b_ + 32, r0:r0 + nr, 3:3 + W],
                                in_=stg[32 * d:32 * d + 32, W * lo:W * hi]
                                    .rearrange("p (r w) -> p r w", r=nr))

        stgpool.release()

        # attention output (channel-major), in space freed by the staging
        ofpool = tc.alloc_tile_pool(name="ofp", bufs=1)
        ofull = ofpool.tile([C, N], bf16)

        # ============ PH3: attention per dilation ============
        # Processed in half-planes (16 band-rows each) so the softmax
        # denominator S accumulates across taps in PSUM via PE matmuls with
        # srepl (= repl/16), freeing DVE of the S adds entirely.
        HF = BH * W // 2                                # 2048 tokens per half
        with tc.tile_pool(name="ph3", bufs=4) as pool, \
             tc.tile_pool(name="ph3f", bufs=4) as fpool, \
             tc.tile_pool(name="ph3acc", bufs=3) as acc, \
             tc.tile_pool(name="ph3ps", bufs=2, space="PSUM") as psum, \
             tc.tile_pool(name="ph3sps", bufs=1, space="PSUM") as spsum:
            for di, dil in enumerate(DILS):
                for hh in range(2):
                    S_ps = spsum.tile([128, HF], f32, tag="Sps")
                    Oab = [acc.tile([128, HF], bf16, tag=f"O{p}",
                                    name=f"O{p}_{di}_{hh}") for p in range(2)]
                    rcp = acc.tile([128, HF], bf16, tag="rcp",
                                   name=f"rcp_{di}_{hh}")
                    qv = Qd[di][:, 16 * hh:16 * hh + 16, :]
                    for ti, (dr, dc) in enumerate([(i - 1, j - 1)
                                                   for i in range(3) for j in range(3)]):
                        r0 = 3 + dr * dil + 16 * hh
                        kwin = Kp[di][:, r0:r0 + 16, 3 + dc * dil:3 + dc * dil + W]
                        vwin = Vp[di][:, r0:r0 + 16, 3 + dc * dil:3 + dc * dil + W]
                        P = fpool.tile([128, HF], bf16, tag="P")
                        nc.vector.tensor_tensor(
                            out=P.rearrange("p (r w) -> p r w", r=16),
                            in0=qv, in1=kwin, op=ALU.mult)
                        # logits -> exp (overwrites P) -> S accumulation
                        for q in range(2):
                            pl = psum.tile([128, 1024], f32, tag="pl")
                            for j in range(2):
                                nc.tensor.matmul(
                                    pl[:, 512 * j:512 * (j + 1)], lhsT=repl,
                                    rhs=P[:, 1024 * q + 512 * j:
                                          1024 * q + 512 * (j + 1)],
                                    start=True, stop=True)
                            nc.scalar.activation(
                                out=P[:, 1024 * q:1024 * (q + 1)], in_=pl,
                                func=AF.Exp)
                            for j in range(2):
                                nc.tensor.matmul(
                                    S_ps[:, 1024 * q + 512 * j:
                                         1024 * q + 512 * (j + 1)],
                                    lhsT=srepl,
                                    rhs=P[:, 1024 * q + 512 * j:
                                          1024 * q + 512 * (j + 1)],
                                    start=(ti == 0), stop=(ti == 8))
                        ev = P.rearrange("p (r w) -> p r w", r=16)
                        Pv = Oab[ti] if ti < 2 else pool.tile([128, HF], bf16,
                                                              tag="Pv")
                        nc.vector.tensor_tensor(
                            out=Pv.rearrange("p (r w) -> p r w", r=16),
                            in0=ev, in1=vwin, op=ALU.mult)
                        if ti >= 2:
                            eng = nc.gpsimd if ti in (3, 4, 7) else nc.vector
                            eng.tensor_tensor(out=Oab[ti % 2], in0=Oab[ti % 2],
                                              in1=Pv, op=ALU.add)
                    with nc.allow_low_precision(reason="softmax recip bf16"):
                        nc.vector.reciprocal(out=rcp, in_=S_ps)
                    nc.vector.tensor_tensor(out=Oab[0], in0=Oab[0], in1=Oab[1],
                                            op=ALU.add)
                    nc.vector.tensor_tensor(out=Oab[0], in0=Oab[0], in1=rcp,
                                            op=ALU.mult)
                    for b in range(NB):
                        nc.sync.dma_start(
                            out=ofull[32 * di:32 * di + 32,
                                      4096 * b + 2048 * hh:
                                      4096 * b + 2048 * hh + 2048],
                            in_=Oab[0][32 * b:32 * b + 32, :])

        # ============ PH4: proj + residual + LN2 stats ============
        r1pool = tc.alloc_tile_pool(name="r1p", bufs=1)
        r1 = r1pool.tile([C, N], bf16)
        with tc.tile_pool(name="ph4", bufs=2) as pool, \
             tc.tile_pool(name="ph4sg", bufs=1) as sgpool, \
             tc.tile_pool(name="ph4st", bufs=1, space="PSUM") as stps, \
             tc.tile_pool(name="ph4ps", bufs=4, space="PSUM") as psum:
            ps_s = ps_q = None
            for c in range(NCHUNK):
                g, i = c // 4, c % 4
                if i == 0:
                    xt4 = pool.tile([C, 4, CH], f32, tag="xt4")
                    nc.sync.dma_start(out=xt4, in_=x_d[:, 16 * g:16 * g + 16, :])
                if c % 2 == 0:
                    ps_s = stps.tile([1, 2 * CH], f32, tag="ps_s4")
                    ps_q = stps.tile([1, 2 * CH], f32, tag="ps_q4")
                h4 = CH * (c % 2)
                pp = psum.tile([C, CH], f32, tag="pp")
                nc.tensor.matmul(pp, lhsT=pbt, rhs=onesrow, start=True, stop=False)
                nc.tensor.matmul(pp, lhsT=wp, rhs=ofull[:, CH * c:CH * (c + 1)],
                                 start=False, stop=True)
                rsl = r1[:, CH * c:CH * (c + 1)]
                nc.vector.tensor_tensor(out=rsl, in0=xt4[:, i, :],
                                        in1=pp, op=ALU.add)
                nc.tensor.matmul(ps_s[:, h4:h4 + CH], lhsT=onescol_b,
                                 rhs=rsl, start=True, stop=True)
                xsq = pool.tile([C, CH], f32r, tag="xsq5")
                nc.vector.tensor_tensor(out=xsq, in0=rsl, in1=rsl, op=ALU.mult)
                nc.tensor.matmul(ps_q[:, h4:h4 + CH], lhsT=onescol,
                                 rhs=xsq, start=True, stop=True)
                if c % 2 == 1:
                    strip_flush(sgpool, ps_s, ps_q, ss2, sq2, c - 1, 2)
                    if c == 31:
                        stats_math(ss2, sq2, rs2, nb2, 0, NCHUNK)

        # ============ PH5: MLP + residual ============
        with tc.tile_pool(name="ph5b", bufs=3) as pool, \
             tc.tile_pool(name="ph5h", bufs=2) as hpool, \
             tc.tile_pool(name="ph5y", bufs=2) as ypool, \
             tc.tile_pool(name="ph5sg", bufs=2) as sgp, \
             tc.tile_pool(name="ph5ps", bufs=3, space="PSUM") as psum, \
             tc.tile_pool(name="ph5ps2", bufs=1, space="PSUM") as psum2:
            srs = snb = None
            for c in range(NCHUNK):
                g, i = c // 4, c % 4
                rsl = r1[:, CH * c:CH * (c + 1)]
                if i == 0:
                    yout4 = ypool.tile([C, 4, CH], f32, tag="yout4")
                if c % 3 == 0:
                    srs, snb = stage_stats(sgp, rs2, nb2, c)
                j = c % 3
                pa = psum2.tile([C, CH], f32, tag="pa5")
                nc.tensor.matmul(pa, lhsT=ones3x[32 * j:32 * j + 1, :],
                                 rhs=srs[32 * j:32 * j + 1, :],
                                 start=True, stop=True)
                pb2 = psum2.tile([C, CH], f32, tag="pb5")
                nc.tensor.matmul(pb2, lhsT=ones3x[32 * j:32 * j + 1, :],
                                 rhs=snb[32 * j:32 * j + 1, :],
                                 start=True, stop=True)
                t1 = pool.tile([C, CH], f32, tag="t15")
                nc.vector.tensor_tensor(out=t1, in0=rsl, in1=pa, op=ALU.mult)
                xn = xns[c % 2]
                nc.vector.tensor_tensor(out=xn[0:C, :], in0=t1,
                                        in1=pb2, op=ALU.add)

                h1 = hpool.tile([128, 3, CH], bf16, tag="h1")
                for j in range(3):
                    pf = psum.tile([128, CH], f32, tag="pf")
                    nc.tensor.matmul(pf, lhsT=w1[:, 128 * j:128 * (j + 1)], rhs=xn,
                                     start=True, stop=True)
                    nc.scalar.activation(out=h1[:, j, :], in_=pf, func=AF.Gelu)
                pm = psum.tile([C, CH], f32, tag="pm")
                nc.tensor.matmul(pm, lhsT=b2t, rhs=onesrow, start=True, stop=False)
                for j in range(3):
                    nc.tensor.matmul(pm, lhsT=w2[j],
                                     rhs=h1[:, j, :], start=False, stop=(j == 2))
                nc.vector.tensor_tensor(out=yout4[:, i, :], in0=rsl,
                                        in1=pm, op=ALU.add)
                if i == 3:
                    nc.sync.dma_start(out=y_d[:, 16 * g:16 * g + 16, :], in_=yout4)

        r1pool.release()
        ofpool.release()
        apool.release()
        wpool.release()

    _split_multi_waits(nc, mybir)
    return nc


def _prep_weights(inputs):
    """Host-side weight preparation (fold LN affine, scale, bias rows)."""
    qkv_w = np.asarray(inputs['qkv_w'], np.float32)       # (288, 96)
    proj_w = np.asarray(inputs['proj_w'], np.float32)     # (96, 96)
    proj_b = np.asarray(inputs['proj_b'], np.float32)
    ln1_w = np.asarray(inputs['ln1_w'], np.float32)
    ln1_b = np.asarray(inputs['ln1_b'], np.float32)
    ln2_w = np.asarray(inputs['ln2_w'], np.float32)
    ln2_b = np.asarray(inputs['ln2_b'], np.float32)
    fc1_w = np.asarray(inputs['fc1_w'], np.float32)       # (384, 96)
    fc1_b = np.asarray(inputs['fc1_b'], np.float32)
    fc2_w = np.asarray(inputs['fc2_w'], np.float32)       # (96, 384)
    fc2_b = np.asarray(inputs['fc2_b'], np.float32)

    wq = qkv_w * ln1_w[None, :]                            # (288, 96)
    c0 = qkv_w @ ln1_b                                     # (288,)
    wq[0:C] *= SCALE                                       # scale q rows
    c0[0:C] *= SCALE
    wqb = np.concatenate([wq.T, c0[None, :]], axis=0)      # (97, 288)

    w1 = fc1_w * ln2_w[None, :]
    c1 = fc1_w @ ln2_b + fc1_b
    w1b = np.concatenate([w1.T, c1[None, :]], axis=0)      # (97, 384)

    repl = np.zeros((128, 128), np.float32)
    for b in range(NB):
        for ch in range(GD):
            h0 = (ch // HD) * HD
            repl[32 * b + h0:32 * b + h0 + HD, 32 * b + ch] = 1.0

    wsum3 = np.zeros((C, 3 * C), np.float32)
    for j in range(3):
        wsum3[32 * j, :] = wq.T[0:C, :].sum(axis=0)        # per-column sums

    return {
        'wsum3': wsum3,
        'wqkv': np.ascontiguousarray(wqb),                 # (97, 288) lhsT
        'wproj': np.ascontiguousarray(proj_w.T),           # (96, 96) lhsT
        'projb': proj_b.reshape(-1, 1).astype(np.float32),
        'w1': np.ascontiguousarray(w1b),                   # (97, 384) lhsT
        'w2': np.ascontiguousarray(fc2_w.T),               # (384, 96) lhsT
        'b2': fc2_b.reshape(-1, 1).astype(np.float32),
        'repl': repl,
        'onesc': np.ones((C, 1), np.float32),
    }


def kernel(**inputs):
    from concourse.bass_utils import run_bass_kernel_spmd

    if 'nc' not in _cache:
        t0 = time.time()
        _cache['nc'] = _build()
        print(f"[kernel] built bass module in {time.time() - t0:.1f}s",
              file=sys.stderr)

    nc = _cache['nc']
    wmap = _prep_weights(inputs)
    x = np.asarray(inputs['x'], np.float32)                # (8, 96, 128, 128)

    in_maps = []
    for b in range(B):
        m = {'x': np.ascontiguousarray(x[b])}
        m.update(wmap)
        in_maps.append(m)

    res = run_bass_kernel_spmd(nc, in_maps, core_ids=list(range(B)))
    _cache['last_exec_ns'] = res.exec_time_ns
    out = np.stack([res.results[b]['y'] for b in range(B)], axis=0)
    return out.astype(np.float32)


# revision 96
# speedup vs baseline: 1.0704x; 1.0045x over previous
"""DilateBlock kernel for 8x Trainium2 NeuronCores (Bass/Tile).

Data-parallel over batch B=8 (one image per core). Per core, the whole block
(LN1 -> qkv -> 3-dilation 3x3 neighborhood attention -> proj -> residual ->
LN2 -> MLP -> residual) runs in channels-on-partitions layout; spatial shifts
for the attention unfold live on the free dimension of zero-padded (h, w)
planes, packed 4-hbands x 32-channels across partitions.

Key tricks:
  - LayerNorm stats via ones-matmul on PE into a [32, 512] PSUM strip tile
    (chunk index on partitions), stats math runs wide on 32 partitions, and
    the per-token scale/shift rows feed rank-1 PSUM matmuls directly.
  - qkv/fc1 biases folded into the matmuls via a 97th ones-row of the
    LN-applied activations (contract-97 lhsT with a bias row).
  - K/V drained to contiguous staging as single 96-partition copies, then
    band-packed into padded planes by SBUF->SBUF DMAs on the idle DMA rings.
  - QK tap logits reduced over head_dim AND replicated back to all 16
    channel rows in one PE matmul with a static block-ones matrix; exp runs
    full width on Act; softmax denominator and output accumulate across taps
    via gpsimd DMA-accumulate (even/odd partial tiles, merged on DVE).
  - Softmax normalization applied to the attention OUTPUT.
  - Attention output repacked in SBUF (no DRAM roundtrip) for the proj.
"""
import sys
import time

sys.path.insert(0, '/opt/trn_rl_repo')

import numpy as np

# ---- problem constants (hardcoded per contract) ----
B, C, H, W = 8, 96, 128, 128
DILS = (1, 2, 3)
GD = 32                 # channels per dilation branch
HD = 16                 # head dim
NB = 4                  # h-bands packed on partitions
BH = H // NB            # rows per band = 32
N = H * W               # tokens per image
NCHUNK = 32             # token chunks of 512 (4 image rows each)
CH = N // NCHUNK        # 512
PADR = 38               # BH + 6 halo rows
PADC = 136              # W + 8 halo cols (EVEN pitch: enables DVE 2x mode)
EPS = 1e-5
SCALE = HD ** -0.5
MLPH = 384

_cache = {}
import os
_USE_DMA_ACCUM = os.environ.get('KDMA', '0') == '1'


def _patch_tile(tile_mod, bass_mod):
    """Work around this walrus build's 1-sem-wait-per-instruction limit and
    the multi-wait tail drain."""
    from concourse.vector_clock import ScopedClock, VectorClock

    def _drain_and_barrier(self, tick_clock, wait_clock):
        vclock = tick_clock.global_clock
        n = len(vclock)
        idxs = [i for i in range(n) if vclock[i] > 0]
        for i in idxs:
            vec = [0] * n
            vec[i] = vclock[i]
            nop_inst = self.nc.sync.nop(nofuse=True)
            wait_clock.add_sem_waits(nop_inst.ins,
                                     ScopedClock({None: VectorClock(vec)}))
        self.nc.sync.drain()
        self.nc.all_engine_barrier()
        popped = self.nc._tile_sem_poison_stack.pop()
        assert popped is self._sem_poison
        self.nc.clear_and_free_semaphores(list(self.sems.allocated().values()))
        self.nc.all_engine_barrier()

    tile_mod.TileContext._drain_and_barrier = _drain_and_barrier


_ws_counter = [0]


def _split_multi_waits(nc, mybir):
    for fn in nc.m.functions:
        for blk in fn.blocks:
            insts = list(blk.instructions)
            out = []
            changed = False
            for inst in insts:
                si = inst.sync_info
                waits = list(si.on_wait) if si and si.on_wait else []
                if len(waits) > 1:
                    for w in waits[:-1]:
                        _ws_counter[0] += 1
                        out.append(mybir.InstNoOp(
                            name=f"I-ws-{_ws_counter[0]}",
                            engine=inst.engine, ins=[], outs=[],
                            sync_info=mybir.SyncInfo(on_wait=[w], on_update=[])))
                    si.on_wait = [waits[-1]]
                    changed = True
                out.append(inst)
            if changed:
                blk.instructions[:] = out


def _build():
    import concourse.bass as bass
    import concourse.tile as tile
    from concourse import mybir

    _patch_tile(tile, bass)

    f32 = mybir.dt.float32
    f32r = mybir.dt.float32r
    bf16 = mybir.dt.bfloat16
    AF = mybir.ActivationFunctionType
    ALU = mybir.AluOpType

    nc = bass.Bass()

    # ---- DRAM I/O ----
    x_d = nc.dram_tensor("x", (C, H, W), f32, kind="ExternalInput")
    wq_d = nc.dram_tensor("wqkv", (C + 1, 3 * C), f32, kind="ExternalInput")  # lhsT+bias
    wp_d = nc.dram_tensor("wproj", (C, C), f32, kind="ExternalInput")         # lhsT
    pb_d = nc.dram_tensor("projb", (C, 1), f32, kind="ExternalInput")
    w1_d = nc.dram_tensor("w1", (C + 1, MLPH), f32, kind="ExternalInput")     # lhsT+bias
    w2_d = nc.dram_tensor("w2", (MLPH, C), f32, kind="ExternalInput")         # lhsT
    b2_d = nc.dram_tensor("b2", (C, 1), f32, kind="ExternalInput")
    repl_d = nc.dram_tensor("repl", (128, 128), f32, kind="ExternalInput")
    ws_d = nc.dram_tensor("wsum3", (C, 3 * C), f32, kind="ExternalInput")
    ones_d = nc.dram_tensor("onesc", (C, 1), f32, kind="ExternalInput")

    y_d = nc.dram_tensor("y", (C, H, W), f32, kind="ExternalOutput")

    with tile.TileContext(nc) as tc:
        # ---------------- persistent pools ----------------
        wpool = tc.alloc_tile_pool(name="weights", bufs=1)
        wq = wpool.tile([C + 1, 3 * C], f32r)
        nc.sync.dma_start(out=wq, in_=wq_d[:, :].bitcast(f32r))
        wp = wpool.tile([C, C], bf16)
        nc.gpsimd.dma_start(out=wp, in_=wp_d[:, :])     # gpsimd dma casts
        pbt = wpool.tile([1, C], f32r)                  # proj bias as rank-1 lhsT
        nc.sync.dma_start(out=pbt, in_=pb_d[:, :].rearrange("a b -> b a").bitcast(f32r))
        w1 = wpool.tile([C + 1, MLPH], f32r)
        nc.sync.dma_start(out=w1, in_=w1_d[:, :].bitcast(f32r))
        w2 = [wpool.tile([128, C], bf16, tag=f"w2{i}", name=f"w2{i}") for i in range(3)]
        for i in range(3):
            nc.gpsimd.dma_start(out=w2[i], in_=w2_d[128 * i:128 * (i + 1), :])
        b2t = wpool.tile([1, C], f32r)                  # fc2 bias as rank-1 lhsT
        nc.sync.dma_start(out=b2t, in_=b2_d[:, :].rearrange("a b -> b a").bitcast(f32r))
        repl = wpool.tile([128, 128], bf16)
        nc.gpsimd.dma_start(out=repl, in_=repl_d[:, :])
        srepl = wpool.tile([128, 128], bf16)            # repl/16: S accumulation
        nc.scalar.mul(out=srepl, in_=repl, mul=1.0 / HD)
        onescol = wpool.tile([C, 1], f32r)              # stats lhsT [96,1]
        nc.sync.dma_start(out=onescol, in_=ones_d[:, :].bitcast(f32r))
        onescol_b = wpool.tile([C, 1], bf16)            # bf16 variant (bf16 rhs)
        nc.vector.memset(onescol_b, 1.0)
        wsum3 = wpool.tile([C, 3 * C], f32r)            # qkv col-sums at {0,32,64}
        nc.sync.dma_start(out=wsum3, in_=ws_d[:, :].bitcast(f32r))
        ones1x = wpool.tile([1, C], f32r)               # rank-1 lhsT [1,96]
        nc.sync.dma_start(out=ones1x, in_=ones_d[:, :].rearrange("a b -> b a").bitcast(f32r))
        onesrow = wpool.tile([1, CH], f32r)             # static ones row (f32r)
        ones3x = wpool.tile([C, C], f32r)               # ones rows at {0,32,64}
        epst = wpool.tile([128, 1], f32)
        nc.vector.memset(epst, EPS)
        # LN-applied activation tiles with a persistent ones bias row
        xns = [wpool.tile([C + 1, CH], f32r, tag=f"xn{i}", name=f"xn{i}")
               for i in range(2)]
        # f32 scratch (init-only) to produce properly-rounded f32r constants
        initp = tc.alloc_tile_pool(name="initp", bufs=1)
        onesrow_f = initp.tile([1, CH], f32)
        nc.vector.memset(onesrow_f, 1.0)
        nc.vector.tensor_copy(out=onesrow, in_=onesrow_f)
        ones3f = initp.tile([C, C], f32)
        nc.vector.memset(ones3f, 0.0)
        for j in range(3):
            nc.vector.memset(ones3f[32 * j:32 * j + 1, :], 1.0)
        nc.vector.tensor_copy(out=ones3x, in_=ones3f)
        for i in range(2):
            nc.vector.tensor_copy(out=xns[i][C:C + 1, :], in_=onesrow_f)
        initp.release()
        # LN stats rows: rs/nb per chunk on partitions [32, 512]
        # (LN2 pass reuses the same tiles after LN1's readers are done)
        rs1 = wpool.tile([NCHUNK, CH], f32r)
        nb1 = wpool.tile([NCHUNK, CH], f32r)
        rs2, nb2 = rs1, nb1
        # channel-sum / channel-sumsq strips (chunk on partitions)
        ss1 = wpool.tile([NCHUNK, CH], f32)
        sq1 = wpool.tile([NCHUNK, CH], f32)
        ss2, sq2 = ss1, sq1

        # big persistent activation tensors
        apool = tc.alloc_tile_pool(name="acts", bufs=1)
        Qd = [apool.tile([128, BH, W], bf16, tag=f"qd{d}", name=f"qd{d}") for d in range(3)]
        Kp = [apool.tile([128, PADR, PADC], bf16, tag=f"kp{d}", name=f"kp{d}") for d in range(3)]
        Vp = [apool.tile([128, PADR, PADC], bf16, tag=f"vp{d}", name=f"vp{d}") for d in range(3)]


        # ============ shared stats math ============
        # strips ss/sq: [32, CH] SBUF, chunk on partitions.  Runs on row
        # slices [c0:c0+n] right after each flush so downstream chunks can
        # start without waiting for the whole stats pass.  mu overwrites ss,
        # var/sd overwrite sq in place; vtmp holds -mu^2.
        vtmp = wpool.tile([NCHUNK, CH], f32)

        def stats_math_early(ss, sq, rs, nb):
            sl = slice(0, 8)
            mu = vtmp[sl, :]
            nc.scalar.mul(out=mu, in_=ss[sl, :], mul=1.0 / C)
            t = rs[sl, :].bitcast(f32)
            nc.scalar.mul(out=t, in_=sq[sl, :], mul=1.0 / C)
            nc.vector.scalar_tensor_tensor(out=nb[sl, :].bitcast(f32), in0=mu,
                                           scalar=-1.0, in1=mu,
                                           op0=ALU.mult, op1=ALU.mult)
            nc.vector.tensor_tensor(out=t, in0=t, in1=nb[sl, :].bitcast(f32),
                                    op=ALU.add)
            nc.scalar.activation(out=t, in_=t, func=AF.Sqrt,
                                 bias=epst[0:8, 0:1], scale=1.0)
            with nc.allow_low_precision(reason="f32r-typed LN stats rows"):
                nc.vector.reciprocal(out=rs[sl, :], in_=t)
                nc.vector.scalar_tensor_tensor(out=nb[sl, :], in0=mu,
                                               scalar=-1.0,
                                               in1=rs[sl, :].bitcast(f32),
                                               op0=ALU.mult, op1=ALU.mult)

        def stats_math(ss, sq, rs, nb, c0, n):
            sl = slice(c0, c0 + n)
            nc.scalar.mul(out=ss[sl, :], in_=ss[sl, :], mul=1.0 / C)
            nc.scalar.mul(out=sq[sl, :], in_=sq[sl, :], mul=1.0 / C)
            nc.vector.scalar_tensor_tensor(out=vtmp[sl, :], in0=ss[sl, :],
                                           scalar=-1.0, in1=ss[sl, :],
                                           op0=ALU.mult, op1=ALU.mult)
            nc.vector.tensor_tensor(out=sq[sl, :], in0=sq[sl, :],
                                    in1=vtmp[sl, :], op=ALU.add)
            nc.scalar.activation(out=sq[sl, :], in_=sq[sl, :], func=AF.Sqrt,
                                 bias=epst[c0:c0 + n, 0:1], scale=1.0)
            with nc.allow_low_precision(reason="f32r-typed LN stats rows"):
                nc.vector.reciprocal(out=rs[sl, :], in_=sq[sl, :])
                nc.vector.scalar_tensor_tensor(out=nb[sl, :], in0=ss[sl, :],
                                               scalar=-1.0,
                                               in1=rs[sl, :].bitcast(f32),
                                               op0=ALU.mult, op1=ALU.mult)

        # strip helper: drain a [1, n*CH] psum strip (partition 0) to a
        # 1-partition SBUF stage, then one DMA reshapes it into rows
        # [c0:c0+n] of the compact [32, CH] stats tiles.
        def strip_flush(pool, ps_s, ps_q, ss, sq, c0, n, qeng=None):
            stg_s = pool.tile([1, n * CH], f32, tag="stg_s")
            nc.scalar.copy(stg_s, ps_s[:, 0:n * CH])
            stg_q = pool.tile([1, n * CH], f32, tag="stg_q")
            if qeng is None:
                nc.scalar.copy(stg_q, ps_q[:, 0:n * CH])
            else:
                qeng.tensor_copy(out=stg_q, in_=ps_q[:, 0:n * CH])
            nc.sync.dma_start(out=ss[c0:c0 + n, :], in_=stg_s)
            nc.sync.dma_start(out=sq[c0:c0 + n, :], in_=stg_q)

        # ============ PH1: LN1 stats sweep ============
        with tc.tile_pool(name="ph1", bufs=3) as pool, \
             tc.tile_pool(name="ph1st", bufs=2) as sgpool, \
             tc.tile_pool(name="ph1ps", bufs=2, space="PSUM") as stps:
            ps_s = ps_q = None
            for g in range(NCHUNK // 4):
                xt4 = pool.tile([C, 4, CH], f32r, tag="xt")
                nc.sync.dma_start(out=xt4,
                                  in_=x_d[:, 16 * g:16 * g + 16, :].bitcast(f32r))
                for i in range(4):
                    c = 4 * g + i
                    if i % 2 == 0:
                        ps_s = stps.tile([1, 2 * CH], f32, tag="ps_s")
                        ps_q = stps.tile([1, 2 * CH], f32, tag="ps_q")
                    h = CH * (i % 2)
                    xt = xt4[:, i, :]
                    nc.tensor.matmul(ps_s[:, h:h + CH], lhsT=onescol,
                                     rhs=xt, start=True, stop=True)
                    xf = xt.bitcast(f32)
                    xsq = pool.tile([C, CH], f32r, tag="xsq")
                    nc.vector.tensor_tensor(out=xsq, in0=xf, in1=xf, op=ALU.mult)
                    nc.tensor.matmul(ps_q[:, h:h + CH], lhsT=onescol,
                                     rhs=xsq, start=True, stop=True)
                    if i % 2 == 1:
                        strip_flush(sgpool, ps_s, ps_q, ss1, sq1, c - 1, 2,
                                    qeng=nc.vector)
                    if i == 3:
                        if g == 7:
                            stats_math(ss1, sq1, rs1, nb1, 0, NCHUNK)

        # zero only the pad strips (interior fully overwritten by repack)
        for d in range(3):
            for t in (Kp[d], Vp[d]):
                nc.gpsimd.memset(t[:, 0:3, :], 0.0)
                nc.gpsimd.memset(t[:, 35:38, :], 0.0)
                nc.gpsimd.memset(t[:, 3:35, 0:3], 0.0)
                nc.gpsimd.memset(t[:, 3:35, 3 + W:PADC], 0.0)

        stgpool = tc.alloc_tile_pool(name="stg", bufs=1)
        stg_k = stgpool.tile([C, N], bf16)
        stg_v = stgpool.tile([C, N], bf16)


        # ============ PH2: LN1 apply + qkv + stage/scatter ============
        def band_rows(b):
            lo = max(0, BH * b - 3)
            hi = min(H, BH * b + BH + 3)
            return lo, hi

        # stage rs/nb rows at partitions {0,32,64} so rank-1 matmuls can
        # read them (PE base-partition rule); one strided DMA per 3 chunks
        def stage_stats(sgp, rs, nb, c0):
            n = min(3, NCHUNK - c0)
            srs = sgp.tile([C, CH], f32r, tag="srs")
            snb = sgp.tile([C, CH], f32r, tag="snb")
            dst_rs = srs.rearrange("(a b) f -> a b f", a=3)[0:n, 0:1, :]
            dst_nb = snb.rearrange("(a b) f -> a b f", a=3)[0:n, 0:1, :]
            nc.sync.dma_start(out=dst_rs, in_=rs[c0:c0 + n, :])
            nc.sync.dma_start(out=dst_nb, in_=nb[c0:c0 + n, :])
            return srs, snb

        with tc.tile_pool(name="ph2", bufs=3) as pool, \
             tc.tile_pool(name="ph2t", bufs=2) as tpool, \
             tc.tile_pool(name="ph2sg", bufs=2) as sgp, \
             tc.tile_pool(name="ph2ps", bufs=2, space="PSUM") as psum, \
             tc.tile_pool(name="ph2ps2", bufs=2, space="PSUM") as psum2:
            srs = snb = None
            for c in range(NCHUNK):
                g, i = c // 4, c % 4
                if i == 0:
                    xt4 = pool.tile([C, 4, CH], f32, tag="xt2")
                    nc.sync.dma_start(out=xt4, in_=x_d[:, 16 * g:16 * g + 16, :])
                if c % 3 == 0:
                    srs, snb = stage_stats(sgp, rs1, nb1, c)
                j = c % 3
                xt = xt4[:, i, :]
                pa = psum2.tile([C, CH], f32, tag="pa")
                nc.tensor.matmul(pa, lhsT=ones3x[32 * j:32 * j + 1, :],
                                 rhs=srs[32 * j:32 * j + 1, :],
                                 start=True, stop=True)
                xn = xns[c % 2]
                nc.vector.tensor_tensor(out=xn[0:C, :], in0=xt, in1=pa,
                                        op=ALU.mult)

                pq = psum.tile([C, CH], f32, tag="pq")
                pk = psum.tile([C, CH], f32, tag="pk")
                pv = psum.tile([C, CH], f32, tag="pv")
                snbj = snb[32 * j:32 * j + 1, :]
                for t, pt in enumerate((pq, pk, pv)):
                    nc.tensor.matmul(pt, lhsT=wsum3[32 * j:32 * j + 1,
                                                    C * t:C * (t + 1)],
                                     rhs=snbj, start=True, stop=False)
                    nc.tensor.matmul(pt, lhsT=wq[:, C * t:C * (t + 1)], rhs=xn,
                                     start=False, stop=True)

                # K/V -> contiguous staging (single 96-partition copies);
                # gpsimd cannot touch PSUM, so drains go DVE/Act only
                nc.scalar.copy(stg_k[:, CH * c:CH * (c + 1)], pk)
                nc.vector.tensor_copy(out=stg_v[:, CH * c:CH * (c + 1)], in_=pv)
                # Q -> band-packed planes directly
                b = c // 8
                r_off = 4 * c - BH * b
                for d in range(3):
                    src = pq[32 * d:32 * d + 32, :].rearrange("p (r w) -> p r w", r=4)
                    dst = Qd[d][32 * b:32 * b + 32, r_off:r_off + 4, :]
                    if d == 0:
                        nc.vector.tensor_copy(out=dst, in_=src)
                    else:
                        nc.scalar.copy(dst, src)

                # band-packed K/V repack via SBUF->SBUF DMA on idle rings
                if c in (8, 16, 24, 31):
                    b_ = (c - 1) // 8
                    lo, hi = band_rows(b_)
                    nr = hi - lo
                    r0 = lo - (BH * b_ - 3)
                    for d in range(3):
                        for stg, dstp in ((stg_k, Kp[d]), (stg_v, Vp[d])):
                            nc.sync.dma_start(
                                out=dstp[32 * b_:32 * b_ + 32, r0:r0 + nr, 3:3 + W],
                                in_=stg[32 * d:32 * d + 32, W * lo:W * hi]
                                    .rearrange("p (r w) -> p r w", r=nr))

        stgpool.release()

        # attention output (channel-major), in space freed by the staging
        ofpool = tc.alloc_tile_pool(name="ofp", bufs=1)
        ofull = ofpool.tile([C, N], bf16)

        # ============ PH3: attention per dilation ============
        # Processed in half-planes (16 band-rows each) so the softmax
        # denominator S accumulates across taps in PSUM via PE matmuls with
        # srepl (= repl/16), freeing DVE of the S adds entirely.
        HF = BH * W // 2                                # 2048 tokens per half
        with tc.tile_pool(name="ph3", bufs=4) as pool, \
             tc.tile_pool(name="ph3f", bufs=4) as fpool, \
             tc.tile_pool(name="ph3acc", bufs=3) as acc, \
             tc.tile_pool(name="ph3ps", bufs=2, space="PSUM") as psum, \
             tc.tile_pool(name="ph3sps", bufs=1, space="PSUM") as spsum:
            for di, dil in enumerate(DILS):
                for hh in range(2):
                    S_ps = spsum.tile([128, HF], f32, tag="Sps")
                    Oab = [acc.tile([128, HF], bf16, tag=f"O{p}",
                                    name=f"O{p}_{di}_{hh}") for p in range(2)]
                    rcp = acc.tile([128, HF], bf16, tag="rcp",
                                   name=f"rcp_{di}_{hh}")
                    qv = Qd[di][:, 16 * hh:16 * hh + 16, :]
                    for ti, (dr, dc) in enumerate([(i - 1, j - 1)
                                                   for i in range(3) for j in range(3)]):
                        r0 = 3 + dr * dil + 16 * hh
                        kwin = Kp[di][:, r0:r0 + 16, 3 + dc * dil:3 + dc * dil + W]
                        vwin = Vp[di][:, r0:r0 + 16, 3 + dc * dil:3 + dc * dil + W]
                        P = fpool.tile([128, HF], bf16, tag="P")
                        nc.vector.tensor_tensor(
                            out=P.rearrange("p (r w) -> p r w", r=16),
                            in0=qv, in1=kwin, op=ALU.mult)
                        # logits -> exp (overwrites P) -> S accumulation
                        for q in range(2):
                            pl = psum.tile([128, 1024], f32, tag="pl")
                            for j in range(2):
                                nc.tensor.matmul(
                                    pl[:, 512 * j:512 * (j + 1)], lhsT=repl,
                                    rhs=P[:, 1024 * q + 512 * j:
                                          1024 * q + 512 * (j + 1)],
                                    start=True, stop=True)
                            nc.scalar.activation(
                                out=P[:, 1024 * q:1024 * (q + 1)], in_=pl,
                                func=AF.Exp)
                            for j in range(2):
                                nc.tensor.matmul(
                                    S_ps[:, 1024 * q + 512 * j:
                                         1024 * q + 512 * (j + 1)],
                                    lhsT=srepl,
                                    rhs=P[:, 1024 * q + 512 * j:
                                          1024 * q + 512 * (j + 1)],
                                    start=(ti == 0), stop=(ti == 8))
                        ev = P.rearrange("p (r w) -> p r w", r=16)
                        Pv = Oab[ti] if ti < 2 else pool.tile([128, HF], bf16,
                                                              tag="Pv")
                        nc.vector.tensor_tensor(
                            out=Pv.rearrange("p (r w) -> p r w", r=16),
                            in0=ev, in1=vwin, op=ALU.mult)
                        if ti >= 2:
                            eng = nc.gpsimd if ti in (3, 4, 7) else nc.vector
                            eng.tensor_tensor(out=Oab[ti % 2], in0=Oab[ti % 2],
                                              in1=Pv, op=ALU.add)
                    with nc.allow_low_precision(reason="softmax recip bf16"):
                        nc.vector.reciprocal(out=rcp, in_=S_ps)
                    nc.vector.tensor_tensor(out=Oab[0], in0=Oab[0], in1=Oab[1],
                                            op=ALU.add)
                    nc.vector.tensor_tensor(out=Oab[0], in0=Oab[0], in1=rcp,
                                            op=ALU.mult)
                    for b in range(NB):
                        nc.sync.dma_start(
                            out=ofull[32 * di:32 * di + 32,
                                      4096 * b + 2048 * hh:
                                      4096 * b + 2048 * hh + 2048],
                            in_=Oab[0][32 * b:32 * b + 32, :])

        # ============ PH4: proj + residual + LN2 stats ============
        r1pool = tc.alloc_tile_pool(name="r1p", bufs=1)
        r1 = r1pool.tile([C, N], bf16)
        with tc.tile_pool(name="ph4", bufs=2) as pool, \
             tc.tile_pool(name="ph4sg", bufs=1) as sgpool, \
             tc.tile_pool(name="ph4st", bufs=1, space="PSUM") as stps, \
             tc.tile_pool(name="ph4ps", bufs=4, space="PSUM") as psum:
            ps_s = ps_q = None
            for c in range(NCHUNK):
                g, i = c // 4, c % 4
                if i == 0:
                    xt4 = pool.tile([C, 4, CH], f32, tag="xt4")
                    nc.sync.dma_start(out=xt4, in_=x_d[:, 16 * g:16 * g + 16, :])
                if c % 2 == 0:
                    ps_s = stps.tile([1, 2 * CH], f32, tag="ps_s4")
                    ps_q = stps.tile([1, 2 * CH], f32, tag="ps_q4")
                h4 = CH * (c % 2)
                pp = psum.tile([C, CH], f32, tag="pp")
                nc.tensor.matmul(pp, lhsT=pbt, rhs=onesrow, start=True, stop=False)
                nc.tensor.matmul(pp, lhsT=wp, rhs=ofull[:, CH * c:CH * (c + 1)],
                                 start=False, stop=True)
                rsl = r1[:, CH * c:CH * (c + 1)]
                nc.vector.tensor_tensor(out=rsl, in0=xt4[:, i, :],
                                        in1=pp, op=ALU.add)
                nc.tensor.matmul(ps_s[:, h4:h4 + CH], lhsT=onescol_b,
                                 rhs=rsl, start=True, stop=True)
                xsq = pool.tile([C, CH], f32r, tag="xsq5")
                nc.vector.tensor_tensor(out=xsq, in0=rsl, in1=rsl, op=ALU.mult)
                nc.tensor.matmul(ps_q[:, h4:h4 + CH], lhsT=onescol,
                                 rhs=xsq, start=True, stop=True)
                if c % 2 == 1:
                    strip_flush(sgpool, ps_s, ps_q, ss2, sq2, c - 1, 2)
                    if c == 31:
                        stats_math(ss2, sq2, rs2, nb2, 0, NCHUNK)

        # ============ PH5: MLP + residual ============
        with tc.tile_pool(name="ph5b", bufs=3) as pool, \
             tc.tile_pool(name="ph5h", bufs=2) as hpool, \
             tc.tile_pool(name="ph5y", bufs=2) as ypool, \
             tc.tile_pool(name="ph5sg", bufs=2) as sgp, \
             tc.tile_pool(name="ph5ps", bufs=3, space="PSUM") as psum, \
             tc.tile_pool(name="ph5ps2", bufs=1, space="PSUM") as psum2:
            srs = snb = None
            for c in range(NCHUNK):
                g, i = c // 4, c % 4
                rsl = r1[:, CH * c:CH * (c + 1)]
                if i == 0:
                    yout4 = ypool.tile([C, 4, CH], f32, tag="yout4")
                if c % 3 == 0:
                    srs, snb = stage_stats(sgp, rs2, nb2, c)
                j = c % 3
                pa = psum2.tile([C, CH], f32, tag="pa5")
                nc.tensor.matmul(pa, lhsT=ones3x[32 * j:32 * j + 1, :],
                                 rhs=srs[32 * j:32 * j + 1, :],
                                 start=True, stop=True)
                pb2 = psum2.tile([C, CH], f32, tag="pb5")
                nc.tensor.matmul(pb2, lhsT=ones3x[32 * j:32 * j + 1, :],
                                 rhs=snb[32 * j:32 * j + 1, :],
                                 start=True, stop=True)
                t1 = pool.tile([C, CH], f32, tag="t15")
                nc.vector.tensor_tensor(out=t1, in0=rsl, in1=pa, op=ALU.mult)
                xn = xns[c % 2]
                nc.vector.tensor_tensor(out=xn[0:C, :], in0=t1,
                                        in1=pb2, op=ALU.add)

                h1 = hpool.tile([128, 3, CH], bf16, tag="h1")
                for j in range(3):
                    pf = psum.tile([128, CH], f32, tag="pf")
                    nc.tensor.matmul(pf, lhsT=w1[:, 128 * j:128 * (j + 1)], rhs=xn,
                                     start=True, stop=True)
                    nc.scalar.activation(out=h1[:, j, :], in_=pf, func=AF.Gelu)
                pm = psum.tile([C, CH], f32, tag="pm")
                nc.tensor.matmul(pm, lhsT=b2t, rhs=onesrow, start=True, stop=False)
                for j in range(3):
                    nc.tensor.matmul(pm, lhsT=w2[j],
                                     rhs=h1[:, j, :], start=False, stop=(j == 2))
                nc.vector.tensor_tensor(out=yout4[:, i, :], in0=rsl,
                                        in1=pm, op=ALU.add)
                if i == 3:
                    nc.sync.dma_start(out=y_d[:, 16 * g:16 * g + 16, :], in_=yout4)

        r1pool.release()
        ofpool.release()
        apool.release()
        wpool.release()

    _split_multi_waits(nc, mybir)
    return nc


def _prep_weights(inputs):
    """Host-side weight preparation (fold LN affine, scale, bias rows)."""
    qkv_w = np.asarray(inputs['qkv_w'], np.float32)       # (288, 96)
    proj_w = np.asarray(inputs['proj_w'], np.float32)     # (96, 96)
    proj_b = np.asarray(inputs['proj_b'], np.float32)
    ln1_w = np.asarray(inputs['ln1_w'], np.float32)
    ln1_b = np.asarray(inputs['ln1_b'], np.float32)
    ln2_w = np.asarray(inputs['ln2_w'], np.float32)
    ln2_b = np.asarray(inputs['ln2_b'], np.float32)
    fc1_w = np.asarray(inputs['fc1_w'], np.float32)       # (384, 96)
    fc1_b = np.asarray(inputs['fc1_b'], np.float32)
    fc2_w = np.asarray(inputs['fc2_w'], np.float32)       # (96, 384)
    fc2_b = np.asarray(inputs['fc2_b'], np.float32)

    wq = qkv_w * ln1_w[None, :]                            # (288, 96)
    c0 = qkv_w @ ln1_b                                     # (288,)
    wq[0:C] *= SCALE                                       # scale q rows
    c0[0:C] *= SCALE
    wqb = np.concatenate([wq.T, c0[None, :]], axis=0)      # (97, 288)

    w1 = fc1_w * ln2_w[None, :]
    c1 = fc1_w @ ln2_b + fc1_b
    w1b = np.concatenate([w1.T, c1[None, :]], axis=0)      # (97, 384)

    repl = np.zeros((128, 128), np.float32)
    for b in range(NB):
        for ch in range(GD):
            h0 = (ch // HD) * HD
            repl[32 * b + h0:32 * b + h0 + HD, 32 * b + ch] = 1.0

    wsum3 = np.zeros((C, 3 * C), np.float32)
    for j in range(3):
        wsum3[32 * j, :] = wq.T[0:C, :].sum(axis=0)        # per-column sums

    return {
        'wsum3': wsum3,
        'wqkv': np.ascontiguousarray(wqb),                 # (97, 288) lhsT
        'wproj': np.ascontiguousarray(proj_w.T),           # (96, 96) lhsT
        'projb': proj_b.reshape(-1, 1).astype(np.float32),
        'w1': np.ascontiguousarray(w1b),                   # (97, 384) lhsT
        'w2': np.ascontiguousarray(fc2_w.T),               # (384, 96) lhsT
        'b2': fc2_b.reshape(-1, 1).astype(np.float32),
        'repl': repl,
        'onesc': np.ones((C, 1), np.float32),
    }


def kernel(**inputs):
    from concourse.bass_utils import run_bass_kernel_spmd

    if 'nc' not in _cache:
        t0 = time.time()
        _cache['nc'] = _build()
        print(f"[kernel] built bass module in {time.time() - t0:.1f}s",
              file=sys.stderr)

    nc = _cache['nc']
    wmap = _prep_weights(inputs)
    x = np.asarray(inputs['x'], np.float32)                # (8, 96, 128, 128)

    in_maps = []
    for b in range(B):
        m = {'x': np.ascontiguousarray(x[b])}
        m.update(wmap)
        in_maps.append(m)

    res = run_bass_kernel_spmd(nc, in_maps, core_ids=list(range(B)))
    _cache['last_exec_ns'] = res.exec_time_ns
    out = np.stack([res.results[b]['y'] for b in range(B)], axis=0)
    return out.astype(np.float32)


# revision 97
# speedup vs baseline: 1.0778x; 1.0069x over previous
"""DilateBlock kernel for 8x Trainium2 NeuronCores (Bass/Tile).

Data-parallel over batch B=8 (one image per core). Per core, the whole block
(LN1 -> qkv -> 3-dilation 3x3 neighborhood attention -> proj -> residual ->
LN2 -> MLP -> residual) runs in channels-on-partitions layout; spatial shifts
for the attention unfold live on the free dimension of zero-padded (h, w)
planes, packed 4-hbands x 32-channels across partitions.

Key tricks:
  - LayerNorm stats via ones-matmul on PE into a [32, 512] PSUM strip tile
    (chunk index on partitions), stats math runs wide on 32 partitions, and
    the per-token scale/shift rows feed rank-1 PSUM matmuls directly.
  - qkv/fc1 biases folded into the matmuls via a 97th ones-row of the
    LN-applied activations (contract-97 lhsT with a bias row).
  - K/V drained to contiguous staging as single 96-partition copies, then
    band-packed into padded planes by SBUF->SBUF DMAs on the idle DMA rings.
  - QK tap logits reduced over head_dim AND replicated back to all 16
    channel rows in one PE matmul with a static block-ones matrix; exp runs
    full width on Act; softmax denominator and output accumulate across taps
    via gpsimd DMA-accumulate (even/odd partial tiles, merged on DVE).
  - Softmax normalization applied to the attention OUTPUT.
  - Attention output repacked in SBUF (no DRAM roundtrip) for the proj.
"""
import sys
import time

sys.path.insert(0, '/opt/trn_rl_repo')

import numpy as np

# ---- problem constants (hardcoded per contract) ----
B, C, H, W = 8, 96, 128, 128
DILS = (1, 2, 3)
GD = 32                 # channels per dilation branch
HD = 16                 # head dim
NB = 4                  # h-bands packed on partitions
BH = H // NB            # rows per band = 32
N = H * W               # tokens per image
NCHUNK = 32             # token chunks of 512 (4 image rows each)
CH = N // NCHUNK        # 512
PADR = 38               # BH + 6 halo rows
PADC = 136              # W + 8 halo cols (EVEN pitch: enables DVE 2x mode)
EPS = 1e-5
SCALE = HD ** -0.5
MLPH = 384

_cache = {}
import os
_USE_DMA_ACCUM = os.environ.get('KDMA', '0') == '1'


def _patch_tile(tile_mod, bass_mod):
    """Work around this walrus build's 1-sem-wait-per-instruction limit and
    the multi-wait tail drain."""
    from concourse.vector_clock import ScopedClock, VectorClock

    def _drain_and_barrier(self, tick_clock, wait_clock):
        vclock = tick_clock.global_clock
        n = len(vclock)
        idxs = [i for i in range(n) if vclock[i] > 0]
        for i in idxs:
            vec = [0] * n
            vec[i] = vclock[i]
            nop_inst = self.nc.sync.nop(nofuse=True)
            wait_clock.add_sem_waits(nop_inst.ins,
                                     ScopedClock({None: VectorClock(vec)}))
        self.nc.sync.drain()
        self.nc.all_engine_barrier()
        popped = self.nc._tile_sem_poison_stack.pop()
        assert popped is self._sem_poison
        self.nc.clear_and_free_semaphores(list(self.sems.allocated().values()))
        self.nc.all_engine_barrier()

    tile_mod.TileContext._drain_and_barrier = _drain_and_barrier


_ws_counter = [0]


def _split_multi_waits(nc, mybir):
    for fn in nc.m.functions:
        for blk in fn.blocks:
            insts = list(blk.instructions)
            out = []
            changed = False
            for inst in insts:
                si = inst.sync_info
                waits = list(si.on_wait) if si and si.on_wait else []
                if len(waits) > 1:
                    for w in waits[:-1]:
                        _ws_counter[0] += 1
                        out.append(mybir.InstNoOp(
                            name=f"I-ws-{_ws_counter[0]}",
                            engine=inst.engine, ins=[], outs=[],
                            sync_info=mybir.SyncInfo(on_wait=[w], on_update=[])))
                    si.on_wait = [waits[-1]]
                    changed = True
                out.append(inst)
            if changed:
                blk.instructions[:] = out


def _build():
    import concourse.bass as bass
    import concourse.tile as tile
    from concourse import mybir

    _patch_tile(tile, bass)

    f32 = mybir.dt.float32
    f32r = mybir.dt.float32r
    bf16 = mybir.dt.bfloat16
    AF = mybir.ActivationFunctionType
    ALU = mybir.AluOpType

    nc = bass.Bass()

    # ---- DRAM I/O ----
    x_d = nc.dram_tensor("x", (C, H, W), f32, kind="ExternalInput")
    wq_d = nc.dram_tensor("wqkv", (C + 1, 3 * C), f32, kind="ExternalInput")  # lhsT+bias
    wp_d = nc.dram_tensor("wproj", (C, C), f32, kind="ExternalInput")         # lhsT
    pb_d = nc.dram_tensor("projb", (C, 1), f32, kind="ExternalInput")
    w1_d = nc.dram_tensor("w1", (C + 1, MLPH), f32, kind="ExternalInput")     # lhsT+bias
    w2_d = nc.dram_tensor("w2", (MLPH, C), f32, kind="ExternalInput")         # lhsT
    b2_d = nc.dram_tensor("b2", (C, 1), f32, kind="ExternalInput")
    repl_d = nc.dram_tensor("repl", (128, 128), f32, kind="ExternalInput")
    ws_d = nc.dram_tensor("wsum3", (C, 3 * C), f32, kind="ExternalInput")
    ones_d = nc.dram_tensor("onesc", (C, 1), f32, kind="ExternalInput")

    y_d = nc.dram_tensor("y", (C, H, W), f32, kind="ExternalOutput")

    with tile.TileContext(nc) as tc:
        # ---------------- persistent pools ----------------
        wpool = tc.alloc_tile_pool(name="weights", bufs=1)
        wq = wpool.tile([C + 1, 3 * C], f32r)
        nc.sync.dma_start(out=wq, in_=wq_d[:, :].bitcast(f32r))
        wp = wpool.tile([C, C], bf16)
        nc.gpsimd.dma_start(out=wp, in_=wp_d[:, :])     # gpsimd dma casts
        pbt = wpool.tile([1, C], f32r)                  # proj bias as rank-1 lhsT
        nc.sync.dma_start(out=pbt, in_=pb_d[:, :].rearrange("a b -> b a").bitcast(f32r))
        w1 = wpool.tile([C + 1, MLPH], f32r)
        nc.sync.dma_start(out=w1, in_=w1_d[:, :].bitcast(f32r))
        w2 = [wpool.tile([128, C], bf16, tag=f"w2{i}", name=f"w2{i}") for i in range(3)]
        for i in range(3):
            nc.gpsimd.dma_start(out=w2[i], in_=w2_d[128 * i:128 * (i + 1), :])
        b2t = wpool.tile([1, C], f32r)                  # fc2 bias as rank-1 lhsT
        nc.sync.dma_start(out=b2t, in_=b2_d[:, :].rearrange("a b -> b a").bitcast(f32r))
        repl = wpool.tile([128, 128], bf16)
        nc.gpsimd.dma_start(out=repl, in_=repl_d[:, :])
        srepl = wpool.tile([128, 128], bf16)            # repl/16: S accumulation
        nc.scalar.mul(out=srepl, in_=repl, mul=1.0 / HD)
        onescol = wpool.tile([C, 1], f32r)              # stats lhsT [96,1]
        nc.sync.dma_start(out=onescol, in_=ones_d[:, :].bitcast(f32r))
        onescol_b = wpool.tile([C, 1], bf16)            # bf16 variant (bf16 rhs)
        nc.vector.memset(onescol_b, 1.0)
        wsum3 = wpool.tile([C, 3 * C], f32r)            # qkv col-sums at {0,32,64}
        nc.sync.dma_start(out=wsum3, in_=ws_d[:, :].bitcast(f32r))
        ones1x = wpool.tile([1, C], f32r)               # rank-1 lhsT [1,96]
        nc.sync.dma_start(out=ones1x, in_=ones_d[:, :].rearrange("a b -> b a").bitcast(f32r))
        onesrow = wpool.tile([1, CH], f32r)             # static ones row (f32r)
        ones3x = wpool.tile([C, C], f32r)               # ones rows at {0,32,64}
        epst = wpool.tile([128, 1], f32)
        nc.vector.memset(epst, EPS)
        # LN-applied activation tiles with a persistent ones bias row
        xns = [wpool.tile([C + 1, CH], f32r, tag=f"xn{i}", name=f"xn{i}")
               for i in range(2)]
        # f32 scratch (init-only) to produce properly-rounded f32r constants
        initp = tc.alloc_tile_pool(name="initp", bufs=1)
        onesrow_f = initp.tile([1, CH], f32)
        nc.vector.memset(onesrow_f, 1.0)
        nc.vector.tensor_copy(out=onesrow, in_=onesrow_f)
        ones3f = initp.tile([C, C], f32)
        nc.vector.memset(ones3f, 0.0)
        for j in range(3):
            nc.vector.memset(ones3f[32 * j:32 * j + 1, :], 1.0)
        nc.vector.tensor_copy(out=ones3x, in_=ones3f)
        for i in range(2):
            nc.vector.tensor_copy(out=xns[i][C:C + 1, :], in_=onesrow_f)
        initp.release()
        # LN stats rows: rs/nb per chunk on partitions [32, 512]
        # (LN2 pass reuses the same tiles after LN1's readers are done)
        rs1 = wpool.tile([NCHUNK, CH], f32r)
        nb1 = wpool.tile([NCHUNK, CH], f32r)
        rs2, nb2 = rs1, nb1
        # channel-sum / channel-sumsq strips (chunk on partitions)
        ss1 = wpool.tile([NCHUNK, CH], f32)
        sq1 = wpool.tile([NCHUNK, CH], f32)
        ss2, sq2 = ss1, sq1

        # big persistent activation tensors
        apool = tc.alloc_tile_pool(name="acts", bufs=1)
        Qd = [apool.tile([128, BH, W], bf16, tag=f"qd{d}", name=f"qd{d}") for d in range(3)]
        Kp = [apool.tile([128, PADR, PADC], bf16, tag=f"kp{d}", name=f"kp{d}") for d in range(3)]
        Vp = [apool.tile([128, PADR, PADC], bf16, tag=f"vp{d}", name=f"vp{d}") for d in range(3)]


        # ============ shared stats math ============
        # strips ss/sq: [32, CH] SBUF, chunk on partitions.  Runs on row
        # slices [c0:c0+n] right after each flush so downstream chunks can
        # start without waiting for the whole stats pass.  mu overwrites ss,
        # var/sd overwrite sq in place; vtmp holds -mu^2.
        vtmp = wpool.tile([NCHUNK, CH], f32)

        def stats_math_early(ss, sq, rs, nb):
            sl = slice(0, 8)
            mu = vtmp[sl, :]
            nc.scalar.mul(out=mu, in_=ss[sl, :], mul=1.0 / C)
            t = rs[sl, :].bitcast(f32)
            nc.scalar.mul(out=t, in_=sq[sl, :], mul=1.0 / C)
            nc.vector.scalar_tensor_tensor(out=nb[sl, :].bitcast(f32), in0=mu,
                                           scalar=-1.0, in1=mu,
                                           op0=ALU.mult, op1=ALU.mult)
            nc.vector.tensor_tensor(out=t, in0=t, in1=nb[sl, :].bitcast(f32),
                                    op=ALU.add)
            nc.scalar.activation(out=t, in_=t, func=AF.Sqrt,
                                 bias=epst[0:8, 0:1], scale=1.0)
            with nc.allow_low_precision(reason="f32r-typed LN stats rows"):
                nc.vector.reciprocal(out=rs[sl, :], in_=t)
                nc.vector.scalar_tensor_tensor(out=nb[sl, :], in0=mu,
                                               scalar=-1.0,
                                               in1=rs[sl, :].bitcast(f32),
                                               op0=ALU.mult, op1=ALU.mult)

        def stats_math(ss, sq, rs, nb, c0, n):
            sl = slice(c0, c0 + n)
            nc.scalar.mul(out=ss[sl, :], in_=ss[sl, :], mul=1.0 / C)
            nc.scalar.mul(out=sq[sl, :], in_=sq[sl, :], mul=1.0 / C)
            nc.vector.scalar_tensor_tensor(out=vtmp[sl, :], in0=ss[sl, :],
                                           scalar=-1.0, in1=ss[sl, :],
                                           op0=ALU.mult, op1=ALU.mult)
            nc.vector.tensor_tensor(out=sq[sl, :], in0=sq[sl, :],
                                    in1=vtmp[sl, :], op=ALU.add)
            nc.scalar.activation(out=sq[sl, :], in_=sq[sl, :], func=AF.Sqrt,
                                 bias=epst[c0:c0 + n, 0:1], scale=1.0)
            with nc.allow_low_precision(reason="f32r-typed LN stats rows"):
                nc.vector.reciprocal(out=rs[sl, :], in_=sq[sl, :])
                nc.vector.scalar_tensor_tensor(out=nb[sl, :], in0=ss[sl, :],
                                               scalar=-1.0,
                                               in1=rs[sl, :].bitcast(f32),
                                               op0=ALU.mult, op1=ALU.mult)

        # strip helper: drain a [1, n*CH] psum strip (partition 0) to a
        # 1-partition SBUF stage, then one DMA reshapes it into rows
        # [c0:c0+n] of the compact [32, CH] stats tiles.
        def strip_flush(pool, ps_s, ps_q, ss, sq, c0, n, qeng=None):
            stg_s = pool.tile([1, n * CH], f32, tag="stg_s")
            nc.scalar.copy(stg_s, ps_s[:, 0:n * CH])
            stg_q = pool.tile([1, n * CH], f32, tag="stg_q")
            if qeng is None:
                nc.scalar.copy(stg_q, ps_q[:, 0:n * CH])
            else:
                qeng.tensor_copy(out=stg_q, in_=ps_q[:, 0:n * CH])
            nc.sync.dma_start(out=ss[c0:c0 + n, :], in_=stg_s)
            nc.sync.dma_start(out=sq[c0:c0 + n, :], in_=stg_q)

        # ============ PH1: LN1 stats sweep ============
        with tc.tile_pool(name="ph1", bufs=3) as pool, \
             tc.tile_pool(name="ph1st", bufs=2) as sgpool, \
             tc.tile_pool(name="ph1ps", bufs=2, space="PSUM") as stps:
            ps_s = ps_q = None
            for g in range(NCHUNK // 4):
                xt4 = pool.tile([C, 4, CH], f32r, tag="xt")
                nc.sync.dma_start(out=xt4,
                                  in_=x_d[:, 16 * g:16 * g + 16, :].bitcast(f32r))
                for i in range(4):
                    c = 4 * g + i
                    if i % 2 == 0:
                        ps_s = stps.tile([1, 2 * CH], f32, tag="ps_s")
                        ps_q = stps.tile([1, 2 * CH], f32, tag="ps_q")
                    h = CH * (i % 2)
                    xt = xt4[:, i, :]
                    nc.tensor.matmul(ps_s[:, h:h + CH], lhsT=onescol,
                                     rhs=xt, start=True, stop=True)
                    xf = xt.bitcast(f32)
                    xsq = pool.tile([C, CH], f32r, tag="xsq")
                    nc.vector.tensor_tensor(out=xsq, in0=xf, in1=xf, op=ALU.mult)
                    nc.tensor.matmul(ps_q[:, h:h + CH], lhsT=onescol,
                                     rhs=xsq, start=True, stop=True)
                    if i % 2 == 1:
                        strip_flush(sgpool, ps_s, ps_q, ss1, sq1, c - 1, 2,
                                    qeng=nc.vector)
                    if i == 3:
                        if g == 7:
                            stats_math(ss1, sq1, rs1, nb1, 0, NCHUNK)

        # zero only the pad strips (interior fully overwritten by repack)
        for d in range(3):
            for t in (Kp[d], Vp[d]):
                nc.gpsimd.memset(t[:, 0:3, :], 0.0)
                nc.gpsimd.memset(t[:, 35:38, :], 0.0)
                nc.gpsimd.memset(t[:, 3:35, 0:3], 0.0)
                nc.gpsimd.memset(t[:, 3:35, 3 + W:PADC], 0.0)

        stgpool = tc.alloc_tile_pool(name="stg", bufs=1)
        stg_k = stgpool.tile([C, N], bf16)
        stg_v = stgpool.tile([C, N], bf16)


        # ============ PH2: LN1 apply + qkv + stage/scatter ============
        def band_rows(b):
            lo = max(0, BH * b - 3)
            hi = min(H, BH * b + BH + 3)
            return lo, hi

        # stage rs/nb rows at partitions {0,32,64} so rank-1 matmuls can
        # read them (PE base-partition rule); one strided DMA per 3 chunks
        def stage_stats(sgp, rs, nb, c0):
            n = min(3, NCHUNK - c0)
            srs = sgp.tile([C, CH], f32r, tag="srs")
            snb = sgp.tile([C, CH], f32r, tag="snb")
            dst_rs = srs.rearrange("(a b) f -> a b f", a=3)[0:n, 0:1, :]
            dst_nb = snb.rearrange("(a b) f -> a b f", a=3)[0:n, 0:1, :]
            nc.sync.dma_start(out=dst_rs, in_=rs[c0:c0 + n, :])
            nc.sync.dma_start(out=dst_nb, in_=nb[c0:c0 + n, :])
            return srs, snb

        with tc.tile_pool(name="ph2", bufs=3) as pool, \
             tc.tile_pool(name="ph2t", bufs=2) as tpool, \
             tc.tile_pool(name="ph2sg", bufs=2) as sgp, \
             tc.tile_pool(name="ph2ps", bufs=2, space="PSUM") as psum, \
             tc.tile_pool(name="ph2ps2", bufs=2, space="PSUM") as psum2:
            srs = snb = None
            for c in range(NCHUNK):
                g, i = c // 4, c % 4
                if i == 0:
                    xt4 = pool.tile([C, 4, CH], f32, tag="xt2")
                    nc.sync.dma_start(out=xt4, in_=x_d[:, 16 * g:16 * g + 16, :])
                if c % 3 == 0:
                    srs, snb = stage_stats(sgp, rs1, nb1, c)
                j = c % 3
                xt = xt4[:, i, :]
                pa = psum2.tile([C, CH], f32, tag="pa")
                nc.tensor.matmul(pa, lhsT=ones3x[32 * j:32 * j + 1, :],
                                 rhs=srs[32 * j:32 * j + 1, :],
                                 start=True, stop=True)
                xn = xns[c % 2]
                nc.vector.tensor_tensor(out=xn[0:C, :], in0=xt, in1=pa,
                                        op=ALU.mult)

                pq = psum.tile([C, CH], f32, tag="pq")
                pk = psum.tile([C, CH], f32, tag="pk")
                pv = psum.tile([C, CH], f32, tag="pv")
                snbj = snb[32 * j:32 * j + 1, :]
                for t, pt in enumerate((pq, pk, pv)):
                    nc.tensor.matmul(pt, lhsT=wsum3[32 * j:32 * j + 1,
                                                    C * t:C * (t + 1)],
                                     rhs=snbj, start=True, stop=False)
                    nc.tensor.matmul(pt, lhsT=wq[:, C * t:C * (t + 1)], rhs=xn,
                                     start=False, stop=True)

                # K/V -> contiguous staging (single 96-partition copies);
                # gpsimd cannot touch PSUM, so drains go DVE/Act only
                nc.scalar.copy(stg_k[:, CH * c:CH * (c + 1)], pk)
                nc.vector.tensor_copy(out=stg_v[:, CH * c:CH * (c + 1)], in_=pv)
                # Q -> band-packed planes directly
                b = c // 8
                r_off = 4 * c - BH * b
                for d in range(3):
                    src = pq[32 * d:32 * d + 32, :].rearrange("p (r w) -> p r w", r=4)
                    dst = Qd[d][32 * b:32 * b + 32, r_off:r_off + 4, :]
                    if d == 0:
                        nc.vector.tensor_copy(out=dst, in_=src)
                    else:
                        nc.scalar.copy(dst, src)

                # band-packed K/V repack via SBUF->SBUF DMA on idle rings
                if c in (8, 16, 24, 31):
                    b_ = (c - 1) // 8
                    lo, hi = band_rows(b_)
                    nr = hi - lo
                    r0 = lo - (BH * b_ - 3)
                    for d in range(3):
                        for stg, dstp in ((stg_k, Kp[d]), (stg_v, Vp[d])):
                            nc.sync.dma_start(
                                out=dstp[32 * b_:32 * b_ + 32, r0:r0 + nr, 3:3 + W],
                                in_=stg[32 * d:32 * d + 32, W * lo:W * hi]
                                    .rearrange("p (r w) -> p r w", r=nr))

        stgpool.release()

        # attention output (channel-major), in space freed by the staging
        ofpool = tc.alloc_tile_pool(name="ofp", bufs=1)
        ofull = ofpool.tile([C, N], bf16)

        # ============ PH3: attention per dilation ============
        # Processed in half-planes (16 band-rows each) so the softmax
        # denominator S accumulates across taps in PSUM via PE matmuls with
        # srepl (= repl/16), freeing DVE of the S adds entirely.
        HF = BH * W // 2                                # 2048 tokens per half
        with tc.tile_pool(name="ph3", bufs=4) as pool, \
             tc.tile_pool(name="ph3f", bufs=4) as fpool, \
             tc.tile_pool(name="ph3acc", bufs=3) as acc, \
             tc.tile_pool(name="ph3ps", bufs=2, space="PSUM") as psum, \
             tc.tile_pool(name="ph3sps", bufs=1, space="PSUM") as spsum:
            for di, dil in enumerate(DILS):
                for hh in range(2):
                    S_ps = spsum.tile([128, HF], f32, tag="Sps")
                    Oab = [acc.tile([128, HF], bf16, tag=f"O{p}",
                                    name=f"O{p}_{di}_{hh}") for p in range(2)]
                    rcp = acc.tile([128, HF], bf16, tag="rcp",
                                   name=f"rcp_{di}_{hh}")
                    qv = Qd[di][:, 16 * hh:16 * hh + 16, :]
                    taps = [(i - 1, j - 1) for i in range(3) for j in range(3)]

                    def emit_p(ti):
                        dr, dc = taps[ti]
                        r0 = 3 + dr * dil + 16 * hh
                        kwin = Kp[di][:, r0:r0 + 16,
                                      3 + dc * dil:3 + dc * dil + W]
                        P = fpool.tile([128, HF], bf16, tag="P",
                                       name=f"P_{di}_{hh}_{ti}")
                        nc.vector.tensor_tensor(
                            out=P.rearrange("p (r w) -> p r w", r=16),
                            in0=qv, in1=kwin, op=ALU.mult)
                        return P

                    Pnext = emit_p(0)
                    for ti, (dr, dc) in enumerate(taps):
                        r0 = 3 + dr * dil + 16 * hh
                        vwin = Vp[di][:, r0:r0 + 16, 3 + dc * dil:3 + dc * dil + W]
                        P = Pnext
                        # logits -> exp (overwrites P) -> S accumulation
                        for q in range(2):
                            pl = psum.tile([128, 1024], f32, tag="pl")
                            for j in range(2):
                                nc.tensor.matmul(
                                    pl[:, 512 * j:512 * (j + 1)], lhsT=repl,
                                    rhs=P[:, 1024 * q + 512 * j:
                                          1024 * q + 512 * (j + 1)],
                                    start=True, stop=True)
                            nc.scalar.activation(
                                out=P[:, 1024 * q:1024 * (q + 1)], in_=pl,
                                func=AF.Exp)
                            for j in range(2):
                                nc.tensor.matmul(
                                    S_ps[:, 1024 * q + 512 * j:
                                         1024 * q + 512 * (j + 1)],
                                    lhsT=srepl,
                                    rhs=P[:, 1024 * q + 512 * j:
                                          1024 * q + 512 * (j + 1)],
                                    start=(ti == 0), stop=(ti == 8))
                        # next tap's P-mult ahead of Pv in the DVE queue
                        if ti < 8:
                            Pnext = emit_p(ti + 1)
                        ev = P.rearrange("p (r w) -> p r w", r=16)
                        Pv = Oab[ti] if ti < 2 else pool.tile([128, HF], bf16,
                                                              tag="Pv")
                        nc.vector.tensor_tensor(
                            out=Pv.rearrange("p (r w) -> p r w", r=16),
                            in0=ev, in1=vwin, op=ALU.mult)
                        if ti >= 2:
                            eng = nc.gpsimd if ti in (3, 4, 7) else nc.vector
                            eng.tensor_tensor(out=Oab[ti % 2], in0=Oab[ti % 2],
                                              in1=Pv, op=ALU.add)
                    with nc.allow_low_precision(reason="softmax recip bf16"):
                        nc.vector.reciprocal(out=rcp, in_=S_ps)
                    nc.vector.tensor_tensor(out=Oab[0], in0=Oab[0], in1=Oab[1],
                                            op=ALU.add)
                    nc.vector.tensor_tensor(out=Oab[0], in0=Oab[0], in1=rcp,
                                            op=ALU.mult)
                    for b in range(NB):
                        nc.sync.dma_start(
                            out=ofull[32 * di:32 * di + 32,
                                      4096 * b + 2048 * hh:
                                      4096 * b + 2048 * hh + 2048],
                            in_=Oab[0][32 * b:32 * b + 32, :])

        # ============ PH4: proj + residual + LN2 stats ============
        r1pool = tc.alloc_tile_pool(name="r1p", bufs=1)
        r1 = r1pool.tile([C, N], bf16)
        with tc.tile_pool(name="ph4", bufs=2) as pool, \
             tc.tile_pool(name="ph4sg", bufs=1) as sgpool, \
             tc.tile_pool(name="ph4st", bufs=1, space="PSUM") as stps, \
             tc.tile_pool(name="ph4ps", bufs=4, space="PSUM") as psum:
            ps_s = ps_q = None
            for c in range(NCHUNK):
                g, i = c // 4, c % 4
                if i == 0:
                    xt4 = pool.tile([C, 4, CH], f32, tag="xt4")
                    nc.sync.dma_start(out=xt4, in_=x_d[:, 16 * g:16 * g + 16, :])
                if c % 2 == 0:
                    ps_s = stps.tile([1, 2 * CH], f32, tag="ps_s4")
                    ps_q = stps.tile([1, 2 * CH], f32, tag="ps_q4")
                h4 = CH * (c % 2)
                pp = psum.tile([C, CH], f32, tag="pp")
                nc.tensor.matmul(pp, lhsT=pbt, rhs=onesrow, start=True, stop=False)
                nc.tensor.matmul(pp, lhsT=wp, rhs=ofull[:, CH * c:CH * (c + 1)],
                                 start=False, stop=True)
                rsl = r1[:, CH * c:CH * (c + 1)]
                nc.vector.tensor_tensor(out=rsl, in0=xt4[:, i, :],
                                        in1=pp, op=ALU.add)
                nc.tensor.matmul(ps_s[:, h4:h4 + CH], lhsT=onescol_b,
                                 rhs=rsl, start=True, stop=True)
                xsq = pool.tile([C, CH], f32r, tag="xsq5")
                nc.vector.tensor_tensor(out=xsq, in0=rsl, in1=rsl, op=ALU.mult)
                nc.tensor.matmul(ps_q[:, h4:h4 + CH], lhsT=onescol,
                                 rhs=xsq, start=True, stop=True)
                if c % 2 == 1:
                    strip_flush(sgpool, ps_s, ps_q, ss2, sq2, c - 1, 2)
                    if c == 31:
                        stats_math(ss2, sq2, rs2, nb2, 0, NCHUNK)

        # ============ PH5: MLP + residual ============
        with tc.tile_pool(name="ph5b", bufs=3) as pool, \
             tc.tile_pool(name="ph5h", bufs=2) as hpool, \
             tc.tile_pool(name="ph5y", bufs=2) as ypool, \
             tc.tile_pool(name="ph5sg", bufs=2) as sgp, \
             tc.tile_pool(name="ph5ps", bufs=3, space="PSUM") as psum, \
             tc.tile_pool(name="ph5ps2", bufs=1, space="PSUM") as psum2:
            srs = snb = None
            for c in range(NCHUNK):
                g, i = c // 4, c % 4
                rsl = r1[:, CH * c:CH * (c + 1)]
                if i == 0:
                    yout4 = ypool.tile([C, 4, CH], f32, tag="yout4")
                if c % 3 == 0:
                    srs, snb = stage_stats(sgp, rs2, nb2, c)
                j = c % 3
                pa = psum2.tile([C, CH], f32, tag="pa5")
                nc.tensor.matmul(pa, lhsT=ones3x[32 * j:32 * j + 1, :],
                                 rhs=srs[32 * j:32 * j + 1, :],
                                 start=True, stop=True)
                pb2 = psum2.tile([C, CH], f32, tag="pb5")
                nc.tensor.matmul(pb2, lhsT=ones3x[32 * j:32 * j + 1, :],
                                 rhs=snb[32 * j:32 * j + 1, :],
                                 start=True, stop=True)
                t1 = pool.tile([C, CH], f32, tag="t15")
                nc.vector.tensor_tensor(out=t1, in0=rsl, in1=pa, op=ALU.mult)
                xn = xns[c % 2]
                nc.vector.tensor_tensor(out=xn[0:C, :], in0=t1,
                                        in1=pb2, op=ALU.add)

                h1 = hpool.tile([128, 3, CH], bf16, tag="h1")
                for j in range(3):
                    pf = psum.tile([128, CH], f32, tag="pf")
                    nc.tensor.matmul(pf, lhsT=w1[:, 128 * j:128 * (j + 1)], rhs=xn,
                                     start=True, stop=True)
                    nc.scalar.activation(out=h1[:, j, :], in_=pf, func=AF.Gelu)
                pm = psum.tile([C, CH], f32, tag="pm")
                nc.tensor.matmul(pm, lhsT=b2t, rhs=onesrow, start=True, stop=False)
                for j in range(3):
                    nc.tensor.matmul(pm, lhsT=w2[j],
                                     rhs=h1[:, j, :], start=False, stop=(j == 2))
                nc.vector.tensor_tensor(out=yout4[:, i, :], in0=rsl,
                                        in1=pm, op=ALU.add)
                if i == 3:
                    nc.sync.dma_start(out=y_d[:, 16 * g:16 * g + 16, :], in_=yout4)

        r1pool.release()
        ofpool.release()
        apool.release()
        wpool.release()

    _split_multi_waits(nc, mybir)
    return nc


def _prep_weights(inputs):
    """Host-side weight preparation (fold LN affine, scale, bias rows)."""
    qkv_w = np.asarray(inputs['qkv_w'], np.float32)       # (288, 96)
    proj_w = np.asarray(inputs['proj_w'], np.float32)     # (96, 96)
    proj_b = np.asarray(inputs['proj_b'], np.float32)
    ln1_w = np.asarray(inputs['ln1_w'], np.float32)
    ln1_b = np.asarray(inputs['ln1_b'], np.float32)
    ln2_w = np.asarray(inputs['ln2_w'], np.float32)
    ln2_b = np.asarray(inputs['ln2_b'], np.float32)
    fc1_w = np.asarray(inputs['fc1_w'], np.float32)       # (384, 96)
    fc1_b = np.asarray(inputs['fc1_b'], np.float32)
    fc2_w = np.asarray(inputs['fc2_w'], np.float32)       # (96, 384)
    fc2_b = np.asarray(inputs['fc2_b'], np.float32)

    wq = qkv_w * ln1_w[None, :]                            # (288, 96)
    c0 = qkv_w @ ln1_b                                     # (288,)
    wq[0:C] *= SCALE                                       # scale q rows
    c0[0:C] *= SCALE
    wqb = np.concatenate([wq.T, c0[None, :]], axis=0)      # (97, 288)

    w1 = fc1_w * ln2_w[None, :]
    c1 = fc1_w @ ln2_b + fc1_b
    w1b = np.concatenate([w1.T, c1[None, :]], axis=0)      # (97, 384)

    repl = np.zeros((128, 128), np.float32)
    for b in range(NB):
        for ch in range(GD):
            h0 = (ch // HD) * HD
            repl[32 * b + h0:32 * b + h0 + HD, 32 * b + ch] = 1.0

    wsum3 = np.zeros((C, 3 * C), np.float32)
    for j in range(3):
        wsum3[32 * j, :] = wq.T[0:C, :].sum(axis=0)        # per-column sums

    return {
        'wsum3': wsum3,
        'wqkv': np.ascontiguousarray(wqb),                 # (97, 288) lhsT
        'wproj': np.ascontiguousarray(proj_w.T),           # (96, 96) lhsT
        'projb': proj_b.reshape(-1, 1).astype(np.float32),
        'w1': np.ascontiguousarray(w1b),                   # (97, 384) lhsT
        'w2': np.ascontiguousarray(fc2_w.T),               # (384, 96) lhsT
        'b2': fc2_b.reshape(-1, 1).astype(np.float32),
        'repl': repl,
        'onesc': np.ones((C, 1), np.float32),
    }


def kernel(**inputs):
    from concourse.bass_utils import run_bass_kernel_spmd

    if 'nc' not in _cache:
        t0 = time.time()
        _cache['nc'] = _build()
        print(f"[kernel] built bass module in {time.time() - t0:.1f}s",
              file=sys.stderr)

    nc = _cache['nc']
    wmap = _prep_weights(inputs)
    x = np.asarray(inputs['x'], np.float32)                # (8, 96, 128, 128)

    in_maps = []
    for b in range(B):
        m = {'x': np.ascontiguousarray(x[b])}
        m.update(wmap)
        in_maps.append(m)

    res = run_bass_kernel_spmd(nc, in_maps, core_ids=list(range(B)))
    _cache['last_exec_ns'] = res.exec_time_ns
    out = np.stack([res.results[b]['y'] for b in range(B)], axis=0)
    return out.astype(np.float32)


# revision 98
# speedup vs baseline: 1.0791x; 1.0012x over previous
"""DilateBlock kernel for 8x Trainium2 NeuronCores (Bass/Tile).

Data-parallel over batch B=8 (one image per core). Per core, the whole block
(LN1 -> qkv -> 3-dilation 3x3 neighborhood attention -> proj -> residual ->
LN2 -> MLP -> residual) runs in channels-on-partitions layout; spatial shifts
for the attention unfold live on the free dimension of zero-padded (h, w)
planes, packed 4-hbands x 32-channels across partitions.

Key tricks:
  - LayerNorm stats via ones-matmul on PE into a [32, 512] PSUM strip tile
    (chunk index on partitions), stats math runs wide on 32 partitions, and
    the per-token scale/shift rows feed rank-1 PSUM matmuls directly.
  - qkv/fc1 biases folded into the matmuls via a 97th ones-row of the
    LN-applied activations (contract-97 lhsT with a bias row).
  - K/V drained to contiguous staging as single 96-partition copies, then
    band-packed into padded planes by SBUF->SBUF DMAs on the idle DMA rings.
  - QK tap logits reduced over head_dim AND replicated back to all 16
    channel rows in one PE matmul with a static block-ones matrix; exp runs
    full width on Act; softmax denominator and output accumulate across taps
    via gpsimd DMA-accumulate (even/odd partial tiles, merged on DVE).
  - Softmax normalization applied to the attention OUTPUT.
  - Attention output repacked in SBUF (no DRAM roundtrip) for the proj.
"""
import sys
import time

sys.path.insert(0, '/opt/trn_rl_repo')

import numpy as np

# ---- problem constants (hardcoded per contract) ----
B, C, H, W = 8, 96, 128, 128
DILS = (1, 2, 3)
GD = 32                 # channels per dilation branch
HD = 16                 # head dim
NB = 4                  # h-bands packed on partitions
BH = H // NB            # rows per band = 32
N = H * W               # tokens per image
NCHUNK = 32             # token chunks of 512 (4 image rows each)
CH = N // NCHUNK        # 512
PADR = 38               # BH + 6 halo rows
PADC = 136              # W + 8 halo cols (EVEN pitch: enables DVE 2x mode)
EPS = 1e-5
SCALE = HD ** -0.5
MLPH = 384

_cache = {}
import os
_USE_DMA_ACCUM = os.environ.get('KDMA', '0') == '1'


def _patch_tile(tile_mod, bass_mod):
    """Work around this walrus build's 1-sem-wait-per-instruction limit and
    the multi-wait tail drain."""
    from concourse.vector_clock import ScopedClock, VectorClock

    def _drain_and_barrier(self, tick_clock, wait_clock):
        vclock = tick_clock.global_clock
        n = len(vclock)
        idxs = [i for i in range(n) if vclock[i] > 0]
        for i in idxs:
            vec = [0] * n
            vec[i] = vclock[i]
            nop_inst = self.nc.sync.nop(nofuse=True)
            wait_clock.add_sem_waits(nop_inst.ins,
                                     ScopedClock({None: VectorClock(vec)}))
        self.nc.sync.drain()
        self.nc.all_engine_barrier()
        popped = self.nc._tile_sem_poison_stack.pop()
        assert popped is self._sem_poison
        self.nc.clear_and_free_semaphores(list(self.sems.allocated().values()))
        self.nc.all_engine_barrier()

    tile_mod.TileContext._drain_and_barrier = _drain_and_barrier


_ws_counter = [0]


def _split_multi_waits(nc, mybir):
    for fn in nc.m.functions:
        for blk in fn.blocks:
            insts = list(blk.instructions)
            out = []
            changed = False
            for inst in insts:
                si = inst.sync_info
                waits = list(si.on_wait) if si and si.on_wait else []
                if len(waits) > 1:
                    for w in waits[:-1]:
                        _ws_counter[0] += 1
                        out.append(mybir.InstNoOp(
                            name=f"I-ws-{_ws_counter[0]}",
                            engine=inst.engine, ins=[], outs=[],
                            sync_info=mybir.SyncInfo(on_wait=[w], on_update=[])))
                    si.on_wait = [waits[-1]]
                    changed = True
                out.append(inst)
            if changed:
                blk.instructions[:] = out


def _build():
    import concourse.bass as bass
    import concourse.tile as tile
    from concourse import mybir

    _patch_tile(tile, bass)

    f32 = mybir.dt.float32
    f32r = mybir.dt.float32r
    bf16 = mybir.dt.bfloat16
    AF = mybir.ActivationFunctionType
    ALU = mybir.AluOpType

    nc = bass.Bass()

    # ---- DRAM I/O ----
    x_d = nc.dram_tensor("x", (C, H, W), f32, kind="ExternalInput")
    wq_d = nc.dram_tensor("wqkv", (C + 1, 3 * C), f32, kind="ExternalInput")  # lhsT+bias
    wp_d = nc.dram_tensor("wproj", (C, C), f32, kind="ExternalInput")         # lhsT
    pb_d = nc.dram_tensor("projb", (C, 1), f32, kind="ExternalInput")
    w1_d = nc.dram_tensor("w1", (C + 1, MLPH), f32, kind="ExternalInput")     # lhsT+bias
    w2_d = nc.dram_tensor("w2", (MLPH, C), f32, kind="ExternalInput")         # lhsT
    b2_d = nc.dram_tensor("b2", (C, 1), f32, kind="ExternalInput")
    repl_d = nc.dram_tensor("repl", (128, 128), f32, kind="ExternalInput")
    ws_d = nc.dram_tensor("wsum3", (C, 3 * C), f32, kind="ExternalInput")
    ones_d = nc.dram_tensor("onesc", (C, 1), f32, kind="ExternalInput")

    y_d = nc.dram_tensor("y", (C, H, W), f32, kind="ExternalOutput")

    with tile.TileContext(nc) as tc:
        # ---------------- persistent pools ----------------
        wpool = tc.alloc_tile_pool(name="weights", bufs=1)
        wq = wpool.tile([C + 1, 3 * C], f32r)
        nc.sync.dma_start(out=wq, in_=wq_d[:, :].bitcast(f32r))
        wp = wpool.tile([C, C], bf16)
        nc.gpsimd.dma_start(out=wp, in_=wp_d[:, :])     # gpsimd dma casts
        pbt = wpool.tile([1, C], f32r)                  # proj bias as rank-1 lhsT
        nc.sync.dma_start(out=pbt, in_=pb_d[:, :].rearrange("a b -> b a").bitcast(f32r))
        w1 = wpool.tile([C + 1, MLPH], f32r)
        nc.sync.dma_start(out=w1, in_=w1_d[:, :].bitcast(f32r))
        w2 = [wpool.tile([128, C], bf16, tag=f"w2{i}", name=f"w2{i}") for i in range(3)]
        for i in range(3):
            nc.gpsimd.dma_start(out=w2[i], in_=w2_d[128 * i:128 * (i + 1), :])
        b2t = wpool.tile([1, C], f32r)                  # fc2 bias as rank-1 lhsT
        nc.sync.dma_start(out=b2t, in_=b2_d[:, :].rearrange("a b -> b a").bitcast(f32r))
        repl = wpool.tile([128, 128], bf16)
        nc.gpsimd.dma_start(out=repl, in_=repl_d[:, :])
        srepl = wpool.tile([128, 128], bf16)            # repl/16: S accumulation
        nc.scalar.mul(out=srepl, in_=repl, mul=1.0 / HD)
        onescol = wpool.tile([C, 1], f32r)              # stats lhsT [96,1]
        nc.sync.dma_start(out=onescol, in_=ones_d[:, :].bitcast(f32r))
        onescol_b = wpool.tile([C, 1], bf16)            # bf16 variant (bf16 rhs)
        nc.vector.memset(onescol_b, 1.0)
        wsum3 = wpool.tile([C, 3 * C], f32r)            # qkv col-sums at {0,32,64}
        nc.sync.dma_start(out=wsum3, in_=ws_d[:, :].bitcast(f32r))
        ones1x = wpool.tile([1, C], f32r)               # rank-1 lhsT [1,96]
        nc.sync.dma_start(out=ones1x, in_=ones_d[:, :].rearrange("a b -> b a").bitcast(f32r))
        onesrow = wpool.tile([1, CH], f32r)             # static ones row (f32r)
        ones3x = wpool.tile([C, C], f32r)               # ones rows at {0,32,64}
        epst = wpool.tile([128, 1], f32)
        nc.vector.memset(epst, EPS)
        # LN-applied activation tiles with a persistent ones bias row
        xns = [wpool.tile([C + 1, CH], f32r, tag=f"xn{i}", name=f"xn{i}")
               for i in range(2)]
        # f32 scratch (init-only) to produce properly-rounded f32r constants
        initp = tc.alloc_tile_pool(name="initp", bufs=1)
        onesrow_f = initp.tile([1, CH], f32)
        nc.vector.memset(onesrow_f, 1.0)
        nc.vector.tensor_copy(out=onesrow, in_=onesrow_f)
        ones3f = initp.tile([C, C], f32)
        nc.vector.memset(ones3f, 0.0)
        for j in range(3):
            nc.vector.memset(ones3f[32 * j:32 * j + 1, :], 1.0)
        nc.vector.tensor_copy(out=ones3x, in_=ones3f)
        for i in range(2):
            nc.vector.tensor_copy(out=xns[i][C:C + 1, :], in_=onesrow_f)
        initp.release()
        # LN stats rows: rs/nb per chunk on partitions [32, 512]
        # (LN2 pass reuses the same tiles after LN1's readers are done)
        rs1 = wpool.tile([NCHUNK, CH], f32r)
        nb1 = wpool.tile([NCHUNK, CH], f32r)
        rs2, nb2 = rs1, nb1
        # channel-sum / channel-sumsq strips (chunk on partitions)
        ss1 = wpool.tile([NCHUNK, CH], f32)
        sq1 = wpool.tile([NCHUNK, CH], f32)
        ss2, sq2 = ss1, sq1

        # big persistent activation tensors
        apool = tc.alloc_tile_pool(name="acts", bufs=1)
        Qd = [apool.tile([128, BH, W], bf16, tag=f"qd{d}", name=f"qd{d}") for d in range(3)]
        Kp = [apool.tile([128, PADR, PADC], bf16, tag=f"kp{d}", name=f"kp{d}") for d in range(3)]
        Vp = [apool.tile([128, PADR, PADC], bf16, tag=f"vp{d}", name=f"vp{d}") for d in range(3)]


        # ============ shared stats math ============
        # strips ss/sq: [32, CH] SBUF, chunk on partitions.  Runs on row
        # slices [c0:c0+n] right after each flush so downstream chunks can
        # start without waiting for the whole stats pass.  mu overwrites ss,
        # var/sd overwrite sq in place; vtmp holds -mu^2.
        vtmp = wpool.tile([NCHUNK, CH], f32)

        def stats_math_early(ss, sq, rs, nb):
            sl = slice(0, 8)
            mu = vtmp[sl, :]
            nc.scalar.mul(out=mu, in_=ss[sl, :], mul=1.0 / C)
            t = rs[sl, :].bitcast(f32)
            nc.scalar.mul(out=t, in_=sq[sl, :], mul=1.0 / C)
            nc.vector.scalar_tensor_tensor(out=nb[sl, :].bitcast(f32), in0=mu,
                                           scalar=-1.0, in1=mu,
                                           op0=ALU.mult, op1=ALU.mult)
            nc.vector.tensor_tensor(out=t, in0=t, in1=nb[sl, :].bitcast(f32),
                                    op=ALU.add)
            nc.scalar.activation(out=t, in_=t, func=AF.Sqrt,
                                 bias=epst[0:8, 0:1], scale=1.0)
            with nc.allow_low_precision(reason="f32r-typed LN stats rows"):
                nc.vector.reciprocal(out=rs[sl, :], in_=t)
                nc.vector.scalar_tensor_tensor(out=nb[sl, :], in0=mu,
                                               scalar=-1.0,
                                               in1=rs[sl, :].bitcast(f32),
                                               op0=ALU.mult, op1=ALU.mult)

        def stats_math(ss, sq, rs, nb, c0, n):
            sl = slice(c0, c0 + n)
            nc.scalar.mul(out=ss[sl, :], in_=ss[sl, :], mul=1.0 / C)
            nc.scalar.mul(out=sq[sl, :], in_=sq[sl, :], mul=1.0 / C)
            nc.vector.scalar_tensor_tensor(out=vtmp[sl, :], in0=ss[sl, :],
                                           scalar=-1.0, in1=ss[sl, :],
                                           op0=ALU.mult, op1=ALU.mult)
            nc.vector.tensor_tensor(out=sq[sl, :], in0=sq[sl, :],
                                    in1=vtmp[sl, :], op=ALU.add)
            nc.scalar.activation(out=sq[sl, :], in_=sq[sl, :], func=AF.Sqrt,
                                 bias=epst[c0:c0 + n, 0:1], scale=1.0)
            with nc.allow_low_precision(reason="f32r-typed LN stats rows"):
                nc.vector.reciprocal(out=rs[sl, :], in_=sq[sl, :])
                nc.vector.scalar_tensor_tensor(out=nb[sl, :], in0=ss[sl, :],
                                               scalar=-1.0,
                                               in1=rs[sl, :].bitcast(f32),
                                               op0=ALU.mult, op1=ALU.mult)

        # strip helper: drain a [1, n*CH] psum strip (partition 0) to a
        # 1-partition SBUF stage, then one DMA reshapes it into rows
        # [c0:c0+n] of the compact [32, CH] stats tiles.
        def strip_flush(pool, ps_s, ps_q, ss, sq, c0, n, qeng=None):
            stg_s = pool.tile([1, n * CH], f32, tag="stg_s")
            nc.scalar.copy(stg_s, ps_s[:, 0:n * CH])
            stg_q = pool.tile([1, n * CH], f32, tag="stg_q")
            if qeng is None:
                nc.scalar.copy(stg_q, ps_q[:, 0:n * CH])
            else:
                qeng.tensor_copy(out=stg_q, in_=ps_q[:, 0:n * CH])
            nc.sync.dma_start(out=ss[c0:c0 + n, :], in_=stg_s)
            nc.sync.dma_start(out=sq[c0:c0 + n, :], in_=stg_q)

        # ============ PH1: LN1 stats sweep ============
        with tc.tile_pool(name="ph1", bufs=3) as pool, \
             tc.tile_pool(name="ph1st", bufs=2) as sgpool, \
             tc.tile_pool(name="ph1ps", bufs=2, space="PSUM") as stps:
            ps_s = ps_q = None
            for g in range(NCHUNK // 4):
                xt4 = pool.tile([C, 4, CH], f32r, tag="xt")
                nc.sync.dma_start(out=xt4,
                                  in_=x_d[:, 16 * g:16 * g + 16, :].bitcast(f32r))
                for i in range(4):
                    c = 4 * g + i
                    if i % 2 == 0:
                        ps_s = stps.tile([1, 2 * CH], f32, tag="ps_s")
                        ps_q = stps.tile([1, 2 * CH], f32, tag="ps_q")
                    h = CH * (i % 2)
                    xt = xt4[:, i, :]
                    nc.tensor.matmul(ps_s[:, h:h + CH], lhsT=onescol,
                                     rhs=xt, start=True, stop=True)
                    xf = xt.bitcast(f32)
                    xsq = pool.tile([C, CH], f32r, tag="xsq")
                    nc.vector.tensor_tensor(out=xsq, in0=xf, in1=xf, op=ALU.mult)
                    nc.tensor.matmul(ps_q[:, h:h + CH], lhsT=onescol,
                                     rhs=xsq, start=True, stop=True)
                    if i % 2 == 1:
                        strip_flush(sgpool, ps_s, ps_q, ss1, sq1, c - 1, 2,
                                    qeng=nc.vector)
                    if i == 3:
                        if g == 7:
                            stats_math(ss1, sq1, rs1, nb1, 0, NCHUNK)

        # zero only the pad strips (interior fully overwritten by repack)
        for d in range(3):
            for t in (Kp[d], Vp[d]):
                nc.gpsimd.memset(t[:, 0:3, :], 0.0)
                nc.gpsimd.memset(t[:, 35:38, :], 0.0)
                nc.gpsimd.memset(t[:, 3:35, 0:3], 0.0)
                nc.gpsimd.memset(t[:, 3:35, 3 + W:PADC], 0.0)

        stgpool = tc.alloc_tile_pool(name="stg", bufs=1)
        stg_k = stgpool.tile([C, N], bf16)
        stg_v = stgpool.tile([C, N], bf16)


        # ============ PH2: LN1 apply + qkv + stage/scatter ============
        def band_rows(b):
            lo = max(0, BH * b - 3)
            hi = min(H, BH * b + BH + 3)
            return lo, hi

        # stage rs/nb rows at partitions {0,32,64} so rank-1 matmuls can
        # read them (PE base-partition rule); one strided DMA per 3 chunks
        def stage_stats(sgp, rs, nb, c0):
            n = min(3, NCHUNK - c0)
            srs = sgp.tile([C, CH], f32r, tag="srs")
            snb = sgp.tile([C, CH], f32r, tag="snb")
            dst_rs = srs.rearrange("(a b) f -> a b f", a=3)[0:n, 0:1, :]
            dst_nb = snb.rearrange("(a b) f -> a b f", a=3)[0:n, 0:1, :]
            nc.sync.dma_start(out=dst_rs, in_=rs[c0:c0 + n, :])
            nc.sync.dma_start(out=dst_nb, in_=nb[c0:c0 + n, :])
            return srs, snb

        with tc.tile_pool(name="ph2", bufs=3) as pool, \
             tc.tile_pool(name="ph2t", bufs=2) as tpool, \
             tc.tile_pool(name="ph2sg", bufs=2) as sgp, \
             tc.tile_pool(name="ph2ps", bufs=2, space="PSUM") as psum, \
             tc.tile_pool(name="ph2ps2", bufs=2, space="PSUM") as psum2:
            srs = snb = None
            for c in range(NCHUNK):
                g, i = c // 4, c % 4
                if i == 0:
                    xt4 = pool.tile([C, 4, CH], f32, tag="xt2")
                    nc.sync.dma_start(out=xt4, in_=x_d[:, 16 * g:16 * g + 16, :])
                if c % 3 == 0:
                    srs, snb = stage_stats(sgp, rs1, nb1, c)
                j = c % 3
                xt = xt4[:, i, :]
                pa = psum2.tile([C, CH], f32, tag="pa")
                nc.tensor.matmul(pa, lhsT=ones3x[32 * j:32 * j + 1, :],
                                 rhs=srs[32 * j:32 * j + 1, :],
                                 start=True, stop=True)
                xn = xns[c % 2]
                nc.vector.tensor_tensor(out=xn[0:C, :], in0=xt, in1=pa,
                                        op=ALU.mult)

                pq = psum.tile([C, CH], f32, tag="pq")
                pk = psum.tile([C, CH], f32, tag="pk")
                pv = psum.tile([C, CH], f32, tag="pv")
                snbj = snb[32 * j:32 * j + 1, :]
                for t, pt in enumerate((pq, pk, pv)):
                    nc.tensor.matmul(pt, lhsT=wsum3[32 * j:32 * j + 1,
                                                    C * t:C * (t + 1)],
                                     rhs=snbj, start=True, stop=False)
                    nc.tensor.matmul(pt, lhsT=wq[:, C * t:C * (t + 1)], rhs=xn,
                                     start=False, stop=True)

                # K/V -> contiguous staging (single 96-partition copies);
                # gpsimd cannot touch PSUM, so drains go DVE/Act only
                nc.scalar.copy(stg_k[:, CH * c:CH * (c + 1)], pk)
                nc.vector.tensor_copy(out=stg_v[:, CH * c:CH * (c + 1)], in_=pv)
                # Q -> band-packed planes directly
                b = c // 8
                r_off = 4 * c - BH * b
                for d in range(3):
                    src = pq[32 * d:32 * d + 32, :].rearrange("p (r w) -> p r w", r=4)
                    dst = Qd[d][32 * b:32 * b + 32, r_off:r_off + 4, :]
                    if d == 0:
                        nc.vector.tensor_copy(out=dst, in_=src)
                    else:
                        nc.scalar.copy(dst, src)

                # band-packed K/V repack via SBUF->SBUF DMA on idle rings
                if c in (8, 16, 24, 31):
                    b_ = (c - 1) // 8
                    lo, hi = band_rows(b_)
                    nr = hi - lo
                    r0 = lo - (BH * b_ - 3)
                    for d in range(3):
                        for stg, dstp in ((stg_k, Kp[d]), (stg_v, Vp[d])):
                            nc.sync.dma_start(
                                out=dstp[32 * b_:32 * b_ + 32, r0:r0 + nr, 3:3 + W],
                                in_=stg[32 * d:32 * d + 32, W * lo:W * hi]
                                    .rearrange("p (r w) -> p r w", r=nr))

        stgpool.release()

        # attention output (channel-major), in space freed by the staging
        ofpool = tc.alloc_tile_pool(name="ofp", bufs=1)
        ofull = ofpool.tile([C, N], bf16)

        # ============ PH3: attention per dilation ============
        # Processed in half-planes (16 band-rows each) so the softmax
        # denominator S accumulates across taps in PSUM via PE matmuls with
        # srepl (= repl/16), freeing DVE of the S adds entirely.
        HF = BH * W // 2                                # 2048 tokens per half
        with tc.tile_pool(name="ph3", bufs=4) as pool, \
             tc.tile_pool(name="ph3f", bufs=4) as fpool, \
             tc.tile_pool(name="ph3acc", bufs=3) as acc, \
             tc.tile_pool(name="ph3ps", bufs=2, space="PSUM") as psum, \
             tc.tile_pool(name="ph3sps", bufs=1, space="PSUM") as spsum:
            taps = [(i - 1, j - 1) for i in range(3) for j in range(3)]
            halves = [(di, hh) for di in range(3) for hh in range(2)]

            def make_emit_p(di, hh):
                dil = DILS[di]
                qv_ = Qd[di][:, 16 * hh:16 * hh + 16, :]

                def emit_p(ti):
                    dr, dc = taps[ti]
                    r0 = 3 + dr * dil + 16 * hh
                    kwin = Kp[di][:, r0:r0 + 16,
                                  3 + dc * dil:3 + dc * dil + W]
                    P = fpool.tile([128, HF], bf16, tag="P",
                                   name=f"P_{di}_{hh}_{ti}")
                    nc.vector.tensor_tensor(
                        out=P.rearrange("p (r w) -> p r w", r=16),
                        in0=qv_, in1=kwin, op=ALU.mult)
                    return P
                return emit_p

            emitters = [make_emit_p(di, hh) for di, hh in halves]
            Pcarry = emitters[0](0)
            for k, (di, hh) in enumerate(halves):
                dil = DILS[di]
                emit_p = emitters[k]
                if True:
                    S_ps = spsum.tile([128, HF], f32, tag="Sps")
                    Oab = [acc.tile([128, HF], bf16, tag=f"O{p}",
                                    name=f"O{p}_{di}_{hh}") for p in range(2)]
                    rcp = acc.tile([128, HF], bf16, tag="rcp",
                                   name=f"rcp_{di}_{hh}")
                    qv = Qd[di][:, 16 * hh:16 * hh + 16, :]
                    Pnext = Pcarry
                    for ti, (dr, dc) in enumerate(taps):
                        r0 = 3 + dr * dil + 16 * hh
                        vwin = Vp[di][:, r0:r0 + 16, 3 + dc * dil:3 + dc * dil + W]
                        P = Pnext
                        # logits -> exp (overwrites P) -> S accumulation
                        for q in range(2):
                            pl = psum.tile([128, 1024], f32, tag="pl")
                            for j in range(2):
                                nc.tensor.matmul(
                                    pl[:, 512 * j:512 * (j + 1)], lhsT=repl,
                                    rhs=P[:, 1024 * q + 512 * j:
                                          1024 * q + 512 * (j + 1)],
                                    start=True, stop=True)
                            nc.scalar.activation(
                                out=P[:, 1024 * q:1024 * (q + 1)], in_=pl,
                                func=AF.Exp)
                            for j in range(2):
                                nc.tensor.matmul(
                                    S_ps[:, 1024 * q + 512 * j:
                                         1024 * q + 512 * (j + 1)],
                                    lhsT=srepl,
                                    rhs=P[:, 1024 * q + 512 * j:
                                          1024 * q + 512 * (j + 1)],
                                    start=(ti == 0), stop=(ti == 8))
                        # next tap's P-mult ahead of Pv in the DVE queue;
                        # at the last tap, prefetch the next half's first P
                        if ti < 8:
                            Pnext = emit_p(ti + 1)
                        elif k + 1 < len(halves):
                            Pcarry = emitters[k + 1](0)
                        ev = P.rearrange("p (r w) -> p r w", r=16)
                        Pv = Oab[ti] if ti < 2 else pool.tile([128, HF], bf16,
                                                              tag="Pv")
                        nc.vector.tensor_tensor(
                            out=Pv.rearrange("p (r w) -> p r w", r=16),
                            in0=ev, in1=vwin, op=ALU.mult)
                        if ti >= 2:
                            eng = nc.gpsimd if ti in (3, 4, 7) else nc.vector
                            eng.tensor_tensor(out=Oab[ti % 2], in0=Oab[ti % 2],
                                              in1=Pv, op=ALU.add)
                    with nc.allow_low_precision(reason="softmax recip bf16"):
                        nc.vector.reciprocal(out=rcp, in_=S_ps)
                    nc.vector.tensor_tensor(out=Oab[0], in0=Oab[0], in1=Oab[1],
                                            op=ALU.add)
                    nc.vector.tensor_tensor(out=Oab[0], in0=Oab[0], in1=rcp,
                                            op=ALU.mult)
                    for b in range(NB):
                        nc.sync.dma_start(
                            out=ofull[32 * di:32 * di + 32,
                                      4096 * b + 2048 * hh:
                                      4096 * b + 2048 * hh + 2048],
                            in_=Oab[0][32 * b:32 * b + 32, :])

        # ============ PH4: proj + residual + LN2 stats ============
        r1pool = tc.alloc_tile_pool(name="r1p", bufs=1)
        r1 = r1pool.tile([C, N], bf16)
        with tc.tile_pool(name="ph4", bufs=2) as pool, \
             tc.tile_pool(name="ph4sg", bufs=1) as sgpool, \
             tc.tile_pool(name="ph4st", bufs=1, space="PSUM") as stps, \
             tc.tile_pool(name="ph4ps", bufs=4, space="PSUM") as psum:
            ps_s = ps_q = None
            for c in range(NCHUNK):
                g, i = c // 4, c % 4
                if i == 0:
                    xt4 = pool.tile([C, 4, CH], f32, tag="xt4")
                    nc.sync.dma_start(out=xt4, in_=x_d[:, 16 * g:16 * g + 16, :])
                if c % 2 == 0:
                    ps_s = stps.tile([1, 2 * CH], f32, tag="ps_s4")
                    ps_q = stps.tile([1, 2 * CH], f32, tag="ps_q4")
                h4 = CH * (c % 2)
                pp = psum.tile([C, CH], f32, tag="pp")
                nc.tensor.matmul(pp, lhsT=pbt, rhs=onesrow, start=True, stop=False)
                nc.tensor.matmul(pp, lhsT=wp, rhs=ofull[:, CH * c:CH * (c + 1)],
                                 start=False, stop=True)
                rsl = r1[:, CH * c:CH * (c + 1)]
                nc.vector.tensor_tensor(out=rsl, in0=xt4[:, i, :],
                                        in1=pp, op=ALU.add)
                nc.tensor.matmul(ps_s[:, h4:h4 + CH], lhsT=onescol_b,
                                 rhs=rsl, start=True, stop=True)
                xsq = pool.tile([C, CH], f32r, tag="xsq5")
                nc.vector.tensor_tensor(out=xsq, in0=rsl, in1=rsl, op=ALU.mult)
                nc.tensor.matmul(ps_q[:, h4:h4 + CH], lhsT=onescol,
                                 rhs=xsq, start=True, stop=True)
                if c % 2 == 1:
                    strip_flush(sgpool, ps_s, ps_q, ss2, sq2, c - 1, 2)
                    if c == 31:
                        stats_math(ss2, sq2, rs2, nb2, 0, NCHUNK)

        # ============ PH5: MLP + residual ============
        with tc.tile_pool(name="ph5b", bufs=3) as pool, \
             tc.tile_pool(name="ph5h", bufs=2) as hpool, \
             tc.tile_pool(name="ph5y", bufs=2) as ypool, \
             tc.tile_pool(name="ph5sg", bufs=2) as sgp, \
             tc.tile_pool(name="ph5ps", bufs=3, space="PSUM") as psum, \
             tc.tile_pool(name="ph5ps2", bufs=1, space="PSUM") as psum2:
            srs = snb = None
            for c in range(NCHUNK):
                g, i = c // 4, c % 4
                rsl = r1[:, CH * c:CH * (c + 1)]
                if i == 0:
                    yout4 = ypool.tile([C, 4, CH], f32, tag="yout4")
                if c % 3 == 0:
                    srs, snb = stage_stats(sgp, rs2, nb2, c)
                j = c % 3
                pa = psum2.tile([C, CH], f32, tag="pa5")
                nc.tensor.matmul(pa, lhsT=ones3x[32 * j:32 * j + 1, :],
                                 rhs=srs[32 * j:32 * j + 1, :],
                                 start=True, stop=True)
                pb2 = psum2.tile([C, CH], f32, tag="pb5")
                nc.tensor.matmul(pb2, lhsT=ones3x[32 * j:32 * j + 1, :],
                                 rhs=snb[32 * j:32 * j + 1, :],
                                 start=True, stop=True)
                t1 = pool.tile([C, CH], f32, tag="t15")
                nc.vector.tensor_tensor(out=t1, in0=rsl, in1=pa, op=ALU.mult)
                xn = xns[c % 2]
                nc.vector.tensor_tensor(out=xn[0:C, :], in0=t1,
                                        in1=pb2, op=ALU.add)

                h1 = hpool.tile([128, 3, CH], bf16, tag="h1")
                for j in range(3):
                    pf = psum.tile([128, CH], f32, tag="pf")
                    nc.tensor.matmul(pf, lhsT=w1[:, 128 * j:128 * (j + 1)], rhs=xn,
                                     start=True, stop=True)
                    nc.scalar.activation(out=h1[:, j, :], in_=pf, func=AF.Gelu)
                pm = psum.tile([C, CH], f32, tag="pm")
                nc.tensor.matmul(pm, lhsT=b2t, rhs=onesrow, start=True, stop=False)
                for j in range(3):
                    nc.tensor.matmul(pm, lhsT=w2[j],
                                     rhs=h1[:, j, :], start=False, stop=(j == 2))
                nc.vector.tensor_tensor(out=yout4[:, i, :], in0=rsl,
                                        in1=pm, op=ALU.add)
                if i == 3:
                    nc.sync.dma_start(out=y_d[:, 16 * g:16 * g + 16, :], in_=yout4)

        r1pool.release()
        ofpool.release()
        apool.release()
        wpool.release()

    _split_multi_waits(nc, mybir)
    return nc


def _prep_weights(inputs):
    """Host-side weight preparation (fold LN affine, scale, bias rows)."""
    qkv_w = np.asarray(inputs['qkv_w'], np.float32)       # (288, 96)
    proj_w = np.asarray(inputs['proj_w'], np.float32)     # (96, 96)
    proj_b = np.asarray(inputs['proj_b'], np.float32)
    ln1_w = np.asarray(inputs['ln1_w'], np.float32)
    ln1_b = np.asarray(inputs['ln1_b'], np.float32)
    ln2_w = np.asarray(inputs['ln2_w'], np.float32)
    ln2_b = np.asarray(inputs['ln2_b'], np.float32)
    fc1_w = np.asarray(inputs['fc1_w'], np.float32)       # (384, 96)
    fc1_b = np.asarray(inputs['fc1_b'], np.float32)
    fc2_w = np.asarray(inputs['fc2_w'], np.float32)       # (96, 384)
    fc2_b = np.asarray(inputs['fc2_b'], np.float32)

    wq = qkv_w * ln1_w[None, :]                            # (288, 96)
    c0 = qkv_w @ ln1_b                                     # (288,)
    wq[0:C] *= SCALE                                       # scale q rows
    c0[0:C] *= SCALE
    wqb = np.concatenate([wq.T, c0[None, :]], axis=0)      # (97, 288)

    w1 = fc1_w * ln2_w[None, :]
    c1 = fc1_w @ ln2_b + fc1_b
    w1b = np.concatenate([w1.T, c1[None, :]], axis=0)      # (97, 384)

    repl = np.zeros((128, 128), np.float32)
    for b in range(NB):
        for ch in range(GD):
            h0 = (ch // HD) * HD
            repl[32 * b + h0:32 * b + h0 + HD, 32 * b + ch] = 1.0

    wsum3 = np.zeros((C, 3 * C), np.float32)
    for j in range(3):
        wsum3[32 * j, :] = wq.T[0:C, :].sum(axis=0)        # per-column sums

    return {
        'wsum3': wsum3,
        'wqkv': np.ascontiguousarray(wqb),                 # (97, 288) lhsT
        'wproj': np.ascontiguousarray(proj_w.T),           # (96, 96) lhsT
        'projb': proj_b.reshape(-1, 1).astype(np.float32),
        'w1': np.ascontiguousarray(w1b),                   # (97, 384) lhsT
        'w2': np.ascontiguousarray(fc2_w.T),               # (384, 96) lhsT
        'b2': fc2_b.reshape(-1, 1).astype(np.float32),
        'repl': repl,
        'onesc': np.ones((C, 1), np.float32),
    }


def kernel(**inputs):
    from concourse.bass_utils import run_bass_kernel_spmd

    if 'nc' not in _cache:
        t0 = time.time()
        _cache['nc'] = _build()
        print(f"[kernel] built bass module in {time.time() - t0:.1f}s",
              file=sys.stderr)

    nc = _cache['nc']
    wmap = _prep_weights(inputs)
    x = np.asarray(inputs['x'], np.float32)                # (8, 96, 128, 128)

    in_maps = []
    for b in range(B):
        m = {'x': np.ascontiguousarray(x[b])}
        m.update(wmap)
        in_maps.append(m)

    res = run_bass_kernel_spmd(nc, in_maps, core_ids=list(range(B)))
    _cache['last_exec_ns'] = res.exec_time_ns
    out = np.stack([res.results[b]['y'] for b in range(B)], axis=0)
    return out.astype(np.float32)


# revision 99
# speedup vs baseline: 1.0888x; 1.0090x over previous
"""DilateBlock kernel for 8x Trainium2 NeuronCores (Bass/Tile).

Data-parallel over batch B=8 (one image per core). Per core, the whole block
(LN1 -> qkv -> 3-dilation 3x3 neighborhood attention -> proj -> residual ->
LN2 -> MLP -> residual) runs in channels-on-partitions layout; spatial shifts
for the attention unfold live on the free dimension of zero-padded (h, w)
planes, packed 4-hbands x 32-channels across partitions.

Key tricks:
  - LayerNorm stats via ones-matmul on PE into a [32, 512] PSUM strip tile
    (chunk index on partitions), stats math runs wide on 32 partitions, and
    the per-token scale/shift rows feed rank-1 PSUM matmuls directly.
  - qkv/fc1 biases folded into the matmuls via a 97th ones-row of the
    LN-applied activations (contract-97 lhsT with a bias row).
  - K/V drained to contiguous staging as single 96-partition copies, then
    band-packed into padded planes by SBUF->SBUF DMAs on the idle DMA rings.
  - QK tap logits reduced over head_dim AND replicated back to all 16
    channel rows in one PE matmul with a static block-ones matrix; exp runs
    full width on Act; softmax denominator and output accumulate across taps
    via gpsimd DMA-accumulate (even/odd partial tiles, merged on DVE).
  - Softmax normalization applied to the attention OUTPUT.
  - Attention output repacked in SBUF (no DRAM roundtrip) for the proj.
"""
import sys
import time

sys.path.insert(0, '/opt/trn_rl_repo')

import numpy as np

# ---- problem constants (hardcoded per contract) ----
B, C, H, W = 8, 96, 128, 128
DILS = (1, 2, 3)
GD = 32                 # channels per dilation branch
HD = 16                 # head dim
NB = 4                  # h-bands packed on partitions
BH = H // NB            # rows per band = 32
N = H * W               # tokens per image
NCHUNK = 32             # token chunks of 512 (4 image rows each)
CH = N // NCHUNK        # 512
PADR = 38               # BH + 6 halo rows
PADC = 136              # W + 8 halo cols (EVEN pitch: enables DVE 2x mode)
EPS = 1e-5
SCALE = HD ** -0.5
MLPH = 384

_cache = {}
import os
_USE_DMA_ACCUM = os.environ.get('KDMA', '0') == '1'


def _patch_tile(tile_mod, bass_mod):
    """Work around this walrus build's 1-sem-wait-per-instruction limit and
    the multi-wait tail drain."""
    from concourse.vector_clock import ScopedClock, VectorClock

    def _drain_and_barrier(self, tick_clock, wait_clock):
        vclock = tick_clock.global_clock
        n = len(vclock)
        idxs = [i for i in range(n) if vclock[i] > 0]
        for i in idxs:
            vec = [0] * n
            vec[i] = vclock[i]
            nop_inst = self.nc.sync.nop(nofuse=True)
            wait_clock.add_sem_waits(nop_inst.ins,
                                     ScopedClock({None: VectorClock(vec)}))
        self.nc.sync.drain()
        self.nc.all_engine_barrier()
        popped = self.nc._tile_sem_poison_stack.pop()
        assert popped is self._sem_poison
        self.nc.clear_and_free_semaphores(list(self.sems.allocated().values()))
        self.nc.all_engine_barrier()

    tile_mod.TileContext._drain_and_barrier = _drain_and_barrier


_ws_counter = [0]


def _split_multi_waits(nc, mybir):
    for fn in nc.m.functions:
        for blk in fn.blocks:
            insts = list(blk.instructions)
            out = []
            changed = False
            for inst in insts:
                si = inst.sync_info
                waits = list(si.on_wait) if si and si.on_wait else []
                if len(waits) > 1:
                    for w in waits[:-1]:
                        _ws_counter[0] += 1
                        out.append(mybir.InstNoOp(
                            name=f"I-ws-{_ws_counter[0]}",
                            engine=inst.engine, ins=[], outs=[],
                            sync_info=mybir.SyncInfo(on_wait=[w], on_update=[])))
                    si.on_wait = [waits[-1]]
                    changed = True
                out.append(inst)
            if changed:
                blk.instructions[:] = out


def _build():
    import concourse.bass as bass
    import concourse.tile as tile
    from concourse import mybir

    _patch_tile(tile, bass)

    f32 = mybir.dt.float32
    f32r = mybir.dt.float32r
    bf16 = mybir.dt.bfloat16
    AF = mybir.ActivationFunctionType
    ALU = mybir.AluOpType

    nc = bass.Bass()

    # ---- DRAM I/O ----
    x_d = nc.dram_tensor("x", (C, H, W), f32, kind="ExternalInput")
    wq_d = nc.dram_tensor("wqkv", (C + 1, 3 * C), f32, kind="ExternalInput")  # lhsT+bias
    wp_d = nc.dram_tensor("wproj", (C, C), f32, kind="ExternalInput")         # lhsT
    pb_d = nc.dram_tensor("projb", (C, 1), f32, kind="ExternalInput")
    w1_d = nc.dram_tensor("w1", (C + 1, MLPH), f32, kind="ExternalInput")     # lhsT+bias
    w2_d = nc.dram_tensor("w2", (MLPH, C), f32, kind="ExternalInput")         # lhsT
    b2_d = nc.dram_tensor("b2", (C, 1), f32, kind="ExternalInput")
    repl_d = nc.dram_tensor("repl", (128, 128), f32, kind="ExternalInput")
    ws_d = nc.dram_tensor("wsum3", (C, 3 * C), f32, kind="ExternalInput")
    ones_d = nc.dram_tensor("onesc", (C, 1), f32, kind="ExternalInput")

    y_d = nc.dram_tensor("y", (C, H, W), f32, kind="ExternalOutput")

    with tile.TileContext(nc) as tc:
        # ---------------- persistent pools ----------------
        wpool = tc.alloc_tile_pool(name="weights", bufs=1)
        wq = wpool.tile([C + 1, 3 * C], f32r)
        nc.sync.dma_start(out=wq, in_=wq_d[:, :].bitcast(f32r))
        wp = wpool.tile([C, C], bf16)
        nc.gpsimd.dma_start(out=wp, in_=wp_d[:, :])     # gpsimd dma casts
        pbt = wpool.tile([1, C], f32r)                  # proj bias as rank-1 lhsT
        nc.sync.dma_start(out=pbt, in_=pb_d[:, :].rearrange("a b -> b a").bitcast(f32r))
        w1 = wpool.tile([C + 1, MLPH], f32r)
        nc.sync.dma_start(out=w1, in_=w1_d[:, :].bitcast(f32r))
        w2 = [wpool.tile([128, C], bf16, tag=f"w2{i}", name=f"w2{i}") for i in range(3)]
        for i in range(3):
            nc.gpsimd.dma_start(out=w2[i], in_=w2_d[128 * i:128 * (i + 1), :])
        b2t = wpool.tile([1, C], f32r)                  # fc2 bias as rank-1 lhsT
        nc.sync.dma_start(out=b2t, in_=b2_d[:, :].rearrange("a b -> b a").bitcast(f32r))
        repl = wpool.tile([128, 128], bf16)
        nc.gpsimd.dma_start(out=repl, in_=repl_d[:, :])
        srepl = wpool.tile([128, 128], bf16)            # repl/16: S accumulation
        nc.scalar.mul(out=srepl, in_=repl, mul=1.0 / HD)
        onescol = wpool.tile([C, 1], f32r)              # stats lhsT [96,1]
        nc.sync.dma_start(out=onescol, in_=ones_d[:, :].bitcast(f32r))
        onescol_b = wpool.tile([C, 1], bf16)            # bf16 variant (bf16 rhs)
        nc.vector.memset(onescol_b, 1.0)
        wsum3 = wpool.tile([C, 3 * C], f32r)            # qkv col-sums at {0,32,64}
        nc.sync.dma_start(out=wsum3, in_=ws_d[:, :].bitcast(f32r))
        ones1x = wpool.tile([1, C], f32r)               # rank-1 lhsT [1,96]
        nc.sync.dma_start(out=ones1x, in_=ones_d[:, :].rearrange("a b -> b a").bitcast(f32r))
        onesrow = wpool.tile([1, CH], f32r)             # static ones row (f32r)
        ones3x = wpool.tile([C, C], f32r)               # ones rows at {0,32,64}
        epst = wpool.tile([128, 1], f32)
        nc.vector.memset(epst, EPS)
        # LN-applied activation tiles with a persistent ones bias row
        xns = [wpool.tile([C + 1, CH], f32r, tag=f"xn{i}", name=f"xn{i}")
               for i in range(2)]
        # f32 scratch (init-only) to produce properly-rounded f32r constants
        initp = tc.alloc_tile_pool(name="initp", bufs=1)
        onesrow_f = initp.tile([1, CH], f32)
        nc.vector.memset(onesrow_f, 1.0)
        nc.vector.tensor_copy(out=onesrow, in_=onesrow_f)
        ones3f = initp.tile([C, C], f32)
        nc.vector.memset(ones3f, 0.0)
        for j in range(3):
            nc.vector.memset(ones3f[32 * j:32 * j + 1, :], 1.0)
        nc.vector.tensor_copy(out=ones3x, in_=ones3f)
        for i in range(2):
            nc.vector.tensor_copy(out=xns[i][C:C + 1, :], in_=onesrow_f)
        initp.release()
        # LN stats rows: rs/nb per chunk on partitions [32, 512]
        # (LN2 pass reuses the same tiles after LN1's readers are done)
        rs1 = wpool.tile([NCHUNK, CH], f32r)
        nb1 = wpool.tile([NCHUNK, CH], f32r)
        rs2, nb2 = rs1, nb1
        # channel-sum / channel-sumsq strips (chunk on partitions)
        ss1 = wpool.tile([NCHUNK, CH], f32)
        sq1 = wpool.tile([NCHUNK, CH], f32)
        ss2, sq2 = ss1, sq1

        # big persistent activation tensors
        apool = tc.alloc_tile_pool(name="acts", bufs=1)
        Qd = [apool.tile([128, BH, W], bf16, tag=f"qd{d}", name=f"qd{d}") for d in range(3)]
        Kp = [apool.tile([128, PADR, PADC], bf16, tag=f"kp{d}", name=f"kp{d}") for d in range(3)]
        Vp = [apool.tile([128, PADR, PADC], bf16, tag=f"vp{d}", name=f"vp{d}") for d in range(3)]


        # ============ shared stats math ============
        # strips ss/sq: [32, CH] SBUF, chunk on partitions.  Runs on row
        # slices [c0:c0+n] right after each flush so downstream chunks can
        # start without waiting for the whole stats pass.  mu overwrites ss,
        # var/sd overwrite sq in place; vtmp holds -mu^2.
        vtmp = wpool.tile([NCHUNK, CH], f32)

        def stats_math_early(ss, sq, rs, nb):
            sl = slice(0, 8)
            mu = vtmp[sl, :]
            nc.scalar.mul(out=mu, in_=ss[sl, :], mul=1.0 / C)
            t = rs[sl, :].bitcast(f32)
            nc.scalar.mul(out=t, in_=sq[sl, :], mul=1.0 / C)
            nc.vector.scalar_tensor_tensor(out=nb[sl, :].bitcast(f32), in0=mu,
                                           scalar=-1.0, in1=mu,
                                           op0=ALU.mult, op1=ALU.mult)
            nc.vector.tensor_tensor(out=t, in0=t, in1=nb[sl, :].bitcast(f32),
                                    op=ALU.add)
            nc.scalar.activation(out=t, in_=t, func=AF.Sqrt,
                                 bias=epst[0:8, 0:1], scale=1.0)
            with nc.allow_low_precision(reason="f32r-typed LN stats rows"):
                nc.vector.reciprocal(out=rs[sl, :], in_=t)
                nc.vector.scalar_tensor_tensor(out=nb[sl, :], in0=mu,
                                               scalar=-1.0,
                                               in1=rs[sl, :].bitcast(f32),
                                               op0=ALU.mult, op1=ALU.mult)

        def stats_math(ss, sq, rs, nb, c0, n):
            sl = slice(c0, c0 + n)
            nc.scalar.mul(out=ss[sl, :], in_=ss[sl, :], mul=1.0 / C)
            nc.scalar.mul(out=sq[sl, :], in_=sq[sl, :], mul=1.0 / C)
            nc.vector.scalar_tensor_tensor(out=vtmp[sl, :], in0=ss[sl, :],
                                           scalar=-1.0, in1=ss[sl, :],
                                           op0=ALU.mult, op1=ALU.mult)
            nc.vector.tensor_tensor(out=sq[sl, :], in0=sq[sl, :],
                                    in1=vtmp[sl, :], op=ALU.add)
            nc.scalar.activation(out=sq[sl, :], in_=sq[sl, :], func=AF.Sqrt,
                                 bias=epst[c0:c0 + n, 0:1], scale=1.0)
            with nc.allow_low_precision(reason="f32r-typed LN stats rows"):
                nc.vector.reciprocal(out=rs[sl, :], in_=sq[sl, :])
                nc.vector.scalar_tensor_tensor(out=nb[sl, :], in0=ss[sl, :],
                                               scalar=-1.0,
                                               in1=rs[sl, :].bitcast(f32),
                                               op0=ALU.mult, op1=ALU.mult)

        # strip helper: drain a [1, n*CH] psum strip (partition 0) to a
        # 1-partition SBUF stage, then one DMA reshapes it into rows
        # [c0:c0+n] of the compact [32, CH] stats tiles.
        def strip_flush(pool, ps_s, ps_q, ss, sq, c0, n, qeng=None):
            stg_s = pool.tile([1, n * CH], f32, tag="stg_s")
            nc.scalar.copy(stg_s, ps_s[:, 0:n * CH])
            stg_q = pool.tile([1, n * CH], f32, tag="stg_q")
            if qeng is None:
                nc.scalar.copy(stg_q, ps_q[:, 0:n * CH])
            else:
                qeng.tensor_copy(out=stg_q, in_=ps_q[:, 0:n * CH])
            nc.sync.dma_start(out=ss[c0:c0 + n, :], in_=stg_s)
            nc.sync.dma_start(out=sq[c0:c0 + n, :], in_=stg_q)

        # ============ PH1: LN1 stats sweep ============
        with tc.tile_pool(name="ph1", bufs=3) as pool, \
             tc.tile_pool(name="ph1st", bufs=2) as sgpool, \
             tc.tile_pool(name="ph1ps", bufs=2, space="PSUM") as stps:
            ps_s = ps_q = None
            for g in range(NCHUNK // 4):
                xt4 = pool.tile([C, 4, CH], f32r, tag="xt")
                nc.sync.dma_start(out=xt4,
                                  in_=x_d[:, 16 * g:16 * g + 16, :].bitcast(f32r))
                for i in range(4):
                    c = 4 * g + i
                    if i % 2 == 0:
                        ps_s = stps.tile([1, 2 * CH], f32, tag="ps_s")
                        ps_q = stps.tile([1, 2 * CH], f32, tag="ps_q")
                    h = CH * (i % 2)
                    xt = xt4[:, i, :]
                    nc.tensor.matmul(ps_s[:, h:h + CH], lhsT=onescol,
                                     rhs=xt, start=True, stop=True)
                    xf = xt.bitcast(f32)
                    xsq = pool.tile([C, CH], f32r, tag="xsq")
                    nc.vector.tensor_tensor(out=xsq, in0=xf, in1=xf, op=ALU.mult)
                    nc.tensor.matmul(ps_q[:, h:h + CH], lhsT=onescol,
                                     rhs=xsq, start=True, stop=True)
                    if i % 2 == 1:
                        strip_flush(sgpool, ps_s, ps_q, ss1, sq1, c - 1, 2,
                                    qeng=nc.vector)
                    if i == 3:
                        if g == 7:
                            stats_math(ss1, sq1, rs1, nb1, 0, NCHUNK)

        # zero only the pad strips (interior fully overwritten by repack)
        for d in range(3):
            for t in (Kp[d], Vp[d]):
                nc.gpsimd.memset(t[:, 0:3, :], 0.0)
                nc.gpsimd.memset(t[:, 35:38, :], 0.0)
                nc.gpsimd.memset(t[:, 3:35, 0:3], 0.0)
                nc.gpsimd.memset(t[:, 3:35, 3 + W:PADC], 0.0)

        stgpool = tc.alloc_tile_pool(name="stg", bufs=1)
        stg_k = stgpool.tile([C, N], bf16)
        stg_v = stgpool.tile([C, N], bf16)


        # ============ PH2: LN1 apply + qkv + stage/scatter ============
        def band_rows(b):
            lo = max(0, BH * b - 3)
            hi = min(H, BH * b + BH + 3)
            return lo, hi

        # stage rs/nb rows at partitions {0,32,64} so rank-1 matmuls can
        # read them (PE base-partition rule); one strided DMA per 3 chunks
        def stage_stats(sgp, rs, nb, c0):
            n = min(3, NCHUNK - c0)
            srs = sgp.tile([C, CH], f32r, tag="srs")
            snb = sgp.tile([C, CH], f32r, tag="snb")
            dst_rs = srs.rearrange("(a b) f -> a b f", a=3)[0:n, 0:1, :]
            dst_nb = snb.rearrange("(a b) f -> a b f", a=3)[0:n, 0:1, :]
            nc.sync.dma_start(out=dst_rs, in_=rs[c0:c0 + n, :])
            nc.sync.dma_start(out=dst_nb, in_=nb[c0:c0 + n, :])
            return srs, snb

        with tc.tile_pool(name="ph2", bufs=3) as pool, \
             tc.tile_pool(name="ph2t", bufs=2) as tpool, \
             tc.tile_pool(name="ph2sg", bufs=2) as sgp, \
             tc.tile_pool(name="ph2ps", bufs=2, space="PSUM") as psum, \
             tc.tile_pool(name="ph2ps2", bufs=2, space="PSUM") as psum2:
            srs = snb = None
            for c in range(NCHUNK):
                g, i = c // 4, c % 4
                if i == 0:
                    xt4 = pool.tile([C, 4, CH], f32, tag="xt2")
                    nc.sync.dma_start(out=xt4, in_=x_d[:, 16 * g:16 * g + 16, :])
                if c % 3 == 0:
                    srs, snb = stage_stats(sgp, rs1, nb1, c)
                j = c % 3
                xt = xt4[:, i, :]
                pa = psum2.tile([C, CH], f32, tag="pa")
                nc.tensor.matmul(pa, lhsT=ones3x[32 * j:32 * j + 1, :],
                                 rhs=srs[32 * j:32 * j + 1, :],
                                 start=True, stop=True)
                xn = xns[c % 2]
                nc.vector.tensor_tensor(out=xn[0:C, :], in0=xt, in1=pa,
                                        op=ALU.mult)

                pq = psum.tile([C, CH], f32, tag="pq")
                pk = psum.tile([C, CH], f32, tag="pk")
                pv = psum.tile([C, CH], f32, tag="pv")
                snbj = snb[32 * j:32 * j + 1, :]
                for t, pt in enumerate((pq, pk, pv)):
                    nc.tensor.matmul(pt, lhsT=wsum3[32 * j:32 * j + 1,
                                                    C * t:C * (t + 1)],
                                     rhs=snbj, start=True, stop=False)
                    nc.tensor.matmul(pt, lhsT=wq[:, C * t:C * (t + 1)], rhs=xn,
                                     start=False, stop=True)

                # K/V -> contiguous staging (single 96-partition copies);
                # gpsimd cannot touch PSUM, so drains go DVE/Act only
                nc.scalar.copy(stg_k[:, CH * c:CH * (c + 1)], pk)
                nc.vector.tensor_copy(out=stg_v[:, CH * c:CH * (c + 1)], in_=pv)
                # Q -> band-packed planes directly
                b = c // 8
                r_off = 4 * c - BH * b
                for d in range(3):
                    src = pq[32 * d:32 * d + 32, :].rearrange("p (r w) -> p r w", r=4)
                    dst = Qd[d][32 * b:32 * b + 32, r_off:r_off + 4, :]
                    if d == 0:
                        nc.vector.tensor_copy(out=dst, in_=src)
                    else:
                        nc.scalar.copy(dst, src)

                # band-packed K/V repack via SBUF->SBUF DMA on idle rings
                if c in (8, 16, 24, 31):
                    b_ = (c - 1) // 8
                    lo, hi = band_rows(b_)
                    nr = hi - lo
                    r0 = lo - (BH * b_ - 3)
                    for d in range(3):
                        for stg, dstp in ((stg_k, Kp[d]), (stg_v, Vp[d])):
                            nc.sync.dma_start(
                                out=dstp[32 * b_:32 * b_ + 32, r0:r0 + nr, 3:3 + W],
                                in_=stg[32 * d:32 * d + 32, W * lo:W * hi]
                                    .rearrange("p (r w) -> p r w", r=nr))

        stgpool.release()

        # attention output (channel-major), in space freed by the staging
        ofpool = tc.alloc_tile_pool(name="ofp", bufs=1)
        ofull = ofpool.tile([C, N], bf16)

        # ============ PH3: attention per dilation ============
        # Processed in half-planes (16 band-rows each) so the softmax
        # denominator S accumulates across taps in PSUM via PE matmuls with
        # srepl (= repl/16), freeing DVE of the S adds entirely.
        HF = BH * W // 2                                # 2048 tokens per half
        with tc.tile_pool(name="ph3", bufs=4) as pool, \
             tc.tile_pool(name="ph3f", bufs=4) as fpool, \
             tc.tile_pool(name="ph3acc", bufs=3) as acc, \
             tc.tile_pool(name="ph3ps", bufs=2, space="PSUM") as psum, \
             tc.tile_pool(name="ph3sps", bufs=1, space="PSUM") as spsum:
            taps = [(i - 1, j - 1) for i in range(3) for j in range(3)]
            halves = [(di, hh) for di in range(3) for hh in range(2)]

            def make_emit_p(di, hh):
                dil = DILS[di]
                qv_ = Qd[di][:, 16 * hh:16 * hh + 16, :]

                def emit_p(ti):
                    dr, dc = taps[ti]
                    r0 = 3 + dr * dil + 16 * hh
                    kwin = Kp[di][:, r0:r0 + 16,
                                  3 + dc * dil:3 + dc * dil + W]
                    P = fpool.tile([128, HF], bf16, tag="P",
                                   name=f"P_{di}_{hh}_{ti}")
                    nc.vector.tensor_tensor(
                        out=P.rearrange("p (r w) -> p r w", r=16),
                        in0=qv_, in1=kwin, op=ALU.mult)
                    return P
                return emit_p

            emitters = [make_emit_p(di, hh) for di, hh in halves]
            Pcarry = emitters[0](0)
            for k, (di, hh) in enumerate(halves):
                dil = DILS[di]
                emit_p = emitters[k]
                if True:
                    S_ps = spsum.tile([128, HF], f32, tag="Sps")
                    Oab = [acc.tile([128, HF], bf16, tag=f"O{p}",
                                    name=f"O{p}_{di}_{hh}") for p in range(2)]
                    rcp = acc.tile([128, HF], bf16, tag="rcp",
                                   name=f"rcp_{di}_{hh}")
                    qv = Qd[di][:, 16 * hh:16 * hh + 16, :]
                    Pnext = Pcarry
                    for ti, (dr, dc) in enumerate(taps):
                        r0 = 3 + dr * dil + 16 * hh
                        vwin = Vp[di][:, r0:r0 + 16, 3 + dc * dil:3 + dc * dil + W]
                        P = Pnext
                        # logits -> exp (overwrites P) -> S accumulation
                        for q in range(2):
                            pl = psum.tile([128, 1024], f32, tag="pl")
                            for j in range(2):
                                nc.tensor.matmul(
                                    pl[:, 512 * j:512 * (j + 1)], lhsT=repl,
                                    rhs=P[:, 1024 * q + 512 * j:
                                          1024 * q + 512 * (j + 1)],
                                    start=True, stop=True)
                            nc.scalar.activation(
                                out=P[:, 1024 * q:1024 * (q + 1)], in_=pl,
                                func=AF.Exp)
                            for j in range(2):
                                nc.tensor.matmul(
                                    S_ps[:, 1024 * q + 512 * j:
                                         1024 * q + 512 * (j + 1)],
                                    lhsT=srepl,
                                    rhs=P[:, 1024 * q + 512 * j:
                                          1024 * q + 512 * (j + 1)],
                                    start=(ti == 0), stop=(ti == 8))
                        # next tap's P-mult ahead of Pv in the DVE queue;
                        # at the last tap, prefetch the next half's first P
                        if ti < 8:
                            Pnext = emit_p(ti + 1)
                        elif k + 1 < len(halves):
                            Pcarry = emitters[k + 1](0)
                        ev = P.rearrange("p (r w) -> p r w", r=16)
                        Pv = Oab[ti] if ti < 2 else pool.tile([128, HF], bf16,
                                                              tag="Pv")
                        nc.vector.tensor_tensor(
                            out=Pv.rearrange("p (r w) -> p r w", r=16),
                            in0=ev, in1=vwin, op=ALU.mult)
                        if ti >= 2:
                            eng = nc.gpsimd if ti in (3, 5, 7) else nc.vector
                            eng.tensor_tensor(out=Oab[ti % 2], in0=Oab[ti % 2],
                                              in1=Pv, op=ALU.add)
                    with nc.allow_low_precision(reason="softmax recip bf16"):
                        nc.vector.reciprocal(out=rcp, in_=S_ps)
                    nc.vector.tensor_tensor(out=Oab[0], in0=Oab[0], in1=Oab[1],
                                            op=ALU.add)
                    nc.vector.tensor_tensor(out=Oab[0], in0=Oab[0], in1=rcp,
                                            op=ALU.mult)
                    for b in range(NB):
                        nc.sync.dma_start(
                            out=ofull[32 * di:32 * di + 32,
                                      4096 * b + 2048 * hh:
                                      4096 * b + 2048 * hh + 2048],
                            in_=Oab[0][32 * b:32 * b + 32, :])

        # ============ PH4: proj + residual + LN2 stats ============
        r1pool = tc.alloc_tile_pool(name="r1p", bufs=1)
        r1 = r1pool.tile([C, N], bf16)
        with tc.tile_pool(name="ph4", bufs=2) as pool, \
             tc.tile_pool(name="ph4sg", bufs=1) as sgpool, \
             tc.tile_pool(name="ph4st", bufs=1, space="PSUM") as stps, \
             tc.tile_pool(name="ph4ps", bufs=4, space="PSUM") as psum:
            ps_s = ps_q = None
            for c in range(NCHUNK):
                g, i = c // 4, c % 4
                if i == 0:
                    xt4 = pool.tile([C, 4, CH], f32, tag="xt4")
                    nc.sync.dma_start(out=xt4, in_=x_d[:, 16 * g:16 * g + 16, :])
                if c % 2 == 0:
                    ps_s = stps.tile([1, 2 * CH], f32, tag="ps_s4")
                    ps_q = stps.tile([1, 2 * CH], f32, tag="ps_q4")
                h4 = CH * (c % 2)
                pp = psum.tile([C, CH], f32, tag="pp")
                nc.tensor.matmul(pp, lhsT=pbt, rhs=onesrow, start=True, stop=False)
                nc.tensor.matmul(pp, lhsT=wp, rhs=ofull[:, CH * c:CH * (c + 1)],
                                 start=False, stop=True)
                rsl = r1[:, CH * c:CH * (c + 1)]
                nc.vector.tensor_tensor(out=rsl, in0=xt4[:, i, :],
                                        in1=pp, op=ALU.add)
                nc.tensor.matmul(ps_s[:, h4:h4 + CH], lhsT=onescol_b,
                                 rhs=rsl, start=True, stop=True)
                xsq = pool.tile([C, CH], f32r, tag="xsq5")
                nc.vector.tensor_tensor(out=xsq, in0=rsl, in1=rsl, op=ALU.mult)
                nc.tensor.matmul(ps_q[:, h4:h4 + CH], lhsT=onescol,
                                 rhs=xsq, start=True, stop=True)
                if c % 2 == 1:
                    strip_flush(sgpool, ps_s, ps_q, ss2, sq2, c - 1, 2)
                    if c == 31:
                        stats_math(ss2, sq2, rs2, nb2, 0, NCHUNK)

        # ============ PH5: MLP + residual ============
        with tc.tile_pool(name="ph5b", bufs=3) as pool, \
             tc.tile_pool(name="ph5h", bufs=2) as hpool, \
             tc.tile_pool(name="ph5y", bufs=2) as ypool, \
             tc.tile_pool(name="ph5sg", bufs=2) as sgp, \
             tc.tile_pool(name="ph5ps", bufs=3, space="PSUM") as psum, \
             tc.tile_pool(name="ph5ps2", bufs=1, space="PSUM") as psum2:
            srs = snb = None
            for c in range(NCHUNK):
                g, i = c // 4, c % 4
                rsl = r1[:, CH * c:CH * (c + 1)]
                if i == 0:
                    yout4 = ypool.tile([C, 4, CH], f32, tag="yout4")
                if c % 3 == 0:
                    srs, snb = stage_stats(sgp, rs2, nb2, c)
                j = c % 3
                pa = psum2.tile([C, CH], f32, tag="pa5")
                nc.tensor.matmul(pa, lhsT=ones3x[32 * j:32 * j + 1, :],
                                 rhs=srs[32 * j:32 * j + 1, :],
                                 start=True, stop=True)
                pb2 = psum2.tile([C, CH], f32, tag="pb5")
                nc.tensor.matmul(pb2, lhsT=ones3x[32 * j:32 * j + 1, :],
                                 rhs=snb[32 * j:32 * j + 1, :],
                                 start=True, stop=True)
                t1 = pool.tile([C, CH], f32, tag="t15")
                nc.vector.tensor_tensor(out=t1, in0=rsl, in1=pa, op=ALU.mult)
                xn = xns[c % 2]
                nc.vector.tensor_tensor(out=xn[0:C, :], in0=t1,
                                        in1=pb2, op=ALU.add)

                h1 = hpool.tile([128, 3, CH], bf16, tag="h1")
                for j in range(3):
                    pf = psum.tile([128, CH], f32, tag="pf")
                    nc.tensor.matmul(pf, lhsT=w1[:, 128 * j:128 * (j + 1)], rhs=xn,
                                     start=True, stop=True)
                    nc.scalar.activation(out=h1[:, j, :], in_=pf, func=AF.Gelu)
                pm = psum.tile([C, CH], f32, tag="pm")
                nc.tensor.matmul(pm, lhsT=b2t, rhs=onesrow, start=True, stop=False)
                for j in range(3):
                    nc.tensor.matmul(pm, lhsT=w2[j],
                                     rhs=h1[:, j, :], start=False, stop=(j == 2))
                nc.vector.tensor_tensor(out=yout4[:, i, :], in0=rsl,
                                        in1=pm, op=ALU.add)
                if i == 3:
                    nc.sync.dma_start(out=y_d[:, 16 * g:16 * g + 16, :], in_=yout4)

        r1pool.release()
        ofpool.release()
        apool.release()
        wpool.release()

    _split_multi_waits(nc, mybir)
    return nc


def _prep_weights(inputs):
    """Host-side weight preparation (fold LN affine, scale, bias rows)."""
    qkv_w = np.asarray(inputs['qkv_w'], np.float32)       # (288, 96)
    proj_w = np.asarray(inputs['proj_w'], np.float32)     # (96, 96)
    proj_b = np.asarray(inputs['proj_b'], np.float32)
    ln1_w = np.asarray(inputs['ln1_w'], np.float32)
    ln1_b = np.asarray(inputs['ln1_b'], np.float32)
    ln2_w = np.asarray(inputs['ln2_w'], np.float32)
    ln2_b = np.asarray(inputs['ln2_b'], np.float32)
    fc1_w = np.asarray(inputs['fc1_w'], np.float32)       # (384, 96)
    fc1_b = np.asarray(inputs['fc1_b'], np.float32)
    fc2_w = np.asarray(inputs['fc2_w'], np.float32)       # (96, 384)
    fc2_b = np.asarray(inputs['fc2_b'], np.float32)

    wq = qkv_w * ln1_w[None, :]                            # (288, 96)
    c0 = qkv_w @ ln1_b                                     # (288,)
    wq[0:C] *= SCALE                                       # scale q rows
    c0[0:C] *= SCALE
    wqb = np.concatenate([wq.T, c0[None, :]], axis=0)      # (97, 288)

    w1 = fc1_w * ln2_w[None, :]
    c1 = fc1_w @ ln2_b + fc1_b
    w1b = np.concatenate([w1.T, c1[None, :]], axis=0)      # (97, 384)

    repl = np.zeros((128, 128), np.float32)
    for b in range(NB):
        for ch in range(GD):
            h0 = (ch // HD) * HD
            repl[32 * b + h0:32 * b + h0 + HD, 32 * b + ch] = 1.0

    wsum3 = np.zeros((C, 3 * C), np.float32)
    for j in range(3):
        wsum3[32 * j, :] = wq.T[0:C, :].sum(axis=0)        # per-column sums

    return {
        'wsum3': wsum3,
        'wqkv': np.ascontiguousarray(wqb),                 # (97, 288) lhsT
        'wproj': np.ascontiguousarray(proj_w.T),           # (96, 96) lhsT
        'projb': proj_b.reshape(-1, 1).astype(np.float32),
        'w1': np.ascontiguousarray(w1b),                   # (97, 384) lhsT
        'w2': np.ascontiguousarray(fc2_w.T),               # (384, 96) lhsT
        'b2': fc2_b.reshape(-1, 1).astype(np.float32),
        'repl': repl,
        'onesc': np.ones((C, 1), np.float32),
    }


def kernel(**inputs):
    from concourse.bass_utils import run_bass_kernel_spmd

    if 'nc' not in _cache:
        t0 = time.time()
        _cache['nc'] = _build()
        print(f"[kernel] built bass module in {time.time() - t0:.1f}s",
              file=sys.stderr)

    nc = _cache['nc']
    wmap = _prep_weights(inputs)
    x = np.asarray(inputs['x'], np.float32)                # (8, 96, 128, 128)

    in_maps = []
    for b in range(B):
        m = {'x': np.ascontiguousarray(x[b])}
        m.update(wmap)
        in_maps.append(m)

    res = run_bass_kernel_spmd(nc, in_maps, core_ids=list(range(B)))
    _cache['last_exec_ns'] = res.exec_time_ns
    out = np.stack([res.results[b]['y'] for b in range(B)], axis=0)
    return out.astype(np.float32)
